# revision 1
# baseline (speedup 1.0000x reference)
"""8-NeuronCore Trainium2 Bass kernel for nn_AttentionBlock_17789754540111.

Self-contained: builds a hand-written Bass/Tile SPMD program (H-sharded over
8 cores, params replicated, instance-norm stats combined via on-device
AllReduce) and runs it on the 8 axon-tunneled TRN2 cores via
concourse.bass_utils.run_bass_kernel_spmd.
"""

import sys
import types
import ctypes
import contextlib

# ---- NTFF profile hook expected by bass_utils under axon ----------------
def _install_axon_hook():
    if "antenv.axon_hooks" in sys.modules:
        return
    hookmod = types.ModuleType("antenv.axon_hooks")

    def _make_hook():
        try:
            lib = ctypes.CDLL("/opt/axon/libaxon_pjrt.so")
        except OSError:
            return None
        if not hasattr(lib, "axon_start_nrt_profile"):
            return None
        lib.axon_start_nrt_profile.argtypes = [ctypes.POINTER(ctypes.c_int64),
                                               ctypes.c_size_t]
        lib.axon_start_nrt_profile.restype = ctypes.c_int64
        lib.axon_stop_nrt_profile.argtypes = [ctypes.c_char_p]
        lib.axon_stop_nrt_profile.restype = ctypes.c_int64

        @contextlib.contextmanager
        def _hook(output_dir, device_ids):
            import jax
            jax.devices()
            if device_ids:
                ids = (ctypes.c_int64 * len(device_ids))(*device_ids)
                rc = lib.axon_start_nrt_profile(ids, len(device_ids))
            else:
                rc = lib.axon_start_nrt_profile(None, 0)
            if rc != 0:
                raise RuntimeError(f"axon_start_nrt_profile rc={rc}")
            try:
                yield
            finally:
                lib.axon_stop_nrt_profile(str(output_dir).encode())
        return _hook

    hook = _make_hook()
    hookmod.get_axon_ntff_profile_hook = lambda: hook
    hookmod.set_axon_ntff_profile_hook = lambda h: None
    sys.modules["antenv.axon_hooks"] = hookmod

_install_axon_hook()

import math
import sys

sys.path.insert(0, "/opt/trn_rl_repo")

import numpy as np

import concourse.bass as bass  # noqa
import concourse.bacc as bacc
import concourse.mybir as mybir
from concourse import tile

F32 = mybir.dt.float32
BF16 = mybir.dt.bfloat16
AF = mybir.ActivationFunctionType
ALU = mybir.AluOpType
AX = mybir.AxisListType

T, B, H, W, C = 64, 2, 32, 32, 128
HE, HD = 8, 16
NCORES = 8
HL = H // NCORES
PIX = B * HL * W                  # 256
NP_ = T * PIX                     # 16384
NSAMP = T * B                     # 128
SPATIAL = H * W                   # 1024 (global)
EPS = 1e-5
VST = 136
NPAIR = PIX // 2                  # 128


def _rel_bias_table(rel_emb):
    rp = np.arange(T)[None, :] - np.arange(T)[:, None]
    n = -rp
    ret = (n < 0).astype(np.int64) * 16
    n = np.abs(n)
    mx = 8
    small = n < mx
    vl = mx + (np.log(np.maximum(n, 1) / mx) / math.log(32 / mx) * 8).astype(np.int64)
    vl = np.minimum(vl, 15)
    buckets = ret + np.where(small, n, vl)
    return np.ascontiguousarray(
        rel_emb[buckets].transpose(2, 0, 1)).astype(np.float32)  # (he, tq, tk)


def host_prep(inputs):
    w_in = np.asarray(inputs["w_in"], np.float32)
    b_in = np.asarray(inputs["b_in"], np.float32)
    rows = w_in.reshape(HE, 3, HD, C)
    b3 = b_in.reshape(HE, 3, HD)
    wq = rows[:, 0].reshape(HE * HD, C)
    wk = rows[:, 1].reshape(HE * HD, C)
    wv = rows[:, 2].reshape(HE * HD, C)

    bias = _rel_bias_table(np.asarray(inputs["rel_emb"], np.float32))
    eb = np.exp(bias)                                          # (he, tq, tk)
    expb, wo, n2w, n2b = {}, {}, {}, {}
    n2w_full = np.asarray(inputs["norm2_w"], np.float32)
    n2b_full = np.asarray(inputs["norm2_b"], np.float32)
    w_out = np.asarray(inputs["w_out"], np.float32)
    for eo in (0, 1):
        heads = [2 * j + eo for j in range(4)]
        h = eb[heads]                                          # (4, tq, tk)
        # col = j4*128 + jj*64 + tq ; rows = 2 px * 64 tk
        e1 = h.transpose(2, 0, 1)                              # (tk, j, tq)
        e2 = np.stack([e1, e1], axis=2).reshape(T, 4 * 2 * T)  # (tk, (j, jj, tq))
        expb[eo] = np.concatenate([e2, e2], axis=0).astype(np.float32)
        m = np.zeros((128, 128), np.float32)
        wv_ = np.zeros((128, 1), np.float32)
        bv_ = np.zeros((128, 1), np.float32)
        for j, he in enumerate(heads):
            m[32 * j: 32 * j + 16, :] = w_out[:, he * 16: (he + 1) * 16].T
            wv_[32 * j: 32 * j + 16, 0] = n2w_full[he * 16: (he + 1) * 16]
            bv_[32 * j: 32 * j + 16, 0] = n2b_full[he * 16: (he + 1) * 16]
        wo[eo], n2w[eo], n2b[eo] = m, wv_, bv_

    J = np.kron(np.eye(HE, dtype=np.float32), np.ones((HD, HD), np.float32))
    E4 = np.zeros((4, 128), np.float32)
    for s in range(4):
        E4[s, 32 * s: 32 * s + 16] = 1.0

    col = lambda a: np.asarray(a, np.float32).reshape(-1, 1)
    return {
        "ident": np.eye(128, dtype=np.float32),
        "identb": np.eye(128, dtype=np.float32),
        "wqT": wq.T.copy(), "wkT": wk.T.copy(), "wvT": wv.T.copy(),
        "bq": col(b3[:, 0].reshape(-1)), "bk": col(b3[:, 1].reshape(-1)),
        "bvrow": np.broadcast_to(b3[:, 2].reshape(-1), (128, 128)).copy(),
        "expbE": expb[0], "expbO": expb[1],
        "J": J, "E4": E4, "woE": wo[0], "woO": wo[1],
        "b_o": col(inputs["b_out"]), "gamma": col(inputs["gamma"]),
        "n1w": col(inputs["norm1_w"]), "n1b": col(inputs["norm1_b"]),
        "epscol": np.full((128, 1), EPS, np.float32),
        "n2wE": n2w[0], "n2bE": n2b[0], "n2wO": n2w[1], "n2bO": n2b[1],
        "qnw": col(np.tile(np.asarray(inputs["qn_w"], np.float32), HE)),
        "qnb": col(np.tile(np.asarray(inputs["qn_b"], np.float32), HE)),
        "knw": col(np.tile(np.asarray(inputs["kn_w"], np.float32), HE)),
        "knb": col(np.tile(np.asarray(inputs["kn_b"], np.float32), HE)),
    }


_BF16_IN = {"wqT", "wkT", "wvT", "bvrow", "expbE", "expbO", "J", "E4",
            "woE", "woO", "identb"}
_CONST_SHAPES = {
    "ident": (128, 128), "identb": (128, 128), "wqT": (128, 128), "wkT": (128, 128), "wvT": (128, 128),
    "bq": (128, 1), "bk": (128, 1), "bvrow": (128, 128),
    "expbE": (128, 512), "expbO": (128, 512), "J": (128, 128), "E4": (4, 128),
    "woE": (128, 128), "woO": (128, 128), "b_o": (128, 1), "gamma": (128, 1),
    "n1w": (128, 1), "n1b": (128, 1), "epscol": (128, 1),
    "n2wE": (128, 1), "n2bE": (128, 1), "n2wO": (128, 1), "n2bO": (128, 1),
    "qnw": (128, 1), "qnb": (128, 1), "knw": (128, 1), "knb": (128, 1),
}


def build_nc(ln_general=True):
    nc = bacc.Bacc("TRN2", target_bir_lowering=False, debug=False,
                   enable_asserts=False, num_devices=NCORES)
    ins = {"x": nc.dram_tensor("x", [NP_, C], F32, kind="ExternalInput").ap()}
    for name, shp in _CONST_SHAPES.items():
        dt = BF16 if name in _BF16_IN else F32
        ins[name] = nc.dram_tensor(name, list(shp), dt, kind="ExternalInput").ap()
    y = nc.dram_tensor("y", [NP_, C], F32, kind="ExternalOutput").ap()
    with tile.TileContext(nc) as tc:
        _body(tc, nc, ins, y, ln_general)
    nc.compile()
    return nc


def _body(tc, nc, ins, y, ln_general):
    x = ins["x"]
    pools = {}

    def pool(name, bufs=1, space="SBUF"):
        if name not in pools:
            pools[name] = tc.alloc_tile_pool(name=name, bufs=bufs, space=space)
        return pools[name]

    cp = pool("consts")
    big = pool("big")
    ps = pool("psA", bufs=2, space="PSUM")
    ps_s = pool("psS", bufs=1, space="PSUM")
    ps_t = pool("psT", bufs=1, space="PSUM")
    dram = pool("dram", bufs=1, space="DRAM")
    sm = pool("small", bufs=1)
    ldp = pool("ldp", bufs=2)
    lnp = pool("lnp", bufs=1)
    esp = pool("esp", bufs=2)
    outp = pool("outp", bufs=2)

    cst = {}
    for name in _CONST_SHAPES:
        ap = ins[name]
        t = cp.tile(list(ap.shape), ap.dtype, tag=name)
        nc.sync.dma_start(t[:], ap)
        cst[name] = t

    # Big slots: S1 {xT, v_t} / S2 {kk} / S3 {xn, attH} / S4 {q} / S5 {sq, kEO}
    xT = big.tile([128, VST * NPAIR + 32], BF16, tag="S1", name="xT")[:, :NP_]
    q = big.tile([128, NP_], BF16, tag="S4")
    kk = big.tile([128, NP_], BF16, tag="S2")

    # ---------------- Phase 0: load + transpose --------------------------
    for k8 in range(32):
        ld = ldp.tile([128, 512], F32, tag="ld")
        src = x.rearrange("(k8 k p) c -> k8 p k c", p=128, k=4)[k8]
        nc.sync.dma_start(ld[:].rearrange("p (k c) -> p k c", k=4), src)
        for j in range(4):
            k = 4 * k8 + j
            pt = ps_t.tile([128, 128], F32, tag="t", name="tp")
            nc.tensor.transpose(pt[:], ld[:, 128 * j: 128 * (j + 1)], cst["ident"][:])
            if j % 2 == 0:
                nc.vector.tensor_copy(xT[:, 128 * k: 128 * (k + 1)], pt[:])
            else:
                nc.scalar.copy(xT[:, 128 * k: 128 * (k + 1)], pt[:])

    # ---------------- Phase 1: norm1 stats + AllReduce -------------------
    s1 = sm.tile([128, NSAMP], F32, tag="s1")
    s2 = sm.tile([128, NSAMP], F32, tag="s2")
    sq = big.tile([128, NP_], BF16, tag="S5")
    nc.vector.reduce_sum(s1[:], xT[:].rearrange("p (k s) -> p k s", s=128), axis=AX.X)
    nc.scalar.square(sq[:], xT[:])
    nc.vector.reduce_sum(s2[:], sq[:].rearrange("p (k s) -> p k s", s=128), axis=AX.X)

    stats = sm.tile([128, 2 * NSAMP], F32, tag="st2")
    nc.vector.tensor_copy(stats[:, :NSAMP], s1[:])
    nc.vector.tensor_copy(stats[:, NSAMP:], s2[:])
    cc_in = dram.tile([128, 2 * NSAMP], F32, tag="cc_in")
    cc_out = dram.tile([128, 2 * NSAMP], F32, tag="cc_out")
    nc.gpsimd.dma_start(cc_in[:], stats[:])
    nc.gpsimd.collective_compute("AllReduce", ALU.add,
                                 replica_groups=[list(range(NCORES))],
                                 ins=[cc_in[:].opt()], outs=[cc_out[:].opt()])
    nc.sync.dma_start(stats[:], cc_out[:])

    mean = sm.tile([128, NSAMP], F32, tag="m2")
    scl1 = sm.tile([128, NSAMP], F32, tag="sc2")
    sft1 = sm.tile([128, NSAMP], F32, tag="sf2")
    tmp = sm.tile([128, NSAMP], F32, tag="t2")

    def norm_coeffs(st, scl, sft, mn, tp, w_ap, b_ap):
        nc.vector.tensor_scalar_mul(mn[:], st[:, :NSAMP], 1.0 / SPATIAL)
        nc.vector.tensor_scalar_mul(tp[:], st[:, NSAMP:], 1.0 / SPATIAL)
        nc.vector.tensor_tensor(out=scl[:], in0=mn[:], in1=mn[:], op=ALU.mult)
        nc.vector.tensor_tensor(out=tp[:], in0=tp[:], in1=scl[:], op=ALU.subtract)
        nc.scalar.activation(tp[:], tp[:], AF.Sqrt, bias=cst["epscol"][:, 0:1], scale=1.0)
        nc.vector.reciprocal(scl[:], tp[:])
        nc.vector.tensor_scalar_mul(scl[:], scl[:], w_ap[:, 0:1])
        nc.vector.tensor_tensor(out=sft[:], in0=mn[:], in1=scl[:], op=ALU.mult)
        nc.vector.tensor_scalar(out=sft[:], in0=sft[:], scalar1=-1.0,
                                scalar2=b_ap[:, 0:1], op0=ALU.mult, op1=ALU.add)

    norm_coeffs(stats, scl1, sft1, mean, tmp, cst["n1w"], cst["n1b"])

    # ---------------- Phase 2: norm1 apply -------------------------------
    xn = big.tile([128, NP_], BF16, tag="S3")
    for k in range(NSAMP):
        sl = slice(128 * k, 128 * (k + 1))
        nc.vector.tensor_scalar(out=xn[:, sl], in0=xT[:, sl],
                                scalar1=scl1[:, k: k + 1], scalar2=sft1[:, k: k + 1],
                                op0=ALU.mult, op1=ALU.add)

    # ---------------- Phase 3: conv1 q, k --------------------------------
    for n in range(32):
        sl = slice(512 * n, 512 * (n + 1))
        pq = ps.tile([128, 512], F32, tag="a", name="pq")
        nc.tensor.matmul(pq[:], cst["wqT"][:], xn[:, sl], start=True, stop=True)
        nc.vector.tensor_scalar_add(q[:, sl], pq[:], cst["bq"][:, 0:1])
        pk = ps_s.tile([128, 512], F32, tag="s", name="pk")
        nc.tensor.matmul(pk[:], cst["wkT"][:], xn[:, sl], start=True, stop=True)
        nc.scalar.add(kk[:, sl], pk[:], cst["bk"][:, 0:1])

    # ---------------- Phase 4: LN on q and k (chunked) -------------------
    def layernorm(tt, w_ap, b_ap):
        for n in range(16):
            base = 1024 * n
            mu = lnp.tile([128, 1024], BF16, tag="ln_mu")
            vv = lnp.tile([128, 1024], BF16, tag="ln_vv")
            t1 = lnp.tile([128, 1024], BF16, tag="ln_t1")
            for c4 in range(2):
                sl = slice(base + 512 * c4, base + 512 * (c4 + 1))
                sll = slice(512 * c4, 512 * (c4 + 1))
                nc.scalar.square(t1[:, sll], tt[:, sl])
                pm = ps.tile([128, 512], F32, tag="a", name="pm")
                nc.tensor.matmul(pm[:], cst["J"][:], tt[:, sl], start=True, stop=True)
                pv = ps_s.tile([128, 512], F32, tag="s", name="pv")
                nc.tensor.matmul(pv[:], cst["J"][:], t1[:, sll], start=True, stop=True)
                nc.scalar.mul(mu[:, sll], pm[:], 1.0 / HD)
                nc.scalar.activation(vv[:, sll], pv[:], AF.Copy, bias=0.0,
                                     scale=1.0 / HD)
            nc.vector.tensor_tensor(out=t1[:], in0=mu[:], in1=mu[:], op=ALU.mult)
            nc.vector.tensor_tensor(out=vv[:], in0=vv[:], in1=t1[:], op=ALU.subtract)
            nc.scalar.activation(vv[:], vv[:], AF.Ln, bias=cst["epscol"][:, 0:1], scale=1.0)
            nc.scalar.activation(vv[:], vv[:], AF.Exp, bias=0.0, scale=-0.5)
            sl = slice(base, base + 1024)
            nc.vector.tensor_tensor(out=t1[:], in0=tt[:, sl], in1=mu[:],
                                    op=ALU.subtract)
            nc.vector.tensor_tensor(out=tt[:, sl], in0=t1[:], in1=vv[:], op=ALU.mult)
            if ln_general:
                nc.vector.tensor_scalar(out=tt[:, sl], in0=tt[:, sl],
                                        scalar1=w_ap[:, 0:1], scalar2=b_ap[:, 0:1],
                                        op0=ALU.mult, op1=ALU.add)

    layernorm(q, cst["qnw"], cst["qnb"])
    layernorm(kk, cst["knw"], cst["knb"])

    # ---------------- Phase 5: conv v (transposed layout) ----------------
    v_t = big.tile([128, VST * NPAIR + 32], BF16, tag="S1")
    nc.vector.memset(v_t[:], 0.0)
    ones_ap = v_t[:, :VST * NPAIR].rearrange(
        "p (m e d) -> p m e d", e=8, d=17)[:, :, :, 16:17]
    nc.vector.memset(ones_ap, 1.0)
    xnv = xn[:].rearrange("c (t m z) -> c t m z", m=NPAIR, z=2)
    for m in range(NPAIR):
        pv = ps_t.tile([128, 128], F32, tag="t", name="pvt")
        for z in range(2):
            nc.tensor.matmul(pv[64 * z: 64 * z + 64, :], xnv[:, :, m, z],
                             cst["wvT"][:], start=True, stop=True,
                             tile_position=(0, 64 * z))
        dst = v_t[:, VST * m: VST * m + 136].rearrange(
            "p (e d) -> p e d", d=17)[:, :, 0:16]
        nc.vector.tensor_tensor(out=dst,
                                in0=pv[:].rearrange("p (e d) -> p e d", d=16),
                                in1=cst["bvrow"][:].rearrange("p (e d) -> p e d", d=16),
                                op=ALU.add)

    # ---------------- Phases 6-8: attention (two head parities) ----------
    attH = big.tile([128, NP_], BF16, tag="S3")
    attE_d = dram.tile([128, NP_], BF16, tag="attE_d")
    kE_d = dram.tile([128, NP_], BF16, tag="kE_d")
    kO_d = dram.tile([128, NP_], BF16, tag="kO_d")
    rs_d = dram.tile([4, NP_], F32, tag="rs_d")
    qv = q[:].rearrange("a (t p) -> a t p", p=256)
    kEO = big.tile([128, NP_], BF16, tag="S5")
    kv = kEO[:].rearrange("a (t p) -> a t p", p=256)
    # Build zero-padded parity copies of k in DRAM (DMA-only: engine ops
    # cannot address 16-row partition groups; SBUF-strided DMA APs defeat
    # the race tracker, so stride only on the DRAM side).
    k_d = dram.tile([128, NP_], BF16, tag="k_d")
    nc.vector.memset(kEO[:], 0.0)
    nc.sync.dma_start(k_d[:], kk[:])
    nc.sync.dma_start(kE_d[:], kEO[:])
    nc.sync.dma_start(kO_d[:], kEO[:])
    for par, kd in ((0, kE_d), (1, kO_d)):
        src = k_d[:].rearrange("(s r) n -> s r n", r=32)[:, 16 * par: 16 * par + 16, :]
        dst = kd[:].rearrange("(s r) n -> s r n", r=32)[:, 16 * par: 16 * par + 16, :]
        nc.sync.dma_start(dst, src)

    def att_pass(eo, expb_c, woname, wname, bname):
        # build kEO for this parity
        nc.sync.dma_start(kEO[:], (kE_d if eo == 0 else kO_d)[:])

        for grp in range(32):
            pa0 = ps.tile([128, 256], F32, tag="pa0", name="pa0", bufs=1)
            pa1 = ps.tile([128, 256], F32, tag="pa1", name="pa1", bufs=1)
            for sg in range(2):
                pairs = (8 * grp + 4 * sg, 8 * grp + 4 * sg + 2)
                es = esp.tile([128, 512], BF16, tag="es")
                for rr in range(2):
                    qk0 = ps_s.tile([128, 128], F32, tag="qk0", name="qk0", bufs=1)
                    qk1 = ps_s.tile([128, 128], F32, tag="qk1", name="qk1", bufs=1)
                    for rb, qkb in ((0, qk0), (1, qk1)):
                        r = 2 * rr + rb
                        prt = slice(32 * r, 32 * r + 32)
                        for jj, pbase in enumerate(pairs):
                            for h01 in range(2):
                                p = pbase + h01
                                for tkc in range(2):
                                    nc.tensor.matmul(
                                        qkb[64 * h01 + 32 * tkc:
                                            64 * h01 + 32 * tkc + 32,
                                            64 * jj: 64 * jj + 64],
                                        kv[prt, 32 * tkc: 32 * tkc + 32, p],
                                        qv[prt, :, p], start=True, stop=True,
                                        tile_position=(32 * r,
                                                       64 * h01 + 32 * tkc))
                        nc.scalar.activation(
                            es[:, 128 * r: 128 * r + 128], qkb[:], AF.Exp,
                            bias=0.0, scale=0.25)
                nc.vector.tensor_tensor(out=es[:], in0=es[:], in1=expb_c[:],
                                        op=ALU.mult)
                for jj, pbase in enumerate(pairs):
                    pair = pbase // 2
                    lp = 2 * sg + jj
                    for h01, pah in ((0, pa0), (1, pa1)):
                        for j4 in range(4):
                            eg = 2 * j4 + eo
                            lhs_v = v_t[64 * h01: 64 * h01 + 64,
                                        VST * pair + 17 * eg: VST * pair + 17 * eg + 32]
                            nc.tensor.matmul(
                                pah[32 * j4: 32 * j4 + 32, 64 * lp: 64 * lp + 64],
                                lhs_v,
                                es[64 * h01: 64 * h01 + 64,
                                   128 * j4 + 64 * jj: 128 * j4 + 64 * jj + 64],
                                start=True, stop=True,
                                tile_position=(64 * h01, 32 * j4))
            for h01, pah in ((0, pa0), (1, pa1)):
                dstv = attH[:, 512 * grp: 512 * (grp + 1)].rearrange(
                    "c (lp z) -> c lp z", z=128)[:, :, 64 * h01: 64 * h01 + 64]
                srcv = pah[:].rearrange("c (lp z) -> c lp z", z=64)
                if h01 == 0:
                    nc.vector.tensor_copy(dstv, srcv)
                else:
                    nc.scalar.copy(dstv, srcv)

        # rowsum division
        rs = sm.tile([128, 512], F32, tag="rs")
        for s in range(4):
            nc.gpsimd.dma_start(rs_d[s: s + 1, :], attH[32 * s + 16: 32 * s + 17, :])
        nc.sync.dma_start(rs[:], rs_d[:].rearrange("e (g n) -> (e g) n", n=512))
        nc.vector.reciprocal(rs[:], rs[:])
        nc.gpsimd.dma_start(rs_d[:].rearrange("e (g n) -> (e g) n", n=512), rs[:])
        for n in range(32):
            sl = slice(512 * n, 512 * (n + 1))
            rs8 = esp.tile([4, 512], BF16, tag="rs8")
            nc.gpsimd.dma_start(rs8[:], rs_d[:, sl])
            pb = ps_s.tile([128, 512], F32, tag="s", name="pb")
            nc.tensor.matmul(pb[:], cst["E4"][:], rs8[:], start=True, stop=True)
            rbc = esp.tile([128, 512], BF16, tag="rbc")
            nc.scalar.copy(rbc[:], pb[:])
            nc.vector.tensor_tensor(out=attH[:, sl], in0=attH[:, sl], in1=rbc[:],
                                    op=ALU.mult)

        # norm2 stats (chunked; cols of chunk n = pixels 8n..8n+8, b = n//16)
        s1b = sm.tile([128, NSAMP], F32, tag="s1")
        s2b = sm.tile([128, NSAMP], F32, tag="s2")
        nc.vector.memset(s1b[:], 0.0)
        nc.vector.memset(s2b[:], 0.0)
        for n in range(32):
            sl = slice(512 * n, 512 * (n + 1))
            b = n // 16
            ssl = slice(64 * b, 64 * b + 64)
            sqc = esp.tile([128, 512], BF16, tag="sqc")
            nc.scalar.square(sqc[:], attH[:, sl])
            p1 = sm.tile([128, T], F32, tag="p1")
            p2 = sm.tile([128, T], F32, tag="p2")
            nc.vector.reduce_sum(
                p1[:], attH[:, sl].rearrange("c (p t) -> c t p", p=8), axis=AX.X)
            nc.vector.reduce_sum(
                p2[:], sqc[:].rearrange("c (p t) -> c t p", p=8), axis=AX.X)
            nc.vector.tensor_tensor(out=s1b[:, ssl], in0=s1b[:, ssl], in1=p1[:],
                                    op=ALU.add)
            nc.vector.tensor_tensor(out=s2b[:, ssl], in0=s2b[:, ssl], in1=p2[:],
                                    op=ALU.add)
        st2 = sm.tile([128, 2 * NSAMP], F32, tag="st2")
        nc.vector.tensor_copy(st2[:, :NSAMP], s1b[:])
        nc.vector.tensor_copy(st2[:, NSAMP:], s2b[:])
        cc2i = dram.tile([128, 2 * NSAMP], F32, tag="cc_in")
        cc2o = dram.tile([128, 2 * NSAMP], F32, tag="cc_out")
        nc.gpsimd.dma_start(cc2i[:], st2[:])
        nc.gpsimd.collective_compute("AllReduce", ALU.add,
                                     replica_groups=[list(range(NCORES))],
                                     ins=[cc2i[:].opt()], outs=[cc2o[:].opt()])
        nc.sync.dma_start(st2[:], cc2o[:])
        sc2 = sm.tile([128, NSAMP], F32, tag="sc2")
        sf2 = sm.tile([128, NSAMP], F32, tag="sf2")
        m2 = sm.tile([128, NSAMP], F32, tag="m2")
        t2 = sm.tile([128, NSAMP], F32, tag="t2")
        norm_coeffs(st2, sc2, sf2, m2, t2, cst[wname], cst[bname])
        attv = attH[:].rearrange("c (b s t) -> c b s t", b=2, s=128)
        for b in range(2):
            for t in range(T):
                colap = attv[:, b, :, t]
                j = 64 * b + t
                nc.vector.tensor_scalar(out=colap, in0=colap,
                                        scalar1=sc2[:, j: j + 1],
                                        scalar2=sf2[:, j: j + 1],
                                        op0=ALU.mult, op1=ALU.add)
        if eo == 0:
            nc.sync.dma_start(attE_d[:], attH[:])

    att_pass(0, cst["expbE"], "woE", "n2wE", "n2bE")
    att_pass(1, cst["expbO"], "woO", "n2wO", "n2bO")

    # ---------------- Phase 9: conv2 + gamma + residual + store ----------
    yv = y.rearrange("(t m z) c -> t m z c", m=NPAIR, z=2)
    xvr = x.rearrange("(t m z) c -> t m z c", m=NPAIR, z=2)
    for n in range(32):
        sl = slice(512 * n, 512 * (n + 1))
        aA = outp.tile([128, 512], BF16, tag="aA")
        nc.sync.dma_start(aA[:], attE_d[:, sl])
        po = ps.tile([128, 512], F32, tag="a", name="po")
        nc.tensor.matmul(po[:], cst["woE"][:], aA[:], start=True, stop=False)
        nc.tensor.matmul(po[:], cst["woO"][:], attH[:, sl], start=False, stop=True)
        yb = outp.tile([128, 512], BF16, tag="yb")
        nc.vector.tensor_scalar(out=yb[:], in0=po[:], scalar1=cst["b_o"][:, 0:1],
                                scalar2=cst["gamma"][:, 0:1], op0=ALU.add, op1=ALU.mult)
        xr = outp.tile([128, 512], F32, tag="xr")
        for z in range(2):
            nc.sync.dma_start(
                xr[64 * z: 64 * z + 64, :].rearrange("t (m c) -> t m c", m=4),
                xvr[:, 4 * n: 4 * n + 4, z, :])
        y8 = outp.tile([128, 512], F32, tag="y8")
        for j in range(4):
            pt = ps_t.tile([128, 128], BF16, tag="t", name="tp2")
            nc.tensor.transpose(pt[:], yb[:, 128 * j: 128 * (j + 1)], cst["identb"][:])
            nc.vector.tensor_tensor(out=y8[:, 128 * j: 128 * (j + 1)], in0=pt[:],
                                    in1=xr[:, 128 * j: 128 * (j + 1)], op=ALU.add)
        for z in range(2):
            nc.sync.dma_start(
                yv[:, 4 * n: 4 * n + 4, z, :],
                y8[64 * z: 64 * z + 64, :].rearrange("t (m c) -> t m c", m=4))

    for p_ in reversed(list(pools.values())):
        p_.release()


# ---- public entry point -------------------------------------------------
_NC = None


def _get_nc():
    global _NC
    if _NC is None:
        _NC = build_nc(ln_general=True)
    return _NC


def kernel(**inputs) -> np.ndarray:
    import ml_dtypes
    from concourse import bass_utils

    nc = _get_nc()
    consts = host_prep(inputs)
    cmap = {}
    for name, val in consts.items():
        v = np.asarray(val, np.float32)
        if name in _BF16_IN:
            v = v.astype(ml_dtypes.bfloat16)
        cmap[name] = v
    x = np.asarray(inputs["x"], np.float32)
    in_maps = []
    for c in range(NCORES):
        m = dict(cmap)
        m["x"] = np.ascontiguousarray(
            x[:, :, HL * c: HL * (c + 1), :, :].reshape(-1, C))
        in_maps.append(m)
    res = bass_utils.run_bass_kernel_spmd(
        nc, in_maps, core_ids=list(range(NCORES)), trace=False)
    y = np.zeros((T, B, H, W, C), np.float32)
    for c in range(NCORES):
        y[:, :, HL * c: HL * (c + 1), :, :] = \
            res.results[c]["y"].reshape(T, B, HL, W, C)
    return y


def kernel_traced(**inputs):
    """Like kernel() but returns (y, per_core_exec_ns, trace_path)."""
    import ml_dtypes
    from concourse import bass_utils

    nc = _get_nc()
    consts = host_prep(inputs)
    cmap = {}
    for name, val in consts.items():
        v = np.asarray(val, np.float32)
        if name in _BF16_IN:
            v = v.astype(ml_dtypes.bfloat16)
        cmap[name] = v
    x = np.asarray(inputs["x"], np.float32)
    in_maps = []
    for c in range(NCORES):
        m = dict(cmap)
        m["x"] = np.ascontiguousarray(
            x[:, :, HL * c: HL * (c + 1), :, :].reshape(-1, C))
        in_maps.append(m)
    res = bass_utils.run_bass_kernel_spmd(
        nc, in_maps, core_ids=list(range(NCORES)),
        trace=True, trace_cores=list(range(NCORES)))
    y = np.zeros((T, B, H, W, C), np.float32)
    for c in range(NCORES):
        y[:, :, HL * c: HL * (c + 1), :, :] = \
            res.results[c]["y"].reshape(T, B, HL, W, C)
    trace_path = (res.instructions_and_trace[1]
                  if res.instructions_and_trace else None)
    return y, res.exec_time_ns, trace_path



# revision 11
# speedup vs baseline: 1.1158x; 1.1158x over previous
"""8-NeuronCore Trainium2 Bass kernel for nn_AttentionBlock_17789754540111.

Self-contained: builds a hand-written Bass/Tile SPMD program (H-sharded over
8 cores, params replicated, instance-norm stats combined via on-device
AllReduce) and runs it on the 8 axon-tunneled TRN2 cores via
concourse.bass_utils.run_bass_kernel_spmd.
"""

import sys
import types
import ctypes
import contextlib

# ---- NTFF profile hook expected by bass_utils under axon ----------------
def _install_axon_hook():
    if "antenv.axon_hooks" in sys.modules:
        return
    hookmod = types.ModuleType("antenv.axon_hooks")

    def _make_hook():
        try:
            lib = ctypes.CDLL("/opt/axon/libaxon_pjrt.so")
        except OSError:
            return None
        if not hasattr(lib, "axon_start_nrt_profile"):
            return None
        lib.axon_start_nrt_profile.argtypes = [ctypes.POINTER(ctypes.c_int64),
                                               ctypes.c_size_t]
        lib.axon_start_nrt_profile.restype = ctypes.c_int64
        lib.axon_stop_nrt_profile.argtypes = [ctypes.c_char_p]
        lib.axon_stop_nrt_profile.restype = ctypes.c_int64

        @contextlib.contextmanager
        def _hook(output_dir, device_ids):
            import jax
            jax.devices()
            if device_ids:
                ids = (ctypes.c_int64 * len(device_ids))(*device_ids)
                rc = lib.axon_start_nrt_profile(ids, len(device_ids))
            else:
                rc = lib.axon_start_nrt_profile(None, 0)
            if rc != 0:
                raise RuntimeError(f"axon_start_nrt_profile rc={rc}")
            try:
                yield
            finally:
                lib.axon_stop_nrt_profile(str(output_dir).encode())
        return _hook

    hook = _make_hook()
    hookmod.get_axon_ntff_profile_hook = lambda: hook
    hookmod.set_axon_ntff_profile_hook = lambda h: None
    sys.modules["antenv.axon_hooks"] = hookmod

_install_axon_hook()

import math
import sys

sys.path.insert(0, "/opt/trn_rl_repo")

import numpy as np

import concourse.bass as bass  # noqa
import concourse.bacc as bacc
import concourse.mybir as mybir
from concourse import tile

F32 = mybir.dt.float32
BF16 = mybir.dt.bfloat16
AF = mybir.ActivationFunctionType
ALU = mybir.AluOpType
AX = mybir.AxisListType

T, B, H, W, C = 64, 2, 32, 32, 128
HE, HD = 8, 16
NCORES = 8
HL = H // NCORES
PIX = B * HL * W                  # 256
NP_ = T * PIX                     # 16384
NSAMP = T * B                     # 128
SPATIAL = H * W                   # 1024 (global)
EPS = 1e-5
VST = 136
NPAIR = PIX // 2                  # 128


def _rel_bias_table(rel_emb):
    rp = np.arange(T)[None, :] - np.arange(T)[:, None]
    n = -rp
    ret = (n < 0).astype(np.int64) * 16
    n = np.abs(n)
    mx = 8
    small = n < mx
    vl = mx + (np.log(np.maximum(n, 1) / mx) / math.log(32 / mx) * 8).astype(np.int64)
    vl = np.minimum(vl, 15)
    buckets = ret + np.where(small, n, vl)
    return np.ascontiguousarray(
        rel_emb[buckets].transpose(2, 0, 1)).astype(np.float32)  # (he, tq, tk)


def host_prep(inputs):
    w_in = np.asarray(inputs["w_in"], np.float32)
    b_in = np.asarray(inputs["b_in"], np.float32)
    rows = w_in.reshape(HE, 3, HD, C)
    b3 = b_in.reshape(HE, 3, HD)
    wq = rows[:, 0].reshape(HE * HD, C)
    wk = rows[:, 1].reshape(HE * HD, C)
    wv = rows[:, 2].reshape(HE * HD, C)

    bias = _rel_bias_table(np.asarray(inputs["rel_emb"], np.float32))
    eb = np.exp(bias)                                          # (he, tq, tk)
    expb, wo, n2w, n2b = {}, {}, {}, {}
    n2w_full = np.asarray(inputs["norm2_w"], np.float32)
    n2b_full = np.asarray(inputs["norm2_b"], np.float32)
    w_out = np.asarray(inputs["w_out"], np.float32)
    for eo in (0, 1):
        heads = [2 * j + eo for j in range(4)]
        h = eb[heads]                                          # (4, tq, tk)
        # col = j4*128 + jj*64 + tq ; rows = 2 px * 64 tk
        e1 = h.transpose(2, 0, 1)                              # (tk, j, tq)
        e2 = np.stack([e1, e1], axis=2).reshape(T, 4 * 2 * T)  # (tk, (j, jj, tq))
        expb[eo] = np.concatenate([e2, e2], axis=0).astype(np.float32)
        m = np.zeros((128, 128), np.float32)
        wv_ = np.zeros((128, 1), np.float32)
        bv_ = np.zeros((128, 1), np.float32)
        for j, he in enumerate(heads):
            m[32 * j: 32 * j + 16, :] = w_out[:, he * 16: (he + 1) * 16].T
            wv_[32 * j: 32 * j + 16, 0] = n2w_full[he * 16: (he + 1) * 16]
            bv_[32 * j: 32 * j + 16, 0] = n2b_full[he * 16: (he + 1) * 16]
        wo[eo], n2w[eo], n2b[eo] = m, wv_, bv_

    J = np.kron(np.eye(HE, dtype=np.float32), np.ones((HD, HD), np.float32))
    E4 = np.zeros((4, 128), np.float32)
    for s in range(4):
        E4[s, 32 * s: 32 * s + 16] = 1.0
    maskE = np.zeros((128, 128), np.float32)
    maskO = np.zeros((128, 128), np.float32)
    for i in range(128):
        if (i // 16) % 2 == 0:
            maskE[i, i] = 1.0
        else:
            maskO[i, i] = 1.0
    sel4 = np.zeros((128, 4), np.float32)
    for s in range(4):
        sel4[32 * s + 16, s] = 1.0

    col = lambda a: np.asarray(a, np.float32).reshape(-1, 1)
    return {
        "ident": np.eye(128, dtype=np.float32),
        "identb": np.eye(128, dtype=np.float32),
        "wqT": wq.T.copy(), "wkT": wk.T.copy(), "wvT": wv.T.copy(),
        "bq": col(b3[:, 0].reshape(-1)), "bk": col(b3[:, 1].reshape(-1)),
        "bvrow": np.broadcast_to(b3[:, 2].reshape(-1), (128, 128)).copy(),
        "expbE": expb[0], "expbO": expb[1],
        "J": J, "E4": E4, "woE": wo[0], "woO": wo[1],
        "maskE": maskE, "maskO": maskO, "sel4": sel4,
        "b_o": col(inputs["b_out"]), "gamma": col(inputs["gamma"]),
        "n1w": col(inputs["norm1_w"]), "n1b": col(inputs["norm1_b"]),
        "epscol": np.full((128, 1), EPS, np.float32),
        "n2wE": n2w[0], "n2bE": n2b[0], "n2wO": n2w[1], "n2bO": n2b[1],
        "qnw": col(np.tile(np.asarray(inputs["qn_w"], np.float32), HE)),
        "qnb": col(np.tile(np.asarray(inputs["qn_b"], np.float32), HE)),
        "knw": col(np.tile(np.asarray(inputs["kn_w"], np.float32), HE)),
        "knb": col(np.tile(np.asarray(inputs["kn_b"], np.float32), HE)),
    }


_BF16_IN = {"wqT", "wkT", "wvT", "bvrow", "expbE", "expbO", "J", "E4",
            "woE", "woO", "identb", "maskE", "maskO", "sel4"}
_CONST_SHAPES = {
    "ident": (128, 128), "identb": (128, 128), "wqT": (128, 128), "wkT": (128, 128), "wvT": (128, 128),
    "bq": (128, 1), "bk": (128, 1), "bvrow": (128, 128),
    "expbE": (128, 512), "expbO": (128, 512), "J": (128, 128), "E4": (4, 128),
    "maskE": (128, 128), "maskO": (128, 128), "sel4": (128, 4),
    "woE": (128, 128), "woO": (128, 128), "b_o": (128, 1), "gamma": (128, 1),
    "n1w": (128, 1), "n1b": (128, 1), "epscol": (128, 1),
    "n2wE": (128, 1), "n2bE": (128, 1), "n2wO": (128, 1), "n2bO": (128, 1),
    "qnw": (128, 1), "qnb": (128, 1), "knw": (128, 1), "knb": (128, 1),
}


def build_nc(ln_general=True):
    nc = bacc.Bacc("TRN2", target_bir_lowering=False, debug=False,
                   enable_asserts=False, num_devices=NCORES)
    ins = {"x": nc.dram_tensor("x", [NP_, C], F32, kind="ExternalInput").ap()}
    for name, shp in _CONST_SHAPES.items():
        dt = BF16 if name in _BF16_IN else F32
        ins[name] = nc.dram_tensor(name, list(shp), dt, kind="ExternalInput").ap()
    y = nc.dram_tensor("y", [NP_, C], F32, kind="ExternalOutput").ap()
    with tile.TileContext(nc) as tc:
        _body(tc, nc, ins, y, ln_general)
    nc.compile()
    return nc


def _body(tc, nc, ins, y, ln_general):
    x = ins["x"]
    pools = {}

    def pool(name, bufs=1, space="SBUF"):
        if name not in pools:
            pools[name] = tc.alloc_tile_pool(name=name, bufs=bufs, space=space)
        return pools[name]

    cp = pool("consts")
    big = pool("big")
    ps = pool("psA", bufs=4, space="PSUM")
    ps_s = pool("psS", bufs=1, space="PSUM")
    ps_t = pool("psT", bufs=1, space="PSUM")
    dram = pool("dram", bufs=1, space="DRAM")
    sm = pool("small", bufs=1)
    ldp = pool("ldp", bufs=2)
    lnp = pool("lnp", bufs=1)
    esp = pool("esp", bufs=2)
    outp = pool("outp", bufs=2)

    cst = {}
    for name in _CONST_SHAPES:
        ap = ins[name]
        t = cp.tile(list(ap.shape), ap.dtype, tag=name)
        nc.sync.dma_start(t[:], ap)
        cst[name] = t

    # Big slots: S1 {xT, v_t} / S2 {kk} / S3 {xn, attH} / S4 {q} / S5 {sq, kEO}
    xT = big.tile([128, VST * NPAIR + 32], BF16, tag="S1", name="xT")[:, :NP_]
    q = big.tile([128, NP_], BF16, tag="S4")
    kk = big.tile([128, NP_], BF16, tag="S2")

    # ---------------- Phase 0: load + transpose --------------------------
    for k8 in range(32):
        ld = ldp.tile([128, 512], F32, tag="ld")
        src = x.rearrange("(k8 k p) c -> k8 p k c", p=128, k=4)[k8]
        nc.sync.dma_start(ld[:].rearrange("p (k c) -> p k c", k=4), src)
        for j in range(4):
            k = 4 * k8 + j
            pt = ps_t.tile([128, 128], F32, tag="t", name="tp")
            nc.tensor.transpose(pt[:], ld[:, 128 * j: 128 * (j + 1)], cst["ident"][:])
            if j % 2 == 0:
                nc.vector.tensor_copy(xT[:, 128 * k: 128 * (k + 1)], pt[:])
            else:
                nc.scalar.copy(xT[:, 128 * k: 128 * (k + 1)], pt[:])

    # ---------------- Phase 1: norm1 stats + AllReduce -------------------
    s1 = sm.tile([128, NSAMP], F32, tag="s1")
    s2 = sm.tile([128, NSAMP], F32, tag="s2")
    sq = big.tile([128, NP_], BF16, tag="S5")
    nc.vector.reduce_sum(s1[:], xT[:].rearrange("p (k s) -> p k s", s=128), axis=AX.X)
    nc.scalar.square(sq[:], xT[:])
    nc.vector.reduce_sum(s2[:], sq[:].rearrange("p (k s) -> p k s", s=128), axis=AX.X)

    stats = sm.tile([128, 2 * NSAMP], F32, tag="st2")
    nc.vector.tensor_copy(stats[:, :NSAMP], s1[:])
    nc.vector.tensor_copy(stats[:, NSAMP:], s2[:])
    cc_in = dram.tile([128, 2 * NSAMP], F32, tag="cc_in")
    cc_out = dram.tile([128, 2 * NSAMP], F32, tag="cc_out")
    nc.gpsimd.dma_start(cc_in[:], stats[:])
    nc.gpsimd.collective_compute("AllReduce", ALU.add,
                                 replica_groups=[list(range(NCORES))],
                                 ins=[cc_in[:].opt()], outs=[cc_out[:].opt()])
    nc.sync.dma_start(stats[:], cc_out[:])

    mean = sm.tile([128, NSAMP], F32, tag="m2")
    scl1 = sm.tile([128, NSAMP], F32, tag="sc2")
    sft1 = sm.tile([128, NSAMP], F32, tag="sf2")
    tmp = sm.tile([128, NSAMP], F32, tag="t2")

    def norm_coeffs(st, scl, sft, mn, tp, w_ap, b_ap):
        nc.vector.tensor_scalar_mul(mn[:], st[:, :NSAMP], 1.0 / SPATIAL)
        nc.vector.tensor_scalar_mul(tp[:], st[:, NSAMP:], 1.0 / SPATIAL)
        nc.vector.tensor_tensor(out=scl[:], in0=mn[:], in1=mn[:], op=ALU.mult)
        nc.vector.tensor_tensor(out=tp[:], in0=tp[:], in1=scl[:], op=ALU.subtract)
        nc.scalar.activation(tp[:], tp[:], AF.Ln, bias=cst["epscol"][:, 0:1], scale=1.0)
        nc.scalar.activation(scl[:], tp[:], AF.Exp, bias=0.0, scale=-0.5)
        nc.vector.tensor_scalar_mul(scl[:], scl[:], w_ap[:, 0:1])
        nc.vector.tensor_tensor(out=sft[:], in0=mn[:], in1=scl[:], op=ALU.mult)
        nc.vector.tensor_scalar(out=sft[:], in0=sft[:], scalar1=-1.0,
                                scalar2=b_ap[:, 0:1], op0=ALU.mult, op1=ALU.add)

    norm_coeffs(stats, scl1, sft1, mean, tmp, cst["n1w"], cst["n1b"])

    # ---------------- Phase 2: norm1 apply -------------------------------
    xn = big.tile([128, NP_], BF16, tag="S3")
    for k in range(NSAMP):
        sl = slice(128 * k, 128 * (k + 1))
        nc.vector.tensor_scalar(out=xn[:, sl], in0=xT[:, sl],
                                scalar1=scl1[:, k: k + 1], scalar2=sft1[:, k: k + 1],
                                op0=ALU.mult, op1=ALU.add)

    # ---------------- Phase 3: conv1 q, k --------------------------------
    for n in range(32):
        sl = slice(512 * n, 512 * (n + 1))
        pq = ps.tile([128, 512], F32, tag="a", name="pq")
        nc.tensor.matmul(pq[:], cst["wqT"][:], xn[:, sl], start=True, stop=True)
        nc.vector.tensor_scalar_add(q[:, sl], pq[:], cst["bq"][:, 0:1])
        pk = ps_s.tile([128, 512], F32, tag="s", name="pk")
        nc.tensor.matmul(pk[:], cst["wkT"][:], xn[:, sl], start=True, stop=True)
        nc.scalar.add(kk[:, sl], pk[:], cst["bk"][:, 0:1])

    # ---------------- Phase 4: LN on q and k (chunked) -------------------
    def layernorm(tt, w_ap, b_ap):
        for n in range(16):
            base = 1024 * n
            mu = lnp.tile([128, 1024], BF16, tag="ln_mu")
            vv = lnp.tile([128, 1024], BF16, tag="ln_vv")
            t1 = lnp.tile([128, 1024], BF16, tag="ln_t1")
            for c4 in range(2):
                sl = slice(base + 512 * c4, base + 512 * (c4 + 1))
                sll = slice(512 * c4, 512 * (c4 + 1))
                nc.scalar.square(t1[:, sll], tt[:, sl])
                pm = ps.tile([128, 512], F32, tag="a", name="pm")
                nc.tensor.matmul(pm[:], cst["J"][:], tt[:, sl], start=True, stop=True)
                pv = ps_s.tile([128, 512], F32, tag="s", name="pv")
                nc.tensor.matmul(pv[:], cst["J"][:], t1[:, sll], start=True, stop=True)
                nc.scalar.mul(mu[:, sll], pm[:], 1.0 / HD)
                nc.scalar.activation(vv[:, sll], pv[:], AF.Copy, bias=0.0,
                                     scale=1.0 / HD)
            nc.vector.tensor_tensor(out=t1[:], in0=mu[:], in1=mu[:], op=ALU.mult)
            nc.vector.tensor_tensor(out=vv[:], in0=vv[:], in1=t1[:], op=ALU.subtract)
            nc.scalar.activation(vv[:], vv[:], AF.Ln, bias=cst["epscol"][:, 0:1], scale=1.0)
            nc.scalar.activation(vv[:], vv[:], AF.Exp, bias=0.0, scale=-0.5)
            sl = slice(base, base + 1024)
            nc.vector.tensor_tensor(out=t1[:], in0=tt[:, sl], in1=mu[:],
                                    op=ALU.subtract)
            nc.vector.tensor_tensor(out=tt[:, sl], in0=t1[:], in1=vv[:], op=ALU.mult)
            if ln_general:
                nc.vector.tensor_scalar(out=tt[:, sl], in0=tt[:, sl],
                                        scalar1=w_ap[:, 0:1], scalar2=b_ap[:, 0:1],
                                        op0=ALU.mult, op1=ALU.add)

    layernorm(q, cst["qnw"], cst["qnb"])
    layernorm(kk, cst["knw"], cst["knb"])

    # ---------------- Phase 5: conv v (transposed layout) ----------------
    v_t = big.tile([128, VST * NPAIR + 32], BF16, tag="S1")
    nc.vector.memset(v_t[:], 0.0)
    ones_ap = v_t[:, :VST * NPAIR].rearrange(
        "p (m e d) -> p m e d", e=8, d=17)[:, :, :, 16:17]
    nc.vector.memset(ones_ap, 1.0)
    xnv = xn[:].rearrange("c (t m z) -> c t m z", m=NPAIR, z=2)
    for m in range(NPAIR):
        pv = ps_t.tile([128, 128], F32, tag="t", name="pvt")
        for z in range(2):
            nc.tensor.matmul(pv[64 * z: 64 * z + 64, :], xnv[:, :, m, z],
                             cst["wvT"][:], start=True, stop=True,
                             tile_position=(0, 64 * z))
        dst = v_t[:, VST * m: VST * m + 136].rearrange(
            "p (e d) -> p e d", d=17)[:, :, 0:16]
        nc.vector.tensor_tensor(out=dst,
                                in0=pv[:].rearrange("p (e d) -> p e d", d=16),
                                in1=cst["bvrow"][:].rearrange("p (e d) -> p e d", d=16),
                                op=ALU.add)

    # ---------------- Phases 6-8: attention (two head parities) ----------
    attH = big.tile([128, NP_], BF16, tag="S3")
    attE_d = dram.tile([128, NP_], BF16, tag="attE_d")
    qv = q[:].rearrange("a (t p) -> a t p", p=256)
    # Parity-masked copies of k: even-parity into a fresh slot (S5), odd
    # parity rewrites kk in place (PE diag-mask matmul zeroes the partner
    # head's 16 rows in each 32-row band, so contraction over 32 rows only
    # picks up one head).
    kE = big.tile([128, NP_], BF16, tag="S5")
    for n in range(32):
        sl = slice(512 * n, 512 * (n + 1))
        pE = ps.tile([128, 512], F32, tag="a", name="pE")
        nc.tensor.matmul(pE[:], cst["maskE"][:], kk[:, sl], start=True, stop=True)
        nc.vector.tensor_copy(kE[:, sl], pE[:])
        pO = ps_s.tile([128, 512], F32, tag="s", name="pO")
        nc.tensor.matmul(pO[:], cst["maskO"][:], kk[:, sl], start=True, stop=True)
        nc.scalar.copy(kk[:, sl], pO[:])
    kvE = kE[:].rearrange("a (t p) -> a t p", p=256)
    kvO = kk[:].rearrange("a (t p) -> a t p", p=256)

    def att_pass(eo, expb_c, woname, wname, bname):
        kv = kvE if eo == 0 else kvO

        for grp in range(32):
            pa0 = ps.tile([128, 256], F32, tag="pa0", name="pa0", bufs=1)
            pa1 = ps.tile([128, 256], F32, tag="pa1", name="pa1", bufs=1)
            for sg in range(2):
                pairs = (8 * grp + 4 * sg, 8 * grp + 4 * sg + 2)
                es = esp.tile([128, 512], BF16, tag="es")
                # One PSUM bank PER ROW GROUP (concurrent row-tiled matmuls
                # must not write the same bank+partitions), with r-innermost
                # issue order so LDWEIGHTS of the next matmul (different row
                # group) overlaps the in-flight one.
                pqk = [ps.tile([128, 128], F32, tag="a", name=f"pqk{r}",
                               bufs=4) for r in range(4)]
                for jj, pbase in enumerate(pairs):
                    for h01 in range(2):
                        p = pbase + h01
                        for tkc in range(2):
                            for r in range(4):
                                prt = slice(32 * r, 32 * r + 32)
                                nc.tensor.matmul(
                                    pqk[r][64 * h01 + 32 * tkc:
                                           64 * h01 + 32 * tkc + 32,
                                           64 * jj: 64 * jj + 64],
                                    kv[prt, 32 * tkc: 32 * tkc + 32, p],
                                    qv[prt, :, p], start=True, stop=True,
                                    tile_position=(32 * r,
                                                   64 * h01 + 32 * tkc))
                for r in range(4):
                    nc.scalar.activation(es[:, 128 * r: 128 * r + 128],
                                         pqk[r][:], AF.Exp, bias=0.0, scale=0.25)
                nc.vector.tensor_tensor(out=es[:], in0=es[:], in1=expb_c[:],
                                        op=ALU.mult)
                for jj, pbase in enumerate(pairs):
                    pair = pbase // 2
                    lp = 2 * sg + jj
                    for j4 in range(4):
                        eg = 2 * j4 + eo
                        for h01, pah in ((0, pa0), (1, pa1)):
                            lhs_v = v_t[64 * h01: 64 * h01 + 64,
                                        VST * pair + 17 * eg: VST * pair + 17 * eg + 32]
                            nc.tensor.matmul(
                                pah[32 * j4: 32 * j4 + 32, 64 * lp: 64 * lp + 64],
                                lhs_v,
                                es[64 * h01: 64 * h01 + 64,
                                   128 * j4 + 64 * jj: 128 * j4 + 64 * jj + 64],
                                start=True, stop=True,
                                tile_position=(64 * h01, 32 * j4))
            for h01, pah in ((0, pa0), (1, pa1)):
                dstv = attH[:, 512 * grp: 512 * (grp + 1)].rearrange(
                    "c (lp z) -> c lp z", z=128)[:, :, 64 * h01: 64 * h01 + 64]
                srcv = pah[:].rearrange("c (lp z) -> c lp z", z=64)
                if h01 == 0:
                    nc.vector.tensor_copy(dstv, srcv)
                else:
                    nc.scalar.copy(dstv, srcv)

        # rowsum division (fully on-chip: selector matmul pulls the 4 ones
        # rows into partitions 0-3, reciprocal, E4 matmul broadcasts back
        # to all 128 partitions)
        for n in range(32):
            sl = slice(512 * n, 512 * (n + 1))
            prs = ps_t.tile([4, 512], F32, tag="t", name="prs")
            nc.tensor.matmul(prs[:], cst["sel4"][:], attH[:, sl],
                             start=True, stop=True)
            rs4 = esp.tile([4, 512], F32, tag="rs4")
            nc.vector.reciprocal(rs4[:], prs[:])
            rs8 = esp.tile([4, 512], BF16, tag="rs8")
            nc.vector.tensor_copy(rs8[:], rs4[:])
            pb = ps_s.tile([128, 512], F32, tag="s", name="pb")
            nc.tensor.matmul(pb[:], cst["E4"][:], rs8[:], start=True, stop=True)
            rbc = esp.tile([128, 512], BF16, tag="rbc")
            nc.scalar.copy(rbc[:], pb[:])
            nc.vector.tensor_tensor(out=attH[:, sl], in0=attH[:, sl], in1=rbc[:],
                                    op=ALU.mult)

        # norm2 stats (chunked; cols of chunk n = pixels 8n..8n+8, b = n//16)
        s1b = sm.tile([128, NSAMP], F32, tag="s1")
        s2b = sm.tile([128, NSAMP], F32, tag="s2")
        nc.vector.memset(s1b[:], 0.0)
        nc.vector.memset(s2b[:], 0.0)
        for n in range(32):
            sl = slice(512 * n, 512 * (n + 1))
            b = n // 16
            ssl = slice(64 * b, 64 * b + 64)
            sqc = esp.tile([128, 512], BF16, tag="sqc")
            nc.scalar.square(sqc[:], attH[:, sl])
            p1 = sm.tile([128, T], F32, tag="p1")
            p2 = sm.tile([128, T], F32, tag="p2")
            nc.vector.reduce_sum(
                p1[:], attH[:, sl].rearrange("c (p t) -> c t p", p=8), axis=AX.X)
            nc.vector.reduce_sum(
                p2[:], sqc[:].rearrange("c (p t) -> c t p", p=8), axis=AX.X)
            nc.vector.tensor_tensor(out=s1b[:, ssl], in0=s1b[:, ssl], in1=p1[:],
                                    op=ALU.add)
            nc.vector.tensor_tensor(out=s2b[:, ssl], in0=s2b[:, ssl], in1=p2[:],
                                    op=ALU.add)
        st2 = sm.tile([128, 2 * NSAMP], F32, tag="st2")
        nc.vector.tensor_copy(st2[:, :NSAMP], s1b[:])
        nc.vector.tensor_copy(st2[:, NSAMP:], s2b[:])
        cc2i = dram.tile([128, 2 * NSAMP], F32, tag="cc_in")
        cc2o = dram.tile([128, 2 * NSAMP], F32, tag="cc_out")
        nc.gpsimd.dma_start(cc2i[:], st2[:])
        nc.gpsimd.collective_compute("AllReduce", ALU.add,
                                     replica_groups=[list(range(NCORES))],
                                     ins=[cc2i[:].opt()], outs=[cc2o[:].opt()])
        nc.sync.dma_start(st2[:], cc2o[:])
        sc2 = sm.tile([128, NSAMP], F32, tag="sc2")
        sf2 = sm.tile([128, NSAMP], F32, tag="sf2")
        m2 = sm.tile([128, NSAMP], F32, tag="m2")
        t2 = sm.tile([128, NSAMP], F32, tag="t2")
        norm_coeffs(st2, sc2, sf2, m2, t2, cst[wname], cst[bname])
        attv = attH[:].rearrange("c (b s t) -> c b s t", b=2, s=128)
        for b in range(2):
            for t in range(T):
                colap = attv[:, b, :, t]
                j = 64 * b + t
                nc.vector.tensor_scalar(out=colap, in0=colap,
                                        scalar1=sc2[:, j: j + 1],
                                        scalar2=sf2[:, j: j + 1],
                                        op0=ALU.mult, op1=ALU.add)
        if eo == 0:
            nc.sync.dma_start(attE_d[:], attH[:])

    att_pass(0, cst["expbE"], "woE", "n2wE", "n2bE")
    att_pass(1, cst["expbO"], "woO", "n2wO", "n2bO")

    # ---------------- Phase 9: conv2 + gamma + residual + store ----------
    yv = y.rearrange("(t m z) c -> t m z c", m=NPAIR, z=2)
    xvr = x.rearrange("(t m z) c -> t m z c", m=NPAIR, z=2)
    for n in range(32):
        sl = slice(512 * n, 512 * (n + 1))
        aA = outp.tile([128, 512], BF16, tag="aA")
        nc.sync.dma_start(aA[:], attE_d[:, sl])
        po = ps.tile([128, 512], F32, tag="a", name="po")
        nc.tensor.matmul(po[:], cst["woE"][:], aA[:], start=True, stop=False)
        nc.tensor.matmul(po[:], cst["woO"][:], attH[:, sl], start=False, stop=True)
        yb = outp.tile([128, 512], BF16, tag="yb")
        nc.vector.tensor_scalar(out=yb[:], in0=po[:], scalar1=cst["b_o"][:, 0:1],
                                scalar2=cst["gamma"][:, 0:1], op0=ALU.add, op1=ALU.mult)
        xr = outp.tile([128, 512], F32, tag="xr")
        for z in range(2):
            nc.sync.dma_start(
                xr[64 * z: 64 * z + 64, :].rearrange("t (m c) -> t m c", m=4),
                xvr[:, 4 * n: 4 * n + 4, z, :])
        y8 = outp.tile([128, 512], F32, tag="y8")
        for j in range(4):
            pt = ps_t.tile([128, 128], BF16, tag="t", name="tp2")
            nc.tensor.transpose(pt[:], yb[:, 128 * j: 128 * (j + 1)], cst["identb"][:])
            nc.vector.tensor_tensor(out=y8[:, 128 * j: 128 * (j + 1)], in0=pt[:],
                                    in1=xr[:, 128 * j: 128 * (j + 1)], op=ALU.add)
        for z in range(2):
            nc.sync.dma_start(
                yv[:, 4 * n: 4 * n + 4, z, :],
                y8[64 * z: 64 * z + 64, :].rearrange("t (m c) -> t m c", m=4))

    for p_ in reversed(list(pools.values())):
        p_.release()


# ---- public entry point -------------------------------------------------
_NC = None


def _get_nc():
    global _NC
    if _NC is None:
        _NC = build_nc(ln_general=True)
    return _NC


def kernel(**inputs) -> np.ndarray:
    import ml_dtypes
    from concourse import bass_utils

    nc = _get_nc()
    consts = host_prep(inputs)
    cmap = {}
    for name, val in consts.items():
        v = np.asarray(val, np.float32)
        if name in _BF16_IN:
            v = v.astype(ml_dtypes.bfloat16)
        cmap[name] = v
    x = np.asarray(inputs["x"], np.float32)
    in_maps = []
    for c in range(NCORES):
        m = dict(cmap)
        m["x"] = np.ascontiguousarray(
            x[:, :, HL * c: HL * (c + 1), :, :].reshape(-1, C))
        in_maps.append(m)
    res = bass_utils.run_bass_kernel_spmd(
        nc, in_maps, core_ids=list(range(NCORES)), trace=False)
    y = np.zeros((T, B, H, W, C), np.float32)
    for c in range(NCORES):
        y[:, :, HL * c: HL * (c + 1), :, :] = \
            res.results[c]["y"].reshape(T, B, HL, W, C)
    return y


def kernel_traced(**inputs):
    """Like kernel() but returns (y, per_core_exec_ns, trace_path)."""
    import ml_dtypes
    from concourse import bass_utils

    nc = _get_nc()
    consts = host_prep(inputs)
    cmap = {}
    for name, val in consts.items():
        v = np.asarray(val, np.float32)
        if name in _BF16_IN:
            v = v.astype(ml_dtypes.bfloat16)
        cmap[name] = v
    x = np.asarray(inputs["x"], np.float32)
    in_maps = []
    for c in range(NCORES):
        m = dict(cmap)
        m["x"] = np.ascontiguousarray(
            x[:, :, HL * c: HL * (c + 1), :, :].reshape(-1, C))
        in_maps.append(m)
    res = bass_utils.run_bass_kernel_spmd(
        nc, in_maps, core_ids=list(range(NCORES)),
        trace=True, trace_cores=list(range(NCORES)))
    y = np.zeros((T, B, H, W, C), np.float32)
    for c in range(NCORES):
        y[:, :, HL * c: HL * (c + 1), :, :] = \
            res.results[c]["y"].reshape(T, B, HL, W, C)
    trace_path = (res.instructions_and_trace[1]
                  if res.instructions_and_trace else None)
    return y, res.exec_time_ns, trace_path



# revision 35
# speedup vs baseline: 1.2841x; 1.1508x over previous
"""8-NeuronCore Trainium2 Bass kernel for nn_AttentionBlock_17789754540111.

Self-contained: builds a hand-written Bass/Tile SPMD program (H-sharded over
8 cores, params replicated, instance-norm stats combined via on-device
AllReduce) and runs it on the 8 axon-tunneled TRN2 cores via
concourse.bass_utils.run_bass_kernel_spmd.
"""

import sys
import types
import ctypes
import contextlib

# ---- NTFF profile hook expected by bass_utils under axon ----------------
def _install_axon_hook():
    if "antenv.axon_hooks" in sys.modules:
        return
    hookmod = types.ModuleType("antenv.axon_hooks")

    def _make_hook():
        try:
            lib = ctypes.CDLL("/opt/axon/libaxon_pjrt.so")
        except OSError:
            return None
        if not hasattr(lib, "axon_start_nrt_profile"):
            return None
        lib.axon_start_nrt_profile.argtypes = [ctypes.POINTER(ctypes.c_int64),
                                               ctypes.c_size_t]
        lib.axon_start_nrt_profile.restype = ctypes.c_int64
        lib.axon_stop_nrt_profile.argtypes = [ctypes.c_char_p]
        lib.axon_stop_nrt_profile.restype = ctypes.c_int64

        @contextlib.contextmanager
        def _hook(output_dir, device_ids):
            import jax
            jax.devices()
            if device_ids:
                ids = (ctypes.c_int64 * len(device_ids))(*device_ids)
                rc = lib.axon_start_nrt_profile(ids, len(device_ids))
            else:
                rc = lib.axon_start_nrt_profile(None, 0)
            if rc != 0:
                raise RuntimeError(f"axon_start_nrt_profile rc={rc}")
            try:
                yield
            finally:
                lib.axon_stop_nrt_profile(str(output_dir).encode())
        return _hook

    hook = _make_hook()
    hookmod.get_axon_ntff_profile_hook = lambda: hook
    hookmod.set_axon_ntff_profile_hook = lambda h: None
    sys.modules["antenv.axon_hooks"] = hookmod

_install_axon_hook()

import math
import sys

sys.path.insert(0, "/opt/trn_rl_repo")

import numpy as np

import concourse.bass as bass  # noqa
import concourse.bacc as bacc
import concourse.mybir as mybir
from concourse import tile

F32 = mybir.dt.float32
BF16 = mybir.dt.bfloat16
AF = mybir.ActivationFunctionType
ALU = mybir.AluOpType
AX = mybir.AxisListType

T, B, H, W, C = 64, 2, 32, 32, 128
HE, HD = 8, 16
NCORES = 8
HL = H // NCORES
PIX = B * HL * W                  # 256
NP_ = T * PIX                     # 16384
NSAMP = T * B                     # 128
SPATIAL = H * W                   # 1024 (global)
EPS = 1e-5
VST = 136
NPAIR = PIX // 2                  # 128


def _rel_bias_table(rel_emb):
    rp = np.arange(T)[None, :] - np.arange(T)[:, None]
    n = -rp
    ret = (n < 0).astype(np.int64) * 16
    n = np.abs(n)
    mx = 8
    small = n < mx
    vl = mx + (np.log(np.maximum(n, 1) / mx) / math.log(32 / mx) * 8).astype(np.int64)
    vl = np.minimum(vl, 15)
    buckets = ret + np.where(small, n, vl)
    return np.ascontiguousarray(
        rel_emb[buckets].transpose(2, 0, 1)).astype(np.float32)  # (he, tq, tk)


def host_prep(inputs):
    w_in = np.asarray(inputs["w_in"], np.float32)
    b_in = np.asarray(inputs["b_in"], np.float32)
    rows = w_in.reshape(HE, 3, HD, C)
    b3 = b_in.reshape(HE, 3, HD)
    wq = rows[:, 0].reshape(HE * HD, C)
    wk = rows[:, 1].reshape(HE * HD, C)
    wv = rows[:, 2].reshape(HE * HD, C)

    bias = _rel_bias_table(np.asarray(inputs["rel_emb"], np.float32))
    eb = np.exp(bias)                                          # (he, tq, tk)
    expb, wo, n2w, n2b = {}, {}, {}, {}
    n2w_full = np.asarray(inputs["norm2_w"], np.float32)
    n2b_full = np.asarray(inputs["norm2_b"], np.float32)
    w_out = np.asarray(inputs["w_out"], np.float32)
    for eo in (0, 1):
        heads = [2 * j + eo for j in range(4)]
        h = eb[heads]                                          # (4, tq, tk)
        # col = j4*128 + jj*64 + tq ; rows = 2 px * 64 tk
        e1 = h.transpose(2, 0, 1)                              # (tk, j, tq)
        e2 = np.stack([e1, e1], axis=2).reshape(T, 4 * 2 * T)  # (tk, (j, jj, tq))
        expb[eo] = np.concatenate([e2, e2], axis=0).astype(np.float32)
        m = np.zeros((128, 128), np.float32)
        wv_ = np.zeros((128, 1), np.float32)
        bv_ = np.zeros((128, 1), np.float32)
        for j, he in enumerate(heads):
            m[32 * j: 32 * j + 16, :] = w_out[:, he * 16: (he + 1) * 16].T
            wv_[32 * j: 32 * j + 16, 0] = n2w_full[he * 16: (he + 1) * 16]
            bv_[32 * j: 32 * j + 16, 0] = n2b_full[he * 16: (he + 1) * 16]
        wo[eo], n2w[eo], n2b[eo] = m, wv_, bv_

    J = np.kron(np.eye(HE, dtype=np.float32), np.ones((HD, HD), np.float32))
    E4 = np.zeros((4, 128), np.float32)
    for s in range(4):
        E4[s, 32 * s: 32 * s + 16] = 1.0
    sel4 = np.zeros((128, 4), np.float32)
    for s in range(4):
        sel4[32 * s + 16, s] = 1.0
    sel32 = np.zeros((8, 128, 32), np.float32)
    for j in range(8):
        for s in range(4):
            sel32[j, 32 * s + 16, 4 * j + s] = 1.0
    E4b = np.zeros((8, 32, 128), np.float32)
    for j in range(8):
        for s in range(4):
            E4b[j, 4 * j + s, 32 * s: 32 * s + 16] = 1.0

    # Centered conv weights: folding the head-mean subtraction of the q/k
    # LayerNorm into the 1x1 conv (q_hat = (I - J/HD) q is exactly q - mean).
    Cc = np.eye(128, dtype=np.float32) - J / HD
    wq2 = Cc @ wq
    wk2 = Cc @ wk
    bq2 = Cc @ b3[:, 0].reshape(-1)
    bk2 = Cc @ b3[:, 1].reshape(-1)

    S16 = np.zeros((128, 8), np.float32)
    for p in range(128):
        S16[p, p // 16] = 1.0 / HD
    qn_w = np.asarray(inputs["qn_w"], np.float32)
    kn_w = np.asarray(inputs["kn_w"], np.float32)
    qn_b = np.asarray(inputs["qn_b"], np.float32)
    kn_b = np.asarray(inputs["kn_b"], np.float32)
    S8q = np.zeros((8, 128), np.float32)
    S8kE = np.zeros((8, 128), np.float32)
    S8kO = np.zeros((8, 128), np.float32)
    knbE = np.zeros(128, np.float32)
    knbO = np.zeros(128, np.float32)
    for p in range(128):
        h = p // 16
        S8q[h, p] = qn_w[p % 16]
        if h % 2 == 0:
            S8kE[h, p] = kn_w[p % 16]
            knbE[p] = kn_b[p % 16]
        else:
            S8kO[h, p] = kn_w[p % 16]
            knbO[p] = kn_b[p % 16]

    col = lambda a: np.asarray(a, np.float32).reshape(-1, 1)
    return {
        "ident": np.eye(128, dtype=np.float32),
        "identb": np.eye(128, dtype=np.float32),
        "wqT": wq2.T.copy(), "wkT": wk2.T.copy(), "wvT": wv.T.copy(),
        "bq": col(bq2), "bk": col(bk2),
        "bvrow": np.broadcast_to(b3[:, 2].reshape(-1), (128, 128)).copy(),
        "expbE": expb[0], "expbO": expb[1],
        "E4": E4, "woE": wo[0], "woO": wo[1],
        "sel4": sel4,
        "S16": S16, "S8q": S8q, "S8kE": S8kE, "S8kO": S8kO,
        **{f"E4b{j}": E4b[j] for j in range(8)},
        **{f"sel32_{j}": sel32[j] for j in range(8)},
        "b_o": col(inputs["b_out"]), "gamma": col(inputs["gamma"]),
        "n1w": col(inputs["norm1_w"]), "n1b": col(inputs["norm1_b"]),
        "epscol": np.full((128, 1), EPS, np.float32),
        "n2wE": n2w[0], "n2bE": n2b[0], "n2wO": n2w[1], "n2bO": n2b[1],
        "qnb": col(np.tile(qn_b, HE)),
        "knbE": col(knbE), "knbO": col(knbO),
    }


_BF16_IN = {"wqT", "wkT", "wvT", "bvrow", "expbE", "expbO", "E4",
            "woE", "woO", "identb", "sel4", "S16", "S8q", "S8kE", "S8kO",
            *[f"E4b{j}" for j in range(8)],
            *[f"sel32_{j}" for j in range(8)]}
_CONST_SHAPES = {
    "ident": (128, 128), "identb": (128, 128), "wqT": (128, 128), "wkT": (128, 128), "wvT": (128, 128),
    "bq": (128, 1), "bk": (128, 1), "bvrow": (128, 128),
    "expbE": (128, 512), "expbO": (128, 512), "E4": (4, 128),
    "sel4": (128, 4),
    "S16": (128, 8), "S8q": (8, 128), "S8kE": (8, 128), "S8kO": (8, 128),
    **{f"E4b{j}": (32, 128) for j in range(8)},
    **{f"sel32_{j}": (128, 32) for j in range(8)},
    "woE": (128, 128), "woO": (128, 128), "b_o": (128, 1), "gamma": (128, 1),
    "n1w": (128, 1), "n1b": (128, 1), "epscol": (128, 1),
    "n2wE": (128, 1), "n2bE": (128, 1), "n2wO": (128, 1), "n2bO": (128, 1),
    "qnb": (128, 1), "knbE": (128, 1), "knbO": (128, 1),
}


def build_nc(ln_general=True):
    nc = bacc.Bacc("TRN2", target_bir_lowering=False, debug=False,
                   enable_asserts=False, num_devices=NCORES)
    ins = {"x": nc.dram_tensor("x", [NP_, C], F32, kind="ExternalInput").ap()}
    for name, shp in _CONST_SHAPES.items():
        dt = BF16 if name in _BF16_IN else F32
        ins[name] = nc.dram_tensor(name, list(shp), dt, kind="ExternalInput").ap()
    y = nc.dram_tensor("y", [NP_, C], F32, kind="ExternalOutput").ap()
    with tile.TileContext(nc) as tc:
        _body(tc, nc, ins, y, ln_general)
    nc.compile()
    return nc


def _body(tc, nc, ins, y, ln_general):
    x = ins["x"]
    pools = {}

    def pool(name, bufs=1, space="SBUF"):
        if name not in pools:
            pools[name] = tc.alloc_tile_pool(name=name, bufs=bufs, space=space)
        return pools[name]

    cp = pool("consts")
    big = pool("big")
    ps = pool("psA", bufs=4, space="PSUM")
    ps_s = pool("psS", bufs=2, space="PSUM")
    dram = pool("dram", bufs=1, space="DRAM")
    sm = pool("small", bufs=1)
    scr = pool("scr", bufs=2)
    esp = pool("esp", bufs=2)

    cst = {}
    for name in _CONST_SHAPES:
        ap = ins[name]
        t = cp.tile(list(ap.shape), ap.dtype, tag=name)
        nc.sync.dma_start(t[:], ap)
        cst[name] = t

    # Big slots: S1 {xT, v_t} / S2 {kk} / S3 {xn, attH} / S4 {q} / S5 {sq, kEO}
    xT = big.tile([128, VST * NPAIR + 32], BF16, tag="S1", name="xT")[:, :NP_]
    q = big.tile([128, NP_], BF16, tag="S4")
    kk = big.tile([128, NP_], BF16, tag="S2")

    # ---------------- Phase 0: load + transpose + norm1 stats ------------
    # Stats are computed per quarter as the transposed data lands, so the
    # reduce work overlaps the DMA/PE of later quarters and the AllReduce
    # can fire right after the last chunk.
    s1 = sm.tile([128, NSAMP], F32, tag="s1")
    s2 = sm.tile([128, NSAMP], F32, tag="s2")
    sq = big.tile([128, NP_], BF16, tag="S5")
    for k8 in range(32):
        ld = scr.tile([128, 512], F32, tag="ld", name="ld")
        src = x.rearrange("(k8 k p) c -> k8 p k c", p=128, k=4)[k8]
        nc.sync.dma_start(ld[:].rearrange("p (k c) -> p k c", k=4), src)
        for j in range(4):
            k = 4 * k8 + j
            pt = ps.tile([128, 128], F32, tag="a", name="tp")
            nc.tensor.transpose(pt[:], ld[:, 128 * j: 128 * (j + 1)], cst["ident"][:])
            if j % 2 == 0:
                nc.vector.tensor_copy(xT[:, 128 * k: 128 * (k + 1)], pt[:])
            else:
                nc.scalar.copy(xT[:, 128 * k: 128 * (k + 1)], pt[:])
        if k8 % 8 == 7:
            qq = k8 // 8
            qsl = slice(4096 * qq, 4096 * (qq + 1))
            ssl = slice(32 * qq, 32 * (qq + 1))
            nc.vector.reduce_sum(
                s1[:, ssl], xT[:, qsl].rearrange("p (k s) -> p k s", s=128),
                axis=AX.X)
            nc.scalar.square(sq[:, qsl], xT[:, qsl])
            nc.vector.reduce_sum(
                s2[:, ssl], sq[:, qsl].rearrange("p (k s) -> p k s", s=128),
                axis=AX.X)

    stats = sm.tile([128, 2 * NSAMP], F32, tag="st2")
    nc.vector.tensor_copy(stats[:, :NSAMP], s1[:])
    nc.vector.tensor_copy(stats[:, NSAMP:], s2[:])
    cc_in = dram.tile([128, 2 * NSAMP], F32, tag="cc_in")
    cc_out = dram.tile([128, 2 * NSAMP], F32, tag="cc_out")
    nc.gpsimd.dma_start(cc_in[:], stats[:])
    nc.gpsimd.collective_compute("AllReduce", ALU.add,
                                 replica_groups=[list(range(NCORES))],
                                 ins=[cc_in[:].opt()], outs=[cc_out[:].opt()])
    nc.sync.dma_start(stats[:], cc_out[:])

    mean = sm.tile([128, NSAMP], F32, tag="m2")
    scl1 = sm.tile([128, NSAMP], F32, tag="sc2")
    sft1 = sm.tile([128, NSAMP], F32, tag="sf2")
    tmp = sm.tile([128, NSAMP], F32, tag="t2")

    def norm_coeffs(st, scl, sft, mn, tp, w_ap, b_ap):
        nc.vector.tensor_scalar_mul(mn[:], st[:, :NSAMP], 1.0 / SPATIAL)
        nc.vector.tensor_scalar_mul(tp[:], st[:, NSAMP:], 1.0 / SPATIAL)
        nc.vector.tensor_tensor(out=scl[:], in0=mn[:], in1=mn[:], op=ALU.mult)
        nc.vector.tensor_tensor(out=tp[:], in0=tp[:], in1=scl[:], op=ALU.subtract)
        nc.scalar.activation(tp[:], tp[:], AF.Ln, bias=cst["epscol"][:, 0:1], scale=1.0)
        nc.scalar.activation(scl[:], tp[:], AF.Exp, bias=0.0, scale=-0.5)
        nc.vector.tensor_scalar_mul(scl[:], scl[:], w_ap[:, 0:1])
        nc.vector.tensor_tensor(out=sft[:], in0=mn[:], in1=scl[:], op=ALU.mult)
        nc.vector.tensor_scalar(out=sft[:], in0=sft[:], scalar1=-1.0,
                                scalar2=b_ap[:, 0:1], op0=ALU.mult, op1=ALU.add)

    norm_coeffs(stats, scl1, sft1, mean, tmp, cst["n1w"], cst["n1b"])

    # ---------------- Phase 2: norm1 apply -------------------------------
    xn = big.tile([128, NP_], BF16, tag="S3")
    for k in range(NSAMP):
        sl = slice(128 * k, 128 * (k + 1))
        nc.vector.tensor_scalar(out=xn[:, sl], in0=xT[:, sl],
                                scalar1=scl1[:, k: k + 1], scalar2=sft1[:, k: k + 1],
                                op0=ALU.mult, op1=ALU.add)

    # ---------------- Phase 3+4: conv1 + head-LN (centered weights) ------
    # wqT/wkT are pre-centered host-side: the conv directly yields
    # q_hat = q - mean_head(q).  Then var = mean(q_hat^2), rstd broadcast
    # back to all 16 rows of each head via a small selector matmul, and the
    # k apply writes both parity-masked copies (kE: even heads, kO: odd).
    kE = big.tile([128, NP_], BF16, tag="S5")
    for n in range(32):
        sl = slice(512 * n, 512 * (n + 1))
        # q
        pq = ps.tile([128, 512], F32, tag="a", name="pq")
        nc.tensor.matmul(pq[:], cst["wqT"][:], xn[:, sl], start=True, stop=True)
        qh = scr.tile([128, 512], BF16, tag="qh")
        nc.any.tensor_scalar(out=qh[:], in0=pq[:], scalar1=cst["bq"][:, 0:1],
                             scalar2=None, op0=ALU.add)
        sqq = scr.tile([128, 512], BF16, tag="sqq")
        nc.any.tensor_tensor(out=sqq[:], in0=qh[:], in1=qh[:], op=ALU.mult)
        pv8q = ps_s.tile([8, 512], F32, tag="s", name="pv8q", bufs=2)
        nc.tensor.matmul(pv8q[:], cst["S16"][:], sqq[:], start=True, stop=True)
        nc.scalar.activation(pv8q[:], pv8q[:], AF.Ln,
                             bias=cst["epscol"][0:8, 0:1], scale=1.0)
        rstdq = scr.tile([8, 512], BF16, tag="rstdq")
        nc.scalar.activation(rstdq[:], pv8q[:], AF.Exp, bias=0.0, scale=-0.5)
        pRq = ps.tile([128, 512], F32, tag="a", name="pRq")
        nc.tensor.matmul(pRq[:], cst["S8q"][:], rstdq[:], start=True, stop=True)
        Rq = scr.tile([128, 512], BF16, tag="Rq")
        nc.any.tensor_copy(Rq[:], pRq[:])
        nc.vector.tensor_tensor(out=q[:, sl], in0=qh[:], in1=Rq[:], op=ALU.mult)
        nc.vector.tensor_scalar_add(q[:, sl], q[:, sl], cst["qnb"][:, 0:1])
        # k
        pk = ps.tile([128, 512], F32, tag="a", name="pk")
        nc.tensor.matmul(pk[:], cst["wkT"][:], xn[:, sl], start=True, stop=True)
        kh = scr.tile([128, 512], BF16, tag="qh", name="kh")
        nc.any.tensor_scalar(out=kh[:], in0=pk[:], scalar1=cst["bk"][:, 0:1],
                             scalar2=None, op0=ALU.add)
        sqk = scr.tile([128, 512], BF16, tag="sqq", name="sqk")
        nc.any.tensor_tensor(out=sqk[:], in0=kh[:], in1=kh[:], op=ALU.mult)
        pv8k = ps_s.tile([8, 512], F32, tag="s", name="pv8k", bufs=2)
        nc.tensor.matmul(pv8k[:], cst["S16"][:], sqk[:], start=True, stop=True)
        nc.scalar.activation(pv8k[:], pv8k[:], AF.Ln,
                             bias=cst["epscol"][0:8, 0:1], scale=1.0)
        rstdk = scr.tile([8, 512], BF16, tag="rstdq", name="rstdk")
        nc.scalar.activation(rstdk[:], pv8k[:], AF.Exp, bias=0.0, scale=-0.5)
        for sname, bname, dst in (("S8kE", "knbE", kE), ("S8kO", "knbO", kk)):
            pRk = ps.tile([128, 512], F32, tag="a", name="pRk")
            nc.tensor.matmul(pRk[:], cst[sname][:], rstdk[:], start=True, stop=True)
            Rk = scr.tile([128, 512], BF16, tag="Rk")
            nc.any.tensor_copy(Rk[:], pRk[:])
            nc.vector.tensor_tensor(out=dst[:, sl], in0=kh[:], in1=Rk[:],
                                    op=ALU.mult)
            nc.vector.tensor_scalar_add(dst[:, sl], dst[:, sl],
                                        cst[bname][:, 0:1])

    # ---------------- Phase 5: conv v (transposed layout) ----------------
    v_t = big.tile([128, VST * NPAIR + 32], BF16, tag="S1")
    nc.vector.memset(v_t[:], 0.0)
    ones_ap = v_t[:, :VST * NPAIR].rearrange(
        "p (m e d) -> p m e d", e=8, d=17)[:, :, :, 16:17]
    nc.vector.memset(ones_ap, 1.0)
    xnv = xn[:].rearrange("c (t m z) -> c t m z", m=NPAIR, z=2)
    for m in range(NPAIR):
        pv = ps.tile([128, 128], F32, tag="a", name="pvt")
        for z in range(2):
            nc.tensor.matmul(pv[64 * z: 64 * z + 64, :], xnv[:, :, m, z],
                             cst["wvT"][:], start=True, stop=True,
                             tile_position=(0, 64 * z))
        dst = v_t[:, VST * m: VST * m + 136].rearrange(
            "p (e d) -> p e d", d=17)[:, :, 0:16]
        nc.vector.tensor_tensor(out=dst,
                                in0=pv[:].rearrange("p (e d) -> p e d", d=16),
                                in1=cst["bvrow"][:].rearrange("p (e d) -> p e d", d=16),
                                op=ALU.add)

    # ---------------- Phases 6-8: attention (two head parities) ----------
    attH = big.tile([128, NP_], BF16, tag="S3")
    attE_d = dram.tile([128, NP_], BF16, tag="attE_d")
    qv = q[:].rearrange("a (t p) -> a t p", p=256)
    kvE = kE[:].rearrange("a (t p) -> a t p", p=256)
    kvO = kk[:].rearrange("a (t p) -> a t p", p=256)

    def att_pass(eo, expb_c, woname, wname, bname):
        kv = kvE if eo == 0 else kvO
        s1b = sm.tile([128, NSAMP], F32, tag="s1")
        s2b = sm.tile([128, NSAMP], F32, tag="s2")

        def rowsum_stats_block(nb):
            # rowsum division for chunks 8nb..8nb+8 + their norm2 stats,
            # emitted right after the producing groups so the DVE work
            # overlaps later groups' matmuls.
            rs_blk = esp.tile([32, 512], F32, tag="rs_blk", bufs=1)
            rsb_blk = esp.tile([32, 512], BF16, tag="rsb_blk", bufs=1)
            prs = ps.tile([32, 512], F32, tag="a", name="prs")
            for j in range(8):
                n = 8 * nb + j
                sl = slice(512 * n, 512 * (n + 1))
                nc.tensor.matmul(prs[:], cst[f"sel32_{j}"][:], attH[:, sl],
                                 start=(j == 0), stop=(j == 7))
            nc.vector.reciprocal(rs_blk[:], prs[:])
            nc.vector.tensor_copy(rsb_blk[:], rs_blk[:])
            for j in range(8):
                n = 8 * nb + j
                sl = slice(512 * n, 512 * (n + 1))
                pb = ps_s.tile([128, 512], F32, tag="s", name="pb")
                nc.tensor.matmul(pb[:], cst[f"E4b{j}"][:], rsb_blk[:],
                                 start=True, stop=True)
                rbc = esp.tile([128, 512], BF16, tag="rbc", bufs=1)
                nc.scalar.copy(rbc[:], pb[:])
                nc.vector.tensor_tensor(out=attH[:, sl], in0=attH[:, sl],
                                        in1=rbc[:], op=ALU.mult)
                sqc = scr.tile([128, 512], BF16, tag="Rq", name="sqc")
                nc.scalar.square(sqc[:], attH[:, sl])
                nc.vector.reduce_sum(
                    s1b[:, 4 * n: 4 * n + 4],
                    attH[:, sl].rearrange("c (s p) -> c s p", s=4), axis=AX.X)
                nc.vector.reduce_sum(
                    s2b[:, 4 * n: 4 * n + 4],
                    sqc[:].rearrange("c (s p) -> c s p", s=4), axis=AX.X)

        for grp in range(32):
            pa0 = ps.tile([128, 256], F32, tag="pa0", name="pa0", bufs=1)
            pa1 = ps.tile([128, 256], F32, tag="pa1", name="pa1", bufs=1)
            for sg in range(2):
                pairs = (8 * grp + 4 * sg, 8 * grp + 4 * sg + 2)
                es = esp.tile([128, 512], BF16, tag="es")
                # One PSUM bank PER ROW GROUP (concurrent row-tiled matmuls
                # must not write the same bank+partitions), with r-innermost
                # issue order so LDWEIGHTS of the next matmul (different row
                # group) overlaps the in-flight one.
                pqk = [ps.tile([128, 128], F32, tag="a", name=f"pqk{r}",
                               bufs=4) for r in range(4)]
                for jj, pbase in enumerate(pairs):
                    for h01 in range(2):
                        p = pbase + h01
                        for tkc in range(2):
                            for r in range(4):
                                prt = slice(32 * r, 32 * r + 32)
                                nc.tensor.matmul(
                                    pqk[r][64 * h01 + 32 * tkc:
                                           64 * h01 + 32 * tkc + 32,
                                           64 * jj: 64 * jj + 64],
                                    kv[prt, 32 * tkc: 32 * tkc + 32, p],
                                    qv[prt, :, p], start=True, stop=True,
                                    tile_position=(32 * r,
                                                   64 * h01 + 32 * tkc))
                for r in range(4):
                    nc.scalar.activation(es[:, 128 * r: 128 * r + 128],
                                         pqk[r][:], AF.Exp, bias=0.0, scale=0.25)
                nc.vector.tensor_tensor(out=es[:], in0=es[:], in1=expb_c[:],
                                        op=ALU.mult)
                for jj, pbase in enumerate(pairs):
                    pair = pbase // 2
                    lp = 2 * sg + jj
                    for j4 in range(4):
                        eg = 2 * j4 + eo
                        for h01, pah in ((0, pa0), (1, pa1)):
                            lhs_v = v_t[64 * h01: 64 * h01 + 64,
                                        VST * pair + 17 * eg: VST * pair + 17 * eg + 32]
                            nc.tensor.matmul(
                                pah[32 * j4: 32 * j4 + 32, 64 * lp: 64 * lp + 64],
                                lhs_v,
                                es[64 * h01: 64 * h01 + 64,
                                   128 * j4 + 64 * jj: 128 * j4 + 64 * jj + 64],
                                start=True, stop=True,
                                tile_position=(64 * h01, 32 * j4))
            # Sample-major attH: col = b*8192 + t*128 + pix, so the norm2
            # stats/apply see whole samples as dense 128-col blocks.
            b_ = grp // 16
            pp0 = (8 * grp) % 128 // 2
            att5 = attH[:].rearrange("c (b t pp z) -> c b t pp z", b=2, t=T, z=2)
            for h01, pah in ((0, pa0), (1, pa1)):
                dstv = att5[:, b_, :, pp0: pp0 + 4, h01].rearrange(
                    "c t l -> c l t")
                srcv = pah[:].rearrange("c (lp z) -> c lp z", z=64)
                if h01 == 0:
                    nc.vector.tensor_copy(dstv, srcv)
                else:
                    nc.scalar.copy(dstv, srcv)
            # chunk n (samples 4n..4n+4) is complete once all 16 groups of
            # its b-half have written their pixel stripes
            if grp == 15:
                rowsum_stats_block(0)
                rowsum_stats_block(1)
            elif grp == 31:
                rowsum_stats_block(2)
                rowsum_stats_block(3)

        st2 = sm.tile([128, 2 * NSAMP], F32, tag=f"st2_{eo}")
        nc.vector.tensor_copy(st2[:, :NSAMP], s1b[:])
        nc.vector.tensor_copy(st2[:, NSAMP:], s2b[:])
        cc2i = dram.tile([128, 2 * NSAMP], F32, tag="cc_in")
        cc2o = dram.tile([128, 2 * NSAMP], F32, tag="cc_out")
        nc.gpsimd.dma_start(cc2i[:], st2[:])
        nc.gpsimd.collective_compute("AllReduce", ALU.add,
                                     replica_groups=[list(range(NCORES))],
                                     ins=[cc2i[:].opt()], outs=[cc2o[:].opt()])
        nc.sync.dma_start(st2[:], cc2o[:])
        if eo == 0:
            nc.sync.dma_start(attE_d[:], attH[:])
        return st2

    st2E = att_pass(0, cst["expbE"], "woE", "n2wE", "n2bE")
    st2O = att_pass(1, cst["expbO"], "woO", "n2wO", "n2bO")

    # norm2 coefficients for both parities (deferred so neither parity's
    # post-AllReduce work head-of-line blocks the other pass's queues)
    sc2E = sm.tile([128, NSAMP], F32, tag="sc2E")
    sf2E = sm.tile([128, NSAMP], F32, tag="sf2E")
    sc2O = sm.tile([128, NSAMP], F32, tag="sc2O")
    sf2O = sm.tile([128, NSAMP], F32, tag="sf2O")
    m2 = sm.tile([128, NSAMP], F32, tag="m2")
    t2 = sm.tile([128, NSAMP], F32, tag="t2")
    norm_coeffs(st2E, sc2E, sf2E, m2, t2, cst["n2wE"], cst["n2bE"])
    norm_coeffs(st2O, sc2O, sf2O, m2, t2, cst["n2wO"], cst["n2bO"])

    # ---------------- Phase 9: conv2 + gamma + residual + store ----------
    # Sample-major chunks: chunk n = samples 4n..4n+4, each a dense 128-col
    # block.  norm2 is applied LAZILY here (per-sample tensor_scalar with
    # per-partition coeffs) right before the conv matmul; x / y DMAs are
    # contiguous 128x128 blocks per sample.
    xb = x.rearrange("(t b p) c -> b t p c", b=2, p=128)
    yb4 = y.rearrange("(t b p) c -> b t p c", b=2, p=128)
    for n in range(32):
        sl = slice(512 * n, 512 * (n + 1))
        aA = scr.tile([128, 512], BF16, tag="qh", name="aA")
        nc.sync.dma_start(aA[:], attE_d[:, sl])
        aAn = scr.tile([128, 512], BF16, tag="Rq", name="aAn")
        aOn = scr.tile([128, 512], BF16, tag="Rk", name="aOn")
        for j in range(4):
            smp = 4 * n + j
            csl = slice(128 * j, 128 * (j + 1))
            nc.vector.tensor_scalar(out=aAn[:, csl], in0=aA[:, csl],
                                    scalar1=sc2E[:, smp: smp + 1],
                                    scalar2=sf2E[:, smp: smp + 1],
                                    op0=ALU.mult, op1=ALU.add)
            nc.vector.tensor_scalar(out=aOn[:, csl], in0=attH[:, sl][:, csl],
                                    scalar1=sc2O[:, smp: smp + 1],
                                    scalar2=sf2O[:, smp: smp + 1],
                                    op0=ALU.mult, op1=ALU.add)
        po = ps.tile([128, 512], F32, tag="a", name="po")
        nc.tensor.matmul(po[:], cst["woE"][:], aAn[:], start=True, stop=False)
        nc.tensor.matmul(po[:], cst["woO"][:], aOn[:], start=False, stop=True)
        yb = scr.tile([128, 512], BF16, tag="sqq", name="yb")
        nc.vector.tensor_scalar(out=yb[:], in0=po[:], scalar1=cst["b_o"][:, 0:1],
                                scalar2=cst["gamma"][:, 0:1], op0=ALU.add, op1=ALU.mult)
        xr = scr.tile([128, 512], F32, tag="ld", name="xr")
        b_, t0 = (4 * n) // 64, (4 * n) % 64
        nc.sync.dma_start(
            xr[:].rearrange("p (j c) -> p j c", j=4),
            xb[b_, t0: t0 + 4].rearrange("j p c -> p j c"))
        y8 = scr.tile([128, 512], F32, tag="y8f", name="y8")
        for j in range(4):
            pt = ps.tile([128, 128], BF16, tag="a", name="tp2")
            nc.tensor.transpose(pt[:], yb[:, 128 * j: 128 * (j + 1)], cst["identb"][:])
            nc.vector.tensor_tensor(out=y8[:, 128 * j: 128 * (j + 1)], in0=pt[:],
                                    in1=xr[:, 128 * j: 128 * (j + 1)], op=ALU.add)
        nc.sync.dma_start(
            yb4[b_, t0: t0 + 4].rearrange("j p c -> p j c"),
            y8[:].rearrange("p (j c) -> p j c", j=4))

    for p_ in reversed(list(pools.values())):
        p_.release()


# ---- public entry point -------------------------------------------------
_NC = None


def _get_nc():
    global _NC
    if _NC is None:
        _NC = build_nc(ln_general=True)
    return _NC


def kernel(**inputs) -> np.ndarray:
    import ml_dtypes
    from concourse import bass_utils

    nc = _get_nc()
    consts = host_prep(inputs)
    cmap = {}
    for name, val in consts.items():
        v = np.asarray(val, np.float32)
        if name in _BF16_IN:
            v = v.astype(ml_dtypes.bfloat16)
        cmap[name] = v
    x = np.asarray(inputs["x"], np.float32)
    in_maps = []
    for c in range(NCORES):
        m = dict(cmap)
        m["x"] = np.ascontiguousarray(
            x[:, :, HL * c: HL * (c + 1), :, :].reshape(-1, C))
        in_maps.append(m)
    res = bass_utils.run_bass_kernel_spmd(
        nc, in_maps, core_ids=list(range(NCORES)), trace=False)
    y = np.zeros((T, B, H, W, C), np.float32)
    for c in range(NCORES):
        y[:, :, HL * c: HL * (c + 1), :, :] = \
            res.results[c]["y"].reshape(T, B, HL, W, C)
    return y


def kernel_traced(**inputs):
    """Like kernel() but returns (y, per_core_exec_ns, trace_path)."""
    import ml_dtypes
    from concourse import bass_utils

    nc = _get_nc()
    consts = host_prep(inputs)
    cmap = {}
    for name, val in consts.items():
        v = np.asarray(val, np.float32)
        if name in _BF16_IN:
            v = v.astype(ml_dtypes.bfloat16)
        cmap[name] = v
    x = np.asarray(inputs["x"], np.float32)
    in_maps = []
    for c in range(NCORES):
        m = dict(cmap)
        m["x"] = np.ascontiguousarray(
            x[:, :, HL * c: HL * (c + 1), :, :].reshape(-1, C))
        in_maps.append(m)
    res = bass_utils.run_bass_kernel_spmd(
        nc, in_maps, core_ids=list(range(NCORES)),
        trace=True, trace_cores=list(range(NCORES)))
    y = np.zeros((T, B, H, W, C), np.float32)
    for c in range(NCORES):
        y[:, :, HL * c: HL * (c + 1), :, :] = \
            res.results[c]["y"].reshape(T, B, HL, W, C)
    trace_path = (res.instructions_and_trace[1]
                  if res.instructions_and_trace else None)
    return y, res.exec_time_ns, trace_path



# revision 40
# speedup vs baseline: 1.5797x; 1.2302x over previous
"""8-NeuronCore Trainium2 Bass kernel for nn_AttentionBlock_17789754540111.

Self-contained: builds a hand-written Bass/Tile SPMD program (H-sharded over
8 cores, params replicated, instance-norm stats combined via on-device
AllReduce) and runs it on the 8 axon-tunneled TRN2 cores via
concourse.bass_utils.run_bass_kernel_spmd.
"""

import sys
import types
import ctypes
import contextlib

# ---- NTFF profile hook expected by bass_utils under axon ----------------
def _install_axon_hook():
    if "antenv.axon_hooks" in sys.modules:
        return
    hookmod = types.ModuleType("antenv.axon_hooks")

    def _make_hook():
        try:
            lib = ctypes.CDLL("/opt/axon/libaxon_pjrt.so")
        except OSError:
            return None
        if not hasattr(lib, "axon_start_nrt_profile"):
            return None
        lib.axon_start_nrt_profile.argtypes = [ctypes.POINTER(ctypes.c_int64),
                                               ctypes.c_size_t]
        lib.axon_start_nrt_profile.restype = ctypes.c_int64
        lib.axon_stop_nrt_profile.argtypes = [ctypes.c_char_p]
        lib.axon_stop_nrt_profile.restype = ctypes.c_int64

        @contextlib.contextmanager
        def _hook(output_dir, device_ids):
            import jax
            jax.devices()
            if device_ids:
                ids = (ctypes.c_int64 * len(device_ids))(*device_ids)
                rc = lib.axon_start_nrt_profile(ids, len(device_ids))
            else:
                rc = lib.axon_start_nrt_profile(None, 0)
            if rc != 0:
                raise RuntimeError(f"axon_start_nrt_profile rc={rc}")
            try:
                yield
            finally:
                lib.axon_stop_nrt_profile(str(output_dir).encode())
        return _hook

    hook = _make_hook()
    hookmod.get_axon_ntff_profile_hook = lambda: hook
    hookmod.set_axon_ntff_profile_hook = lambda h: None
    sys.modules["antenv.axon_hooks"] = hookmod

_install_axon_hook()

import math
import sys

sys.path.insert(0, "/opt/trn_rl_repo")

import numpy as np

import concourse.bass as bass  # noqa
import concourse.bacc as bacc
import concourse.mybir as mybir
from concourse import tile

F32 = mybir.dt.float32
BF16 = mybir.dt.bfloat16
AF = mybir.ActivationFunctionType
ALU = mybir.AluOpType
AX = mybir.AxisListType

T, B, H, W, C = 64, 2, 32, 32, 128
HE, HD = 8, 16
NCORES = 8
HL = H // NCORES
PIX = B * HL * W                  # 256
NP_ = T * PIX                     # 16384
NSAMP = T * B                     # 128
SPATIAL = H * W                   # 1024 (global)
EPS = 1e-5
VST = 136
NPAIR = PIX // 2                  # 128


def _rel_bias_table(rel_emb):
    rp = np.arange(T)[None, :] - np.arange(T)[:, None]
    n = -rp
    ret = (n < 0).astype(np.int64) * 16
    n = np.abs(n)
    mx = 8
    small = n < mx
    vl = mx + (np.log(np.maximum(n, 1) / mx) / math.log(32 / mx) * 8).astype(np.int64)
    vl = np.minimum(vl, 15)
    buckets = ret + np.where(small, n, vl)
    return np.ascontiguousarray(
        rel_emb[buckets].transpose(2, 0, 1)).astype(np.float32)  # (he, tq, tk)


def host_prep(inputs):
    w_in = np.asarray(inputs["w_in"], np.float32)
    b_in = np.asarray(inputs["b_in"], np.float32)
    rows = w_in.reshape(HE, 3, HD, C)
    b3 = b_in.reshape(HE, 3, HD)
    wq = rows[:, 0].reshape(HE * HD, C)
    wk = rows[:, 1].reshape(HE * HD, C)
    wv = rows[:, 2].reshape(HE * HD, C)

    bias = _rel_bias_table(np.asarray(inputs["rel_emb"], np.float32))
    eb = np.exp(bias)                                          # (he, tq, tk)
    expb, wo, n2w, n2b = {}, {}, {}, {}
    n2w_full = np.asarray(inputs["norm2_w"], np.float32)
    n2b_full = np.asarray(inputs["norm2_b"], np.float32)
    w_out = np.asarray(inputs["w_out"], np.float32)
    for eo in (0, 1):
        heads = [2 * j + eo for j in range(4)]
        h = eb[heads]                                          # (4, tq, tk)
        # col = j4*128 + jj*64 + tq ; rows = 2 px * 64 tk
        e1 = h.transpose(2, 0, 1)                              # (tk, j, tq)
        e2 = np.stack([e1, e1], axis=2).reshape(T, 4 * 2 * T)  # (tk, (j, jj, tq))
        expb[eo] = np.concatenate([e2, e2], axis=0).astype(np.float32)
        m = np.zeros((128, 128), np.float32)
        wv_ = np.zeros((128, 1), np.float32)
        bv_ = np.zeros((128, 1), np.float32)
        for j, he in enumerate(heads):
            m[32 * j: 32 * j + 16, :] = w_out[:, he * 16: (he + 1) * 16].T
            wv_[32 * j: 32 * j + 16, 0] = n2w_full[he * 16: (he + 1) * 16]
            bv_[32 * j: 32 * j + 16, 0] = n2b_full[he * 16: (he + 1) * 16]
        wo[eo], n2w[eo], n2b[eo] = m, wv_, bv_

    J = np.kron(np.eye(HE, dtype=np.float32), np.ones((HD, HD), np.float32))
    E4 = np.zeros((4, 128), np.float32)
    for s in range(4):
        E4[s, 32 * s: 32 * s + 16] = 1.0
    sel4 = np.zeros((128, 4), np.float32)
    for s in range(4):
        sel4[32 * s + 16, s] = 1.0
    sel32 = np.zeros((8, 128, 32), np.float32)
    for j in range(8):
        for s in range(4):
            sel32[j, 32 * s + 16, 4 * j + s] = 1.0
    E4b = np.zeros((8, 32, 128), np.float32)
    for j in range(8):
        for s in range(4):
            E4b[j, 4 * j + s, 32 * s: 32 * s + 16] = 1.0

    # Centered conv weights: folding the head-mean subtraction of the q/k
    # LayerNorm into the 1x1 conv (q_hat = (I - J/HD) q is exactly q - mean).
    Cc = np.eye(128, dtype=np.float32) - J / HD
    wq2 = Cc @ wq
    wk2 = Cc @ wk
    bq2 = Cc @ b3[:, 0].reshape(-1)
    bk2 = Cc @ b3[:, 1].reshape(-1)

    qn_w = np.asarray(inputs["qn_w"], np.float32)
    kn_w = np.asarray(inputs["kn_w"], np.float32)
    qn_b = np.asarray(inputs["qn_b"], np.float32)
    kn_b = np.asarray(inputs["kn_b"], np.float32)
    # Packed-variance selectors: chunk j (j%4 == m) of a 16-chunk batch puts
    # head h's variance at packed row 32*(j//4) + 8*m + h.
    S16P = np.zeros((4, 128, 32), np.float32)
    S8qP = np.zeros((4, 128, 128), np.float32)
    S8kEP = np.zeros((4, 128, 128), np.float32)
    S8kOP = np.zeros((4, 128, 128), np.float32)
    knbE = np.zeros(128, np.float32)
    knbO = np.zeros(128, np.float32)
    for p in range(128):
        h = p // 16
        if h % 2 == 0:
            knbE[p] = kn_b[p % 16]
        else:
            knbO[p] = kn_b[p % 16]
        for m in range(4):
            S16P[m, p, 8 * m + h] = 1.0 / HD
            for b in range(4):
                r = 32 * b + 8 * m + h
                S8qP[m, r, p] = qn_w[p % 16]
                if h % 2 == 0:
                    S8kEP[m, r, p] = kn_w[p % 16]
                else:
                    S8kOP[m, r, p] = kn_w[p % 16]

    col = lambda a: np.asarray(a, np.float32).reshape(-1, 1)
    return {
        "ident": np.eye(128, dtype=np.float32),
        "identb": np.eye(128, dtype=np.float32),
        "wqT": wq2.T.copy(), "wkT": wk2.T.copy(), "wvT": wv.T.copy(),
        "bq": col(bq2), "bk": col(bk2),
        "bvrow": np.broadcast_to(b3[:, 2].reshape(-1), (128, 128)).copy(),
        "expbE": expb[0], "expbO": expb[1],
        "E4": E4, "woE": wo[0], "woO": wo[1],
        "sel4": sel4,
        **{f"S16P{m}": S16P[m] for m in range(4)},
        **{f"S8qP{m}": S8qP[m] for m in range(4)},
        **{f"S8kEP{m}": S8kEP[m] for m in range(4)},
        **{f"S8kOP{m}": S8kOP[m] for m in range(4)},
        **{f"E4b{j}": E4b[j] for j in range(8)},
        **{f"sel32_{j}": sel32[j] for j in range(8)},
        "b_o": col(inputs["b_out"]), "gamma": col(inputs["gamma"]),
        "n1w": col(inputs["norm1_w"]), "n1b": col(inputs["norm1_b"]),
        "epscol": np.full((128, 1), EPS, np.float32),
        "n2wE": n2w[0], "n2bE": n2b[0], "n2wO": n2w[1], "n2bO": n2b[1],
        "qnb": col(np.tile(qn_b, HE)),
        "knbE": col(knbE), "knbO": col(knbO),
    }


_BF16_IN = {"wqT", "wkT", "wvT", "bvrow", "expbE", "expbO", "E4",
            "woE", "woO", "identb", "sel4",
            *[f"S16P{m}" for m in range(4)], *[f"S8qP{m}" for m in range(4)],
            *[f"S8kEP{m}" for m in range(4)], *[f"S8kOP{m}" for m in range(4)],
            *[f"E4b{j}" for j in range(8)],
            *[f"sel32_{j}" for j in range(8)]}
_CONST_SHAPES = {
    "ident": (128, 128), "identb": (128, 128), "wqT": (128, 128), "wkT": (128, 128), "wvT": (128, 128),
    "bq": (128, 1), "bk": (128, 1), "bvrow": (128, 128),
    "expbE": (128, 512), "expbO": (128, 512), "E4": (4, 128),
    "sel4": (128, 4),
    **{f"S16P{m}": (128, 32) for m in range(4)},
    **{f"S8qP{m}": (128, 128) for m in range(4)},
    **{f"S8kEP{m}": (128, 128) for m in range(4)},
    **{f"S8kOP{m}": (128, 128) for m in range(4)},
    **{f"E4b{j}": (32, 128) for j in range(8)},
    **{f"sel32_{j}": (128, 32) for j in range(8)},
    "woE": (128, 128), "woO": (128, 128), "b_o": (128, 1), "gamma": (128, 1),
    "n1w": (128, 1), "n1b": (128, 1), "epscol": (128, 1),
    "n2wE": (128, 1), "n2bE": (128, 1), "n2wO": (128, 1), "n2bO": (128, 1),
    "qnb": (128, 1), "knbE": (128, 1), "knbO": (128, 1),
}


def build_nc(ln_general=True):
    nc = bacc.Bacc("TRN2", target_bir_lowering=False, debug=False,
                   enable_asserts=False, num_devices=NCORES)
    ins = {"x": nc.dram_tensor("x", [NP_, C], F32, kind="ExternalInput").ap()}
    for name, shp in _CONST_SHAPES.items():
        dt = BF16 if name in _BF16_IN else F32
        ins[name] = nc.dram_tensor(name, list(shp), dt, kind="ExternalInput").ap()
    y = nc.dram_tensor("y", [NP_, C], F32, kind="ExternalOutput").ap()
    with tile.TileContext(nc) as tc:
        _body(tc, nc, ins, y, ln_general)
    nc.compile()
    return nc


def _body(tc, nc, ins, y, ln_general):
    x = ins["x"]
    pools = {}

    def pool(name, bufs=1, space="SBUF"):
        if name not in pools:
            pools[name] = tc.alloc_tile_pool(name=name, bufs=bufs, space=space)
        return pools[name]

    cp = pool("consts")
    big = pool("big")
    ps = pool("psA", bufs=4, space="PSUM")
    ps_s = pool("psS", bufs=2, space="PSUM")
    dram = pool("dram", bufs=1, space="DRAM")
    sm = pool("small", bufs=1)
    scr = pool("scr", bufs=2)
    esp = pool("esp", bufs=2)

    cst = {}
    for name in _CONST_SHAPES:
        ap = ins[name]
        t = cp.tile(list(ap.shape), ap.dtype, tag=name)
        nc.sync.dma_start(t[:], ap)
        cst[name] = t

    # Big slots: S1 {xT, v_t} / S2 {kk} / S3 {xn, attH} / S4 {q} / S5 {sq, kEO}
    xT = big.tile([128, VST * NPAIR + 32], BF16, tag="S1", name="xT")[:, :NP_]
    q = big.tile([128, NP_], BF16, tag="S4")
    kk = big.tile([128, NP_], BF16, tag="S2")

    # ---------------- Phase 0: load + transpose + norm1 stats ------------
    # Stats are computed per quarter as the transposed data lands, so the
    # reduce work overlaps the DMA/PE of later quarters and the AllReduce
    # can fire right after the last chunk.
    s1 = sm.tile([128, NSAMP], F32, tag="s1")
    s2 = sm.tile([128, NSAMP], F32, tag="s2")
    sq = big.tile([128, NP_], BF16, tag="S5")
    for k8 in range(32):
        ld = scr.tile([128, 512], F32, tag="ld", name="ld")
        src = x.rearrange("(k8 k p) c -> k8 p k c", p=128, k=4)[k8]
        nc.sync.dma_start(ld[:].rearrange("p (k c) -> p k c", k=4), src)
        for j in range(4):
            k = 4 * k8 + j
            pt = ps.tile([128, 128], F32, tag="a", name="tp")
            nc.tensor.transpose(pt[:], ld[:, 128 * j: 128 * (j + 1)], cst["ident"][:])
            if j % 2 == 0:
                nc.vector.tensor_copy(xT[:, 128 * k: 128 * (k + 1)], pt[:])
            else:
                nc.scalar.copy(xT[:, 128 * k: 128 * (k + 1)], pt[:])
        if k8 % 8 == 7:
            qq = k8 // 8
            qsl = slice(4096 * qq, 4096 * (qq + 1))
            ssl = slice(32 * qq, 32 * (qq + 1))
            nc.vector.reduce_sum(
                s1[:, ssl], xT[:, qsl].rearrange("p (k s) -> p k s", s=128),
                axis=AX.X)
            nc.scalar.square(sq[:, qsl], xT[:, qsl])
            nc.vector.reduce_sum(
                s2[:, ssl], sq[:, qsl].rearrange("p (k s) -> p k s", s=128),
                axis=AX.X)

    stats = sm.tile([128, 2 * NSAMP], F32, tag="st2")
    nc.vector.tensor_copy(stats[:, :NSAMP], s1[:])
    nc.vector.tensor_copy(stats[:, NSAMP:], s2[:])
    cc_in = dram.tile([128, 2 * NSAMP], F32, tag="cc_in")
    cc_out = dram.tile([128, 2 * NSAMP], F32, tag="cc_out")
    nc.gpsimd.dma_start(cc_in[:], stats[:])
    nc.gpsimd.collective_compute("AllReduce", ALU.add,
                                 replica_groups=[list(range(NCORES))],
                                 ins=[cc_in[:].opt()], outs=[cc_out[:].opt()])
    nc.sync.dma_start(stats[:], cc_out[:])

    mean = sm.tile([128, NSAMP], F32, tag="m2")
    scl1 = sm.tile([128, NSAMP], F32, tag="sc2")
    sft1 = sm.tile([128, NSAMP], F32, tag="sf2")
    tmp = sm.tile([128, NSAMP], F32, tag="t2")

    def norm_coeffs(st, scl, sft, mn, tp, w_ap, b_ap):
        nc.vector.tensor_scalar_mul(mn[:], st[:, :NSAMP], 1.0 / SPATIAL)
        nc.vector.tensor_scalar_mul(tp[:], st[:, NSAMP:], 1.0 / SPATIAL)
        nc.vector.tensor_tensor(out=scl[:], in0=mn[:], in1=mn[:], op=ALU.mult)
        nc.vector.tensor_tensor(out=tp[:], in0=tp[:], in1=scl[:], op=ALU.subtract)
        nc.scalar.activation(tp[:], tp[:], AF.Sqrt,
                             bias=cst["epscol"][:, 0:1], scale=1.0)
        nc.vector.reciprocal_approx_fast(out=scl[:], in_=tp[:])
        nc.vector.tensor_scalar_mul(scl[:], scl[:], w_ap[:, 0:1])
        nc.vector.tensor_tensor(out=sft[:], in0=mn[:], in1=scl[:], op=ALU.mult)
        nc.vector.tensor_scalar(out=sft[:], in0=sft[:], scalar1=-1.0,
                                scalar2=b_ap[:, 0:1], op0=ALU.mult, op1=ALU.add)

    norm_coeffs(stats, scl1, sft1, mean, tmp, cst["n1w"], cst["n1b"])

    # ---------------- Phase 2: norm1 apply -------------------------------
    xn = big.tile([128, NP_], BF16, tag="S3")
    for k in range(NSAMP):
        sl = slice(128 * k, 128 * (k + 1))
        nc.vector.tensor_scalar(out=xn[:, sl], in0=xT[:, sl],
                                scalar1=scl1[:, k: k + 1], scalar2=sft1[:, k: k + 1],
                                op0=ALU.mult, op1=ALU.add)

    # ---------------- Phase 3+4: conv1 + head-LN (centered weights) ------
    # wqT/wkT are pre-centered host-side: the conv directly yields
    # q_hat = q - mean_head(q) (written straight into q / kE).  Per-head
    # variances for 16 chunks are PACKED into one [128,512] PSUM tile via
    # 32-aligned accumulating selector matmuls (chunk j's 8 head-rows land
    # at partitions 8j), so ONE Sqrt + ONE reciprocal_approx_fast serves a
    # whole batch -- no activation-table thrash, no per-chunk 8-lane ops.
    # A second sweep broadcasts rstd back per chunk (band-sliced matmuls)
    # and applies scale+bias; the k apply writes both parity-masked copies
    # (kE: even heads, kO into kk's slot: odd).
    kE = big.tile([128, NP_], BF16, tag="S5")
    for bi in range(2):
        pvq = ps_s.tile([128, 512], F32, tag="s", name="pvq", bufs=2)
        pvk = ps_s.tile([128, 512], F32, tag="s", name="pvk", bufs=2)
        for jj in range(16):
            n = 16 * bi + jj
            sl = slice(512 * n, 512 * (n + 1))
            b, m = jj // 4, jj % 4
            pq = ps.tile([128, 512], F32, tag="a", name="pq")
            nc.tensor.matmul(pq[:], cst["wqT"][:], xn[:, sl], start=True, stop=True)
            nc.scalar.add(q[:, sl], pq[:], cst["bq"][:, 0:1])
            sqq = scr.tile([128, 512], BF16, tag="sqq")
            nc.vector.tensor_tensor(out=sqq[:], in0=q[:, sl], in1=q[:, sl],
                                    op=ALU.mult)
            nc.tensor.matmul(pvq[32 * b: 32 * b + 32, :], cst[f"S16P{m}"][:],
                             sqq[:], start=(m == 0), stop=(m == 3),
                             tile_position=(0, 32 * b))
            pk = ps.tile([128, 512], F32, tag="a", name="pk")
            nc.tensor.matmul(pk[:], cst["wkT"][:], xn[:, sl], start=True, stop=True)
            nc.scalar.add(kE[:, sl], pk[:], cst["bk"][:, 0:1])
            sqk = scr.tile([128, 512], BF16, tag="sqq", name="sqk")
            nc.vector.tensor_tensor(out=sqk[:], in0=kE[:, sl], in1=kE[:, sl],
                                    op=ALU.mult)
            nc.tensor.matmul(pvk[32 * b: 32 * b + 32, :], cst[f"S16P{m}"][:],
                             sqk[:], start=(m == 0), stop=(m == 3),
                             tile_position=(0, 32 * b))
        # batch rstd: std = sqrt(var + eps) on ACT, 1/std via the fast
        # Newton-Raphson custom DVE op (fp32), then cast for the matmuls.
        rq = scr.tile([128, 512], F32, tag="ld", name="rq")
        nc.scalar.activation(rq[:], pvq[:], AF.Sqrt,
                             bias=cst["epscol"][:, 0:1], scale=1.0)
        nc.vector.reciprocal_approx_fast(out=rq[:], in_=rq[:])
        rqb = scr.tile([128, 512], BF16, tag="rqb")
        nc.vector.tensor_copy(rqb[:], rq[:])
        rk = scr.tile([128, 512], F32, tag="ld", name="rk")
        nc.scalar.activation(rk[:], pvk[:], AF.Sqrt,
                             bias=cst["epscol"][:, 0:1], scale=1.0)
        nc.vector.reciprocal_approx_fast(out=rk[:], in_=rk[:])
        rkb = scr.tile([128, 512], BF16, tag="rqb", name="rkb")
        nc.vector.tensor_copy(rkb[:], rk[:])
        for jj in range(16):
            n = 16 * bi + jj
            sl = slice(512 * n, 512 * (n + 1))
            b, m = jj // 4, jj % 4
            bsl = slice(32 * b, 32 * b + 32)
            pRq = ps.tile([128, 512], F32, tag="a", name="pRq")
            nc.tensor.matmul(pRq[:], cst[f"S8qP{m}"][bsl, :], rqb[bsl, :],
                             start=True, stop=True, tile_position=(32 * b, 0))
            Rq = scr.tile([128, 512], BF16, tag="Rq")
            nc.scalar.copy(Rq[:], pRq[:])
            nc.vector.tensor_tensor(out=q[:, sl], in0=q[:, sl], in1=Rq[:],
                                    op=ALU.mult)
            nc.vector.tensor_scalar_add(q[:, sl], q[:, sl], cst["qnb"][:, 0:1])
            # kO first (reads pre-scale kE), then kE in place
            for sname, bname, dst in (("S8kOP", "knbO", kk), ("S8kEP", "knbE", kE)):
                pRk = ps.tile([128, 512], F32, tag="a", name="pRk")
                nc.tensor.matmul(pRk[:], cst[f"{sname}{m}"][bsl, :], rkb[bsl, :],
                                 start=True, stop=True, tile_position=(32 * b, 0))
                Rk = scr.tile([128, 512], BF16, tag="Rk")
                nc.scalar.copy(Rk[:], pRk[:])
                nc.vector.tensor_tensor(out=dst[:, sl], in0=kE[:, sl], in1=Rk[:],
                                        op=ALU.mult)
                nc.vector.tensor_scalar_add(dst[:, sl], dst[:, sl],
                                            cst[bname][:, 0:1])

    # ---------------- Phase 5: conv v (transposed layout) ----------------
    v_t = big.tile([128, VST * NPAIR + 32], BF16, tag="S1")
    nc.vector.memset(v_t[:], 0.0)
    ones_ap = v_t[:, :VST * NPAIR].rearrange(
        "p (m e d) -> p m e d", e=8, d=17)[:, :, :, 16:17]
    nc.vector.memset(ones_ap, 1.0)
    xnv = xn[:].rearrange("c (t m z) -> c t m z", m=NPAIR, z=2)
    for m in range(NPAIR):
        pv = ps.tile([128, 128], F32, tag="a", name="pvt")
        for z in range(2):
            nc.tensor.matmul(pv[64 * z: 64 * z + 64, :], xnv[:, :, m, z],
                             cst["wvT"][:], start=True, stop=True,
                             tile_position=(0, 64 * z))
        dst = v_t[:, VST * m: VST * m + 136].rearrange(
            "p (e d) -> p e d", d=17)[:, :, 0:16]
        nc.vector.tensor_tensor(out=dst,
                                in0=pv[:].rearrange("p (e d) -> p e d", d=16),
                                in1=cst["bvrow"][:].rearrange("p (e d) -> p e d", d=16),
                                op=ALU.add)

    # ---------------- Phases 6-8: attention (two head parities) ----------
    attH = big.tile([128, NP_], BF16, tag="S3")
    attE_d = dram.tile([128, NP_], BF16, tag="attE_d")
    qv = q[:].rearrange("a (t p) -> a t p", p=256)
    kvE = kE[:].rearrange("a (t p) -> a t p", p=256)
    kvO = kk[:].rearrange("a (t p) -> a t p", p=256)

    def att_pass(eo, expb_c, woname, wname, bname):
        kv = kvE if eo == 0 else kvO
        s1b = sm.tile([128, NSAMP], F32, tag="s1")
        s2b = sm.tile([128, NSAMP], F32, tag="s2")

        def rowsum_stats_block(nb):
            # rowsum division for chunks 8nb..8nb+8 + their norm2 stats,
            # emitted right after the producing groups so the DVE work
            # overlaps later groups' matmuls.
            rs_blk = esp.tile([32, 512], F32, tag="rs_blk", bufs=1)
            rsb_blk = esp.tile([32, 512], BF16, tag="rsb_blk", bufs=1)
            prs = ps.tile([32, 512], F32, tag="a", name="prs")
            for j in range(8):
                n = 8 * nb + j
                sl = slice(512 * n, 512 * (n + 1))
                nc.tensor.matmul(prs[:], cst[f"sel32_{j}"][:], attH[:, sl],
                                 start=(j == 0), stop=(j == 7))
            nc.vector.reciprocal(rs_blk[:], prs[:])
            nc.vector.tensor_copy(rsb_blk[:], rs_blk[:])
            for j in range(8):
                n = 8 * nb + j
                sl = slice(512 * n, 512 * (n + 1))
                pb = ps_s.tile([128, 512], F32, tag="s", name="pb")
                nc.tensor.matmul(pb[:], cst[f"E4b{j}"][:], rsb_blk[:],
                                 start=True, stop=True)
                rbc = esp.tile([128, 512], BF16, tag="rbc", bufs=1)
                nc.scalar.copy(rbc[:], pb[:])
                nc.vector.tensor_tensor(out=attH[:, sl], in0=attH[:, sl],
                                        in1=rbc[:], op=ALU.mult)
                sqc = scr.tile([128, 512], BF16, tag="Rq", name="sqc")
                nc.scalar.square(sqc[:], attH[:, sl])
                nc.vector.reduce_sum(
                    s1b[:, 4 * n: 4 * n + 4],
                    attH[:, sl].rearrange("c (s p) -> c s p", s=4), axis=AX.X)
                nc.vector.reduce_sum(
                    s2b[:, 4 * n: 4 * n + 4],
                    sqc[:].rearrange("c (s p) -> c s p", s=4), axis=AX.X)

        for grp in range(32):
            pa0 = ps.tile([128, 256], F32, tag="pa0", name="pa0", bufs=1)
            pa1 = ps.tile([128, 256], F32, tag="pa1", name="pa1", bufs=1)
            for sg in range(2):
                pairs = (8 * grp + 4 * sg, 8 * grp + 4 * sg + 2)
                es = esp.tile([128, 512], BF16, tag="es")
                # One PSUM bank PER ROW GROUP (concurrent row-tiled matmuls
                # must not write the same bank+partitions), with r-innermost
                # issue order so LDWEIGHTS of the next matmul (different row
                # group) overlaps the in-flight one.
                pqk = [ps.tile([128, 128], F32, tag="a", name=f"pqk{r}",
                               bufs=4) for r in range(4)]
                for jj, pbase in enumerate(pairs):
                    for h01 in range(2):
                        p = pbase + h01
                        for tkc in range(2):
                            for r in range(4):
                                prt = slice(32 * r, 32 * r + 32)
                                nc.tensor.matmul(
                                    pqk[r][64 * h01 + 32 * tkc:
                                           64 * h01 + 32 * tkc + 32,
                                           64 * jj: 64 * jj + 64],
                                    kv[prt, 32 * tkc: 32 * tkc + 32, p],
                                    qv[prt, :, p], start=True, stop=True,
                                    tile_position=(32 * r,
                                                   64 * h01 + 32 * tkc))
                for r in range(4):
                    nc.scalar.activation(es[:, 128 * r: 128 * r + 128],
                                         pqk[r][:], AF.Exp, bias=0.0, scale=0.25)
                nc.vector.tensor_tensor(out=es[:], in0=es[:], in1=expb_c[:],
                                        op=ALU.mult)
                for jj, pbase in enumerate(pairs):
                    pair = pbase // 2
                    lp = 2 * sg + jj
                    for j4 in range(4):
                        eg = 2 * j4 + eo
                        for h01, pah in ((0, pa0), (1, pa1)):
                            lhs_v = v_t[64 * h01: 64 * h01 + 64,
                                        VST * pair + 17 * eg: VST * pair + 17 * eg + 32]
                            nc.tensor.matmul(
                                pah[32 * j4: 32 * j4 + 32, 64 * lp: 64 * lp + 64],
                                lhs_v,
                                es[64 * h01: 64 * h01 + 64,
                                   128 * j4 + 64 * jj: 128 * j4 + 64 * jj + 64],
                                start=True, stop=True,
                                tile_position=(64 * h01, 32 * j4))
            # Sample-major attH: col = b*8192 + t*128 + pix, so the norm2
            # stats/apply see whole samples as dense 128-col blocks.
            b_ = grp // 16
            pp0 = (8 * grp) % 128 // 2
            att5 = attH[:].rearrange("c (b t pp z) -> c b t pp z", b=2, t=T, z=2)
            for h01, pah in ((0, pa0), (1, pa1)):
                dstv = att5[:, b_, :, pp0: pp0 + 4, h01].rearrange(
                    "c t l -> c l t")
                srcv = pah[:].rearrange("c (lp z) -> c lp z", z=64)
                if h01 == 0:
                    nc.vector.tensor_copy(dstv, srcv)
                else:
                    nc.scalar.copy(dstv, srcv)
            # chunk n (samples 4n..4n+4) is complete once all 16 groups of
            # its b-half have written their pixel stripes
            if grp == 15:
                rowsum_stats_block(0)
                rowsum_stats_block(1)
            elif grp == 31:
                rowsum_stats_block(2)
                rowsum_stats_block(3)

        st2 = sm.tile([128, 2 * NSAMP], F32, tag=f"st2_{eo}")
        nc.vector.tensor_copy(st2[:, :NSAMP], s1b[:])
        nc.vector.tensor_copy(st2[:, NSAMP:], s2b[:])
        cc2i = dram.tile([128, 2 * NSAMP], F32, tag="cc_in")
        cc2o = dram.tile([128, 2 * NSAMP], F32, tag="cc_out")
        nc.gpsimd.dma_start(cc2i[:], st2[:])
        nc.gpsimd.collective_compute("AllReduce", ALU.add,
                                     replica_groups=[list(range(NCORES))],
                                     ins=[cc2i[:].opt()], outs=[cc2o[:].opt()])
        nc.sync.dma_start(st2[:], cc2o[:])
        if eo == 0:
            nc.sync.dma_start(attE_d[:], attH[:])
        return st2

    st2E = att_pass(0, cst["expbE"], "woE", "n2wE", "n2bE")
    st2O = att_pass(1, cst["expbO"], "woO", "n2wO", "n2bO")

    # norm2 coefficients for both parities (deferred so neither parity's
    # post-AllReduce work head-of-line blocks the other pass's queues)
    sc2E = sm.tile([128, NSAMP], F32, tag="sc2E")
    sf2E = sm.tile([128, NSAMP], F32, tag="sf2E")
    sc2O = sm.tile([128, NSAMP], F32, tag="sc2O")
    sf2O = sm.tile([128, NSAMP], F32, tag="sf2O")
    m2 = sm.tile([128, NSAMP], F32, tag="m2")
    t2 = sm.tile([128, NSAMP], F32, tag="t2")
    norm_coeffs(st2E, sc2E, sf2E, m2, t2, cst["n2wE"], cst["n2bE"])
    norm_coeffs(st2O, sc2O, sf2O, m2, t2, cst["n2wO"], cst["n2bO"])

    # ---------------- Phase 9: conv2 + gamma + residual + store ----------
    # Sample-major chunks: chunk n = samples 4n..4n+4, each a dense 128-col
    # block.  norm2 is applied LAZILY here (per-sample tensor_scalar with
    # per-partition coeffs) right before the conv matmul; x / y DMAs are
    # contiguous 128x128 blocks per sample.
    xb = x.rearrange("(t b p) c -> b t p c", b=2, p=128)
    yb4 = y.rearrange("(t b p) c -> b t p c", b=2, p=128)
    for n in range(32):
        sl = slice(512 * n, 512 * (n + 1))
        aA = scr.tile([128, 512], BF16, tag="qh", name="aA")
        nc.sync.dma_start(aA[:], attE_d[:, sl])
        aAn = scr.tile([128, 512], BF16, tag="Rq", name="aAn")
        aOn = scr.tile([128, 512], BF16, tag="Rk", name="aOn")
        for j in range(4):
            smp = 4 * n + j
            csl = slice(128 * j, 128 * (j + 1))
            nc.vector.tensor_scalar(out=aAn[:, csl], in0=aA[:, csl],
                                    scalar1=sc2E[:, smp: smp + 1],
                                    scalar2=sf2E[:, smp: smp + 1],
                                    op0=ALU.mult, op1=ALU.add)
            nc.vector.tensor_scalar(out=aOn[:, csl], in0=attH[:, sl][:, csl],
                                    scalar1=sc2O[:, smp: smp + 1],
                                    scalar2=sf2O[:, smp: smp + 1],
                                    op0=ALU.mult, op1=ALU.add)
        po = ps.tile([128, 512], F32, tag="a", name="po")
        nc.tensor.matmul(po[:], cst["woE"][:], aAn[:], start=True, stop=False)
        nc.tensor.matmul(po[:], cst["woO"][:], aOn[:], start=False, stop=True)
        yb = scr.tile([128, 512], BF16, tag="sqq", name="yb")
        nc.vector.tensor_scalar(out=yb[:], in0=po[:], scalar1=cst["b_o"][:, 0:1],
                                scalar2=cst["gamma"][:, 0:1], op0=ALU.add, op1=ALU.mult)
        xr = scr.tile([128, 512], F32, tag="ld", name="xr")
        b_, t0 = (4 * n) // 64, (4 * n) % 64
        nc.sync.dma_start(
            xr[:].rearrange("p (j c) -> p j c", j=4),
            xb[b_, t0: t0 + 4].rearrange("j p c -> p j c"))
        y8 = scr.tile([128, 512], F32, tag="y8f", name="y8")
        for j in range(4):
            pt = ps.tile([128, 128], BF16, tag="a", name="tp2")
            nc.tensor.transpose(pt[:], yb[:, 128 * j: 128 * (j + 1)], cst["identb"][:])
            nc.vector.tensor_tensor(out=y8[:, 128 * j: 128 * (j + 1)], in0=pt[:],
                                    in1=xr[:, 128 * j: 128 * (j + 1)], op=ALU.add)
        nc.sync.dma_start(
            yb4[b_, t0: t0 + 4].rearrange("j p c -> p j c"),
            y8[:].rearrange("p (j c) -> p j c", j=4))

    for p_ in reversed(list(pools.values())):
        p_.release()


# ---- public entry point -------------------------------------------------
_NC = None


def _get_nc():
    global _NC
    if _NC is None:
        _NC = build_nc(ln_general=True)
    return _NC


def kernel(**inputs) -> np.ndarray:
    import ml_dtypes
    from concourse import bass_utils

    nc = _get_nc()
    consts = host_prep(inputs)
    cmap = {}
    for name, val in consts.items():
        v = np.asarray(val, np.float32)
        if name in _BF16_IN:
            v = v.astype(ml_dtypes.bfloat16)
        cmap[name] = v
    x = np.asarray(inputs["x"], np.float32)
    in_maps = []
    for c in range(NCORES):
        m = dict(cmap)
        m["x"] = np.ascontiguousarray(
            x[:, :, HL * c: HL * (c + 1), :, :].reshape(-1, C))
        in_maps.append(m)
    res = bass_utils.run_bass_kernel_spmd(
        nc, in_maps, core_ids=list(range(NCORES)), trace=False)
    y = np.zeros((T, B, H, W, C), np.float32)
    for c in range(NCORES):
        y[:, :, HL * c: HL * (c + 1), :, :] = \
            res.results[c]["y"].reshape(T, B, HL, W, C)
    return y


def kernel_traced(**inputs):
    """Like kernel() but returns (y, per_core_exec_ns, trace_path)."""
    import ml_dtypes
    from concourse import bass_utils

    nc = _get_nc()
    consts = host_prep(inputs)
    cmap = {}
    for name, val in consts.items():
        v = np.asarray(val, np.float32)
        if name in _BF16_IN:
            v = v.astype(ml_dtypes.bfloat16)
        cmap[name] = v
    x = np.asarray(inputs["x"], np.float32)
    in_maps = []
    for c in range(NCORES):
        m = dict(cmap)
        m["x"] = np.ascontiguousarray(
            x[:, :, HL * c: HL * (c + 1), :, :].reshape(-1, C))
        in_maps.append(m)
    res = bass_utils.run_bass_kernel_spmd(
        nc, in_maps, core_ids=list(range(NCORES)),
        trace=True, trace_cores=list(range(NCORES)))
    y = np.zeros((T, B, H, W, C), np.float32)
    for c in range(NCORES):
        y[:, :, HL * c: HL * (c + 1), :, :] = \
            res.results[c]["y"].reshape(T, B, HL, W, C)
    trace_path = (res.instructions_and_trace[1]
                  if res.instructions_and_trace else None)
    return y, res.exec_time_ns, trace_path



# revision 41
# speedup vs baseline: 1.6210x; 1.0262x over previous
"""8-NeuronCore Trainium2 Bass kernel for nn_AttentionBlock_17789754540111.

Self-contained: builds a hand-written Bass/Tile SPMD program (H-sharded over
8 cores, params replicated, instance-norm stats combined via on-device
AllReduce) and runs it on the 8 axon-tunneled TRN2 cores via
concourse.bass_utils.run_bass_kernel_spmd.
"""

import sys
import types
import ctypes
import contextlib

# ---- NTFF profile hook expected by bass_utils under axon ----------------
def _install_axon_hook():
    if "antenv.axon_hooks" in sys.modules:
        return
    hookmod = types.ModuleType("antenv.axon_hooks")

    def _make_hook():
        try:
            lib = ctypes.CDLL("/opt/axon/libaxon_pjrt.so")
        except OSError:
            return None
        if not hasattr(lib, "axon_start_nrt_profile"):
            return None
        lib.axon_start_nrt_profile.argtypes = [ctypes.POINTER(ctypes.c_int64),
                                               ctypes.c_size_t]
        lib.axon_start_nrt_profile.restype = ctypes.c_int64
        lib.axon_stop_nrt_profile.argtypes = [ctypes.c_char_p]
        lib.axon_stop_nrt_profile.restype = ctypes.c_int64

        @contextlib.contextmanager
        def _hook(output_dir, device_ids):
            import jax
            jax.devices()
            if device_ids:
                ids = (ctypes.c_int64 * len(device_ids))(*device_ids)
                rc = lib.axon_start_nrt_profile(ids, len(device_ids))
            else:
                rc = lib.axon_start_nrt_profile(None, 0)
            if rc != 0:
                raise RuntimeError(f"axon_start_nrt_profile rc={rc}")
            try:
                yield
            finally:
                lib.axon_stop_nrt_profile(str(output_dir).encode())
        return _hook

    hook = _make_hook()
    hookmod.get_axon_ntff_profile_hook = lambda: hook
    hookmod.set_axon_ntff_profile_hook = lambda h: None
    sys.modules["antenv.axon_hooks"] = hookmod

_install_axon_hook()

import math
import sys

sys.path.insert(0, "/opt/trn_rl_repo")

import numpy as np

import concourse.bass as bass  # noqa
import concourse.bacc as bacc
import concourse.mybir as mybir
from concourse import tile

F32 = mybir.dt.float32
BF16 = mybir.dt.bfloat16
AF = mybir.ActivationFunctionType
ALU = mybir.AluOpType
AX = mybir.AxisListType

T, B, H, W, C = 64, 2, 32, 32, 128
HE, HD = 8, 16
NCORES = 8
HL = H // NCORES
PIX = B * HL * W                  # 256
NP_ = T * PIX                     # 16384
NSAMP = T * B                     # 128
SPATIAL = H * W                   # 1024 (global)
EPS = 1e-5
VST = 136
NPAIR = PIX // 2                  # 128


def _rel_bias_table(rel_emb):
    rp = np.arange(T)[None, :] - np.arange(T)[:, None]
    n = -rp
    ret = (n < 0).astype(np.int64) * 16
    n = np.abs(n)
    mx = 8
    small = n < mx
    vl = mx + (np.log(np.maximum(n, 1) / mx) / math.log(32 / mx) * 8).astype(np.int64)
    vl = np.minimum(vl, 15)
    buckets = ret + np.where(small, n, vl)
    return np.ascontiguousarray(
        rel_emb[buckets].transpose(2, 0, 1)).astype(np.float32)  # (he, tq, tk)


def host_prep(inputs):
    w_in = np.asarray(inputs["w_in"], np.float32)
    b_in = np.asarray(inputs["b_in"], np.float32)
    rows = w_in.reshape(HE, 3, HD, C)
    b3 = b_in.reshape(HE, 3, HD)
    wq = rows[:, 0].reshape(HE * HD, C)
    wk = rows[:, 1].reshape(HE * HD, C)
    wv = rows[:, 2].reshape(HE * HD, C)

    bias = _rel_bias_table(np.asarray(inputs["rel_emb"], np.float32))
    eb = np.exp(bias)                                          # (he, tq, tk)
    expb, wo, n2w, n2b = {}, {}, {}, {}
    n2w_full = np.asarray(inputs["norm2_w"], np.float32)
    n2b_full = np.asarray(inputs["norm2_b"], np.float32)
    w_out = np.asarray(inputs["w_out"], np.float32)
    for eo in (0, 1):
        heads = [2 * j + eo for j in range(4)]
        h = eb[heads]                                          # (4, tq, tk)
        # col = j4*128 + jj*64 + tq ; rows = 2 px * 64 tk
        e1 = h.transpose(2, 0, 1)                              # (tk, j, tq)
        e2 = np.stack([e1, e1], axis=2).reshape(T, 4 * 2 * T)  # (tk, (j, jj, tq))
        expb[eo] = np.concatenate([e2, e2], axis=0).astype(np.float32)
        m = np.zeros((128, 128), np.float32)
        wv_ = np.zeros((128, 1), np.float32)
        bv_ = np.zeros((128, 1), np.float32)
        for j, he in enumerate(heads):
            m[32 * j: 32 * j + 16, :] = w_out[:, he * 16: (he + 1) * 16].T
            wv_[32 * j: 32 * j + 16, 0] = n2w_full[he * 16: (he + 1) * 16]
            bv_[32 * j: 32 * j + 16, 0] = n2b_full[he * 16: (he + 1) * 16]
        wo[eo], n2w[eo], n2b[eo] = m, wv_, bv_

    J = np.kron(np.eye(HE, dtype=np.float32), np.ones((HD, HD), np.float32))
    E4 = np.zeros((4, 128), np.float32)
    for s in range(4):
        E4[s, 32 * s: 32 * s + 16] = 1.0
    sel4 = np.zeros((128, 4), np.float32)
    for s in range(4):
        sel4[32 * s + 16, s] = 1.0
    sel32 = np.zeros((8, 128, 32), np.float32)
    for j in range(8):
        for s in range(4):
            sel32[j, 32 * s + 16, 4 * j + s] = 1.0
    E4b = np.zeros((8, 32, 128), np.float32)
    for j in range(8):
        for s in range(4):
            E4b[j, 4 * j + s, 32 * s: 32 * s + 16] = 1.0

    # Centered conv weights: folding the head-mean subtraction of the q/k
    # LayerNorm into the 1x1 conv (q_hat = (I - J/HD) q is exactly q - mean).
    Cc = np.eye(128, dtype=np.float32) - J / HD
    wq2 = Cc @ wq
    wk2 = Cc @ wk
    bq2 = Cc @ b3[:, 0].reshape(-1)
    bk2 = Cc @ b3[:, 1].reshape(-1)

    qn_w = np.asarray(inputs["qn_w"], np.float32)
    kn_w = np.asarray(inputs["kn_w"], np.float32)
    qn_b = np.asarray(inputs["qn_b"], np.float32)
    kn_b = np.asarray(inputs["kn_b"], np.float32)
    # Packed-variance selectors: chunk j (j%4 == m) of a 16-chunk batch puts
    # head h's variance at packed row 32*(j//4) + 8*m + h.
    S16P = np.zeros((4, 128, 32), np.float32)
    S8qP = np.zeros((4, 128, 128), np.float32)
    S8kEP = np.zeros((4, 128, 128), np.float32)
    S8kOP = np.zeros((4, 128, 128), np.float32)
    knbE = np.zeros(128, np.float32)
    knbO = np.zeros(128, np.float32)
    for p in range(128):
        h = p // 16
        if h % 2 == 0:
            knbE[p] = kn_b[p % 16]
        else:
            knbO[p] = kn_b[p % 16]
        for m in range(4):
            S16P[m, p, 8 * m + h] = 1.0 / HD
            for b in range(4):
                r = 32 * b + 8 * m + h
                S8qP[m, r, p] = qn_w[p % 16]
                if h % 2 == 0:
                    S8kEP[m, r, p] = kn_w[p % 16]
                else:
                    S8kOP[m, r, p] = kn_w[p % 16]

    col = lambda a: np.asarray(a, np.float32).reshape(-1, 1)
    return {
        "ident": np.eye(128, dtype=np.float32),
        "identb": np.eye(128, dtype=np.float32),
        "wqT": wq2.T.copy(), "wkT": wk2.T.copy(), "wvT": wv.T.copy(),
        "bq": col(bq2), "bk": col(bk2),
        "bvrow": np.broadcast_to(b3[:, 2].reshape(-1), (128, 128)).copy(),
        "expbE": expb[0], "expbO": expb[1],
        "E4": E4, "woE": wo[0], "woO": wo[1],
        "sel4": sel4,
        **{f"S16P{m}": S16P[m] for m in range(4)},
        **{f"S8qP{m}": S8qP[m] for m in range(4)},
        **{f"S8kEP{m}": S8kEP[m] for m in range(4)},
        **{f"S8kOP{m}": S8kOP[m] for m in range(4)},
        **{f"E4b{j}": E4b[j] for j in range(8)},
        **{f"sel32_{j}": sel32[j] for j in range(8)},
        "b_o": col(inputs["b_out"]), "gamma": col(inputs["gamma"]),
        "n1w": col(inputs["norm1_w"]), "n1b": col(inputs["norm1_b"]),
        "epscol": np.full((128, 1), EPS, np.float32),
        "n2wE": n2w[0], "n2bE": n2b[0], "n2wO": n2w[1], "n2bO": n2b[1],
        "qnb": col(np.tile(qn_b, HE)),
        "knbE": col(knbE), "knbO": col(knbO),
    }


_BF16_IN = {"wqT", "wkT", "wvT", "bvrow", "expbE", "expbO", "E4",
            "woE", "woO", "identb", "sel4",
            *[f"S16P{m}" for m in range(4)], *[f"S8qP{m}" for m in range(4)],
            *[f"S8kEP{m}" for m in range(4)], *[f"S8kOP{m}" for m in range(4)],
            *[f"E4b{j}" for j in range(8)],
            *[f"sel32_{j}" for j in range(8)]}
_CONST_SHAPES = {
    "ident": (128, 128), "identb": (128, 128), "wqT": (128, 128), "wkT": (128, 128), "wvT": (128, 128),
    "bq": (128, 1), "bk": (128, 1), "bvrow": (128, 128),
    "expbE": (128, 512), "expbO": (128, 512), "E4": (4, 128),
    "sel4": (128, 4),
    **{f"S16P{m}": (128, 32) for m in range(4)},
    **{f"S8qP{m}": (128, 128) for m in range(4)},
    **{f"S8kEP{m}": (128, 128) for m in range(4)},
    **{f"S8kOP{m}": (128, 128) for m in range(4)},
    **{f"E4b{j}": (32, 128) for j in range(8)},
    **{f"sel32_{j}": (128, 32) for j in range(8)},
    "woE": (128, 128), "woO": (128, 128), "b_o": (128, 1), "gamma": (128, 1),
    "n1w": (128, 1), "n1b": (128, 1), "epscol": (128, 1),
    "n2wE": (128, 1), "n2bE": (128, 1), "n2wO": (128, 1), "n2bO": (128, 1),
    "qnb": (128, 1), "knbE": (128, 1), "knbO": (128, 1),
}


def build_nc(ln_general=True):
    nc = bacc.Bacc("TRN2", target_bir_lowering=False, debug=False,
                   enable_asserts=False, num_devices=NCORES)
    ins = {"x": nc.dram_tensor("x", [NP_, C], F32, kind="ExternalInput").ap()}
    for name, shp in _CONST_SHAPES.items():
        dt = BF16 if name in _BF16_IN else F32
        ins[name] = nc.dram_tensor(name, list(shp), dt, kind="ExternalInput").ap()
    y = nc.dram_tensor("y", [NP_, C], F32, kind="ExternalOutput").ap()
    with tile.TileContext(nc) as tc:
        _body(tc, nc, ins, y, ln_general)
    nc.compile()
    return nc


def _body(tc, nc, ins, y, ln_general):
    x = ins["x"]
    pools = {}

    def pool(name, bufs=1, space="SBUF"):
        if name not in pools:
            pools[name] = tc.alloc_tile_pool(name=name, bufs=bufs, space=space)
        return pools[name]

    cp = pool("consts")
    big = pool("big")
    ps = pool("psA", bufs=4, space="PSUM")
    ps_s = pool("psS", bufs=2, space="PSUM")
    dram = pool("dram", bufs=1, space="DRAM")
    sm = pool("small", bufs=1)
    scr = pool("scr", bufs=2)
    esp = pool("esp", bufs=2)

    cst = {}
    for name in _CONST_SHAPES:
        ap = ins[name]
        t = cp.tile(list(ap.shape), ap.dtype, tag=name)
        nc.sync.dma_start(t[:], ap)
        cst[name] = t

    # Big slots: S1 {xT, v_t} / S2 {kk} / S3 {xn, attH} / S4 {q} / S5 {sq, kEO}
    xT = big.tile([128, VST * NPAIR + 32], BF16, tag="S1", name="xT")[:, :NP_]
    q = big.tile([128, NP_], BF16, tag="S4")
    kk = big.tile([128, NP_], BF16, tag="S2")

    # ---------------- Phase 0: load + transpose + norm1 stats ------------
    # Stats are computed per quarter as the transposed data lands, so the
    # reduce work overlaps the DMA/PE of later quarters and the AllReduce
    # can fire right after the last chunk.
    s1 = sm.tile([128, NSAMP], F32, tag="s1")
    s2 = sm.tile([128, NSAMP], F32, tag="s2")
    sq = big.tile([128, NP_], BF16, tag="S5")
    for k8 in range(32):
        ld = scr.tile([128, 512], F32, tag="ld", name="ld")
        src = x.rearrange("(k8 k p) c -> k8 p k c", p=128, k=4)[k8]
        nc.sync.dma_start(ld[:].rearrange("p (k c) -> p k c", k=4), src)
        for j in range(4):
            k = 4 * k8 + j
            pt = ps.tile([128, 128], F32, tag="a", name="tp")
            nc.tensor.transpose(pt[:], ld[:, 128 * j: 128 * (j + 1)], cst["ident"][:])
            if j % 2 == 0:
                nc.vector.tensor_copy(xT[:, 128 * k: 128 * (k + 1)], pt[:])
            else:
                nc.scalar.copy(xT[:, 128 * k: 128 * (k + 1)], pt[:])
        if k8 % 8 == 7:
            qq = k8 // 8
            qsl = slice(4096 * qq, 4096 * (qq + 1))
            ssl = slice(32 * qq, 32 * (qq + 1))
            nc.vector.reduce_sum(
                s1[:, ssl], xT[:, qsl].rearrange("p (k s) -> p k s", s=128),
                axis=AX.X)
            nc.scalar.square(sq[:, qsl], xT[:, qsl])
            nc.vector.reduce_sum(
                s2[:, ssl], sq[:, qsl].rearrange("p (k s) -> p k s", s=128),
                axis=AX.X)

    stats = sm.tile([128, 2 * NSAMP], F32, tag="st2")
    nc.vector.tensor_copy(stats[:, :NSAMP], s1[:])
    nc.vector.tensor_copy(stats[:, NSAMP:], s2[:])
    cc_in = dram.tile([128, 2 * NSAMP], F32, tag="cc_in")
    cc_out = dram.tile([128, 2 * NSAMP], F32, tag="cc_out")
    nc.gpsimd.dma_start(cc_in[:], stats[:])
    nc.gpsimd.collective_compute("AllReduce", ALU.add,
                                 replica_groups=[list(range(NCORES))],
                                 ins=[cc_in[:].opt()], outs=[cc_out[:].opt()])
    nc.sync.dma_start(stats[:], cc_out[:])

    mean = sm.tile([128, NSAMP], F32, tag="m2")
    scl1 = sm.tile([128, NSAMP], F32, tag="sc2")
    sft1 = sm.tile([128, NSAMP], F32, tag="sf2")
    tmp = sm.tile([128, NSAMP], F32, tag="t2")

    def norm_coeffs(st, scl, sft, mn, tp, w_ap, b_ap):
        nc.vector.tensor_scalar_mul(mn[:], st[:, :NSAMP], 1.0 / SPATIAL)
        nc.vector.tensor_scalar_mul(tp[:], st[:, NSAMP:], 1.0 / SPATIAL)
        nc.vector.tensor_tensor(out=scl[:], in0=mn[:], in1=mn[:], op=ALU.mult)
        nc.vector.tensor_tensor(out=tp[:], in0=tp[:], in1=scl[:], op=ALU.subtract)
        nc.scalar.activation(tp[:], tp[:], AF.Sqrt,
                             bias=cst["epscol"][:, 0:1], scale=1.0)
        nc.vector.reciprocal_approx_fast(out=scl[:], in_=tp[:])
        nc.vector.tensor_scalar_mul(scl[:], scl[:], w_ap[:, 0:1])
        nc.vector.tensor_tensor(out=sft[:], in0=mn[:], in1=scl[:], op=ALU.mult)
        nc.vector.tensor_scalar(out=sft[:], in0=sft[:], scalar1=-1.0,
                                scalar2=b_ap[:, 0:1], op0=ALU.mult, op1=ALU.add)

    norm_coeffs(stats, scl1, sft1, mean, tmp, cst["n1w"], cst["n1b"])

    # ---------------- Phase 2: norm1 apply -------------------------------
    xn = big.tile([128, NP_], BF16, tag="S3")
    for k in range(NSAMP):
        sl = slice(128 * k, 128 * (k + 1))
        nc.vector.tensor_scalar(out=xn[:, sl], in0=xT[:, sl],
                                scalar1=scl1[:, k: k + 1], scalar2=sft1[:, k: k + 1],
                                op0=ALU.mult, op1=ALU.add)

    # ---------------- Phase 3+4: conv1 + head-LN (centered weights) ------
    # wqT/wkT are pre-centered host-side: the conv directly yields
    # q_hat = q - mean_head(q) (written straight into q / kE).  Per-head
    # variances for 16 chunks are PACKED into one [128,512] PSUM tile via
    # 32-aligned accumulating selector matmuls (chunk j's 8 head-rows land
    # at partitions 8j), so ONE Sqrt + ONE reciprocal_approx_fast serves a
    # whole batch -- no activation-table thrash, no per-chunk 8-lane ops.
    # A second sweep broadcasts rstd back per chunk (band-sliced matmuls)
    # and applies scale+bias; the k apply writes both parity-masked copies
    # (kE: even heads, kO into kk's slot: odd).
    kE = big.tile([128, NP_], BF16, tag="S5")
    for bi in range(2):
        pvq = ps_s.tile([128, 512], F32, tag="s", name="pvq", bufs=2)
        pvk = ps_s.tile([128, 512], F32, tag="s", name="pvk", bufs=2)
        for jj in range(16):
            n = 16 * bi + jj
            sl = slice(512 * n, 512 * (n + 1))
            b, m = jj // 4, jj % 4
            pq = ps.tile([128, 512], F32, tag="a", name="pq")
            nc.tensor.matmul(pq[:], cst["wqT"][:], xn[:, sl], start=True, stop=True)
            nc.scalar.add(q[:, sl], pq[:], cst["bq"][:, 0:1])
            sqq = scr.tile([128, 512], BF16, tag="sqq")
            nc.vector.tensor_tensor(out=sqq[:], in0=q[:, sl], in1=q[:, sl],
                                    op=ALU.mult)
            nc.tensor.matmul(pvq[32 * b: 32 * b + 32, :], cst[f"S16P{m}"][:],
                             sqq[:], start=(m == 0), stop=(m == 3),
                             tile_position=(0, 32 * b))
            pk = ps.tile([128, 512], F32, tag="a", name="pk")
            nc.tensor.matmul(pk[:], cst["wkT"][:], xn[:, sl], start=True, stop=True)
            nc.scalar.add(kE[:, sl], pk[:], cst["bk"][:, 0:1])
            sqk = scr.tile([128, 512], BF16, tag="sqq", name="sqk")
            nc.vector.tensor_tensor(out=sqk[:], in0=kE[:, sl], in1=kE[:, sl],
                                    op=ALU.mult)
            nc.tensor.matmul(pvk[32 * b: 32 * b + 32, :], cst[f"S16P{m}"][:],
                             sqk[:], start=(m == 0), stop=(m == 3),
                             tile_position=(0, 32 * b))
        # batch rstd: std = sqrt(var + eps) on ACT, 1/std via the fast
        # Newton-Raphson custom DVE op (fp32), then cast for the matmuls.
        rq = scr.tile([128, 512], F32, tag="ld", name="rq")
        nc.scalar.activation(rq[:], pvq[:], AF.Sqrt,
                             bias=cst["epscol"][:, 0:1], scale=1.0)
        nc.vector.reciprocal_approx_fast(out=rq[:], in_=rq[:])
        rqb = scr.tile([128, 512], BF16, tag="rqb")
        nc.vector.tensor_copy(rqb[:], rq[:])
        rk = scr.tile([128, 512], F32, tag="ld", name="rk")
        nc.scalar.activation(rk[:], pvk[:], AF.Sqrt,
                             bias=cst["epscol"][:, 0:1], scale=1.0)
        nc.vector.reciprocal_approx_fast(out=rk[:], in_=rk[:])
        rkb = scr.tile([128, 512], BF16, tag="rqb", name="rkb")
        nc.vector.tensor_copy(rkb[:], rk[:])
        for jj in range(16):
            n = 16 * bi + jj
            sl = slice(512 * n, 512 * (n + 1))
            b, m = jj // 4, jj % 4
            bsl = slice(32 * b, 32 * b + 32)
            pRq = ps.tile([128, 512], F32, tag="a", name="pRq")
            nc.tensor.matmul(pRq[:], cst[f"S8qP{m}"][bsl, :], rqb[bsl, :],
                             start=True, stop=True, tile_position=(32 * b, 0))
            Rq = scr.tile([128, 512], BF16, tag="Rq")
            nc.scalar.copy(Rq[:], pRq[:])
            nc.vector.tensor_tensor(out=q[:, sl], in0=q[:, sl], in1=Rq[:],
                                    op=ALU.mult)
            nc.vector.tensor_scalar_add(q[:, sl], q[:, sl], cst["qnb"][:, 0:1])
            # kO first (reads pre-scale kE), then kE in place
            for sname, bname, dst in (("S8kOP", "knbO", kk), ("S8kEP", "knbE", kE)):
                pRk = ps.tile([128, 512], F32, tag="a", name="pRk")
                nc.tensor.matmul(pRk[:], cst[f"{sname}{m}"][bsl, :], rkb[bsl, :],
                                 start=True, stop=True, tile_position=(32 * b, 0))
                Rk = scr.tile([128, 512], BF16, tag="Rk")
                nc.scalar.copy(Rk[:], pRk[:])
                nc.vector.tensor_tensor(out=dst[:, sl], in0=kE[:, sl], in1=Rk[:],
                                        op=ALU.mult)
                nc.vector.tensor_scalar_add(dst[:, sl], dst[:, sl],
                                            cst[bname][:, 0:1])

    # ---------------- Phase 5: conv v (transposed layout) ----------------
    v_t = big.tile([128, VST * NPAIR + 32], BF16, tag="S1")
    nc.vector.memset(v_t[:], 0.0)
    ones_ap = v_t[:, :VST * NPAIR].rearrange(
        "p (m e d) -> p m e d", e=8, d=17)[:, :, :, 16:17]
    nc.vector.memset(ones_ap, 1.0)
    xnv = xn[:].rearrange("c (t m z) -> c t m z", m=NPAIR, z=2)
    for m in range(NPAIR):
        pv = ps.tile([128, 128], F32, tag="a", name="pvt")
        for z in range(2):
            nc.tensor.matmul(pv[64 * z: 64 * z + 64, :], xnv[:, :, m, z],
                             cst["wvT"][:], start=True, stop=True,
                             tile_position=(0, 64 * z))
        dst = v_t[:, VST * m: VST * m + 136].rearrange(
            "p (e d) -> p e d", d=17)[:, :, 0:16]
        nc.vector.tensor_tensor(out=dst,
                                in0=pv[:].rearrange("p (e d) -> p e d", d=16),
                                in1=cst["bvrow"][:].rearrange("p (e d) -> p e d", d=16),
                                op=ALU.add)

    # ---------------- Phases 6-8: attention (two head parities) ----------
    attH = big.tile([128, NP_], BF16, tag="S3")
    attE_d = dram.tile([128, NP_], BF16, tag="attE_d")
    qv = q[:].rearrange("a (t p) -> a t p", p=256)
    kvE = kE[:].rearrange("a (t p) -> a t p", p=256)
    kvO = kk[:].rearrange("a (t p) -> a t p", p=256)

    def att_pass(eo, expb_c, woname, wname, bname):
        kv = kvE if eo == 0 else kvO
        s1b = sm.tile([128, NSAMP], F32, tag="s1")
        s2b = sm.tile([128, NSAMP], F32, tag="s2")

        def rowsum_stats_block(nb):
            # rowsum division for chunks 8nb..8nb+8 + their norm2 stats,
            # emitted right after the producing groups so the DVE work
            # overlaps later groups' matmuls.
            rs_blk = esp.tile([32, 512], F32, tag="rs_blk", bufs=1)
            rsb_blk = esp.tile([32, 512], BF16, tag="rsb_blk", bufs=1)
            prs = ps.tile([32, 512], F32, tag="a", name="prs")
            for j in range(8):
                n = 8 * nb + j
                sl = slice(512 * n, 512 * (n + 1))
                nc.tensor.matmul(prs[:], cst[f"sel32_{j}"][:], attH[:, sl],
                                 start=(j == 0), stop=(j == 7))
            nc.vector.reciprocal(rs_blk[:], prs[:])
            nc.vector.tensor_copy(rsb_blk[:], rs_blk[:])
            for j in range(8):
                n = 8 * nb + j
                sl = slice(512 * n, 512 * (n + 1))
                pb = ps_s.tile([128, 512], F32, tag="s", name="pb")
                nc.tensor.matmul(pb[:], cst[f"E4b{j}"][:], rsb_blk[:],
                                 start=True, stop=True)
                rbc = esp.tile([128, 512], BF16, tag="rbc", bufs=1)
                nc.scalar.copy(rbc[:], pb[:])
                nc.vector.tensor_tensor(out=attH[:, sl], in0=attH[:, sl],
                                        in1=rbc[:], op=ALU.mult)
                sqc = scr.tile([128, 512], BF16, tag="Rq", name="sqc")
                nc.scalar.square(sqc[:], attH[:, sl])
                nc.vector.reduce_sum(
                    s1b[:, 4 * n: 4 * n + 4],
                    attH[:, sl].rearrange("c (s p) -> c s p", s=4), axis=AX.X)
                nc.vector.reduce_sum(
                    s2b[:, 4 * n: 4 * n + 4],
                    sqc[:].rearrange("c (s p) -> c s p", s=4), axis=AX.X)

        for grp in range(32):
            pa0 = ps.tile([128, 256], F32, tag="pa0", name="pa0", bufs=1)
            pa1 = ps.tile([128, 256], F32, tag="pa1", name="pa1", bufs=1)
            for sg in range(2):
                pairs = (8 * grp + 4 * sg, 8 * grp + 4 * sg + 2)
                es = esp.tile([128, 512], BF16, tag="es")
                # One PSUM bank PER ROW GROUP (concurrent row-tiled matmuls
                # must not write the same bank+partitions), with r-innermost
                # issue order so LDWEIGHTS of the next matmul (different row
                # group) overlaps the in-flight one.
                pqk = [ps.tile([128, 128], F32, tag="a", name=f"pqk{r}",
                               bufs=4) for r in range(4)]
                for jj, pbase in enumerate(pairs):
                    for h01 in range(2):
                        p = pbase + h01
                        for tkc in range(2):
                            for r in range(4):
                                prt = slice(32 * r, 32 * r + 32)
                                nc.tensor.matmul(
                                    pqk[r][64 * h01 + 32 * tkc:
                                           64 * h01 + 32 * tkc + 32,
                                           64 * jj: 64 * jj + 64],
                                    kv[prt, 32 * tkc: 32 * tkc + 32, p],
                                    qv[prt, :, p], start=True, stop=True,
                                    tile_position=(32 * r,
                                                   64 * h01 + 32 * tkc))
                for r in range(4):
                    nc.scalar.activation(es[:, 128 * r: 128 * r + 128],
                                         pqk[r][:], AF.Exp, bias=0.0, scale=0.25)
                nc.vector.tensor_tensor(out=es[:], in0=es[:], in1=expb_c[:],
                                        op=ALU.mult)
                for jj, pbase in enumerate(pairs):
                    pair = pbase // 2
                    lp = 2 * sg + jj
                    for j4 in range(4):
                        eg = 2 * j4 + eo
                        for h01, pah in ((0, pa0), (1, pa1)):
                            lhs_v = v_t[64 * h01: 64 * h01 + 64,
                                        VST * pair + 17 * eg: VST * pair + 17 * eg + 32]
                            nc.tensor.matmul(
                                pah[32 * j4: 32 * j4 + 32, 64 * lp: 64 * lp + 64],
                                lhs_v,
                                es[64 * h01: 64 * h01 + 64,
                                   128 * j4 + 64 * jj: 128 * j4 + 64 * jj + 64],
                                start=True, stop=True,
                                tile_position=(64 * h01, 32 * j4))
            # Sample-major attH: col = b*8192 + t*128 + pix, so the norm2
            # stats/apply see whole samples as dense 128-col blocks.
            b_ = grp // 16
            pp0 = (8 * grp) % 128 // 2
            att5 = attH[:].rearrange("c (b t pp z) -> c b t pp z", b=2, t=T, z=2)
            for h01, pah in ((0, pa0), (1, pa1)):
                dstv = att5[:, b_, :, pp0: pp0 + 4, h01].rearrange(
                    "c t l -> c l t")
                srcv = pah[:].rearrange("c (lp z) -> c lp z", z=64)
                if h01 == 0:
                    nc.vector.tensor_copy(dstv, srcv)
                else:
                    nc.scalar.copy(dstv, srcv)
            # chunk n (samples 4n..4n+4) is complete once all 16 groups of
            # its b-half have written their pixel stripes
            if grp == 15:
                rowsum_stats_block(0)
                rowsum_stats_block(1)
            elif grp == 31:
                rowsum_stats_block(2)
                rowsum_stats_block(3)

        st2 = sm.tile([128, 2 * NSAMP], F32, tag=f"st2_{eo}")
        nc.vector.tensor_copy(st2[:, :NSAMP], s1b[:])
        nc.vector.tensor_copy(st2[:, NSAMP:], s2b[:])
        cc2i = dram.tile([128, 2 * NSAMP], F32, tag="cc_in")
        cc2o = dram.tile([128, 2 * NSAMP], F32, tag="cc_out")
        nc.gpsimd.dma_start(cc2i[:], st2[:])
        nc.gpsimd.collective_compute("AllReduce", ALU.add,
                                     replica_groups=[list(range(NCORES))],
                                     ins=[cc2i[:].opt()], outs=[cc2o[:].opt()])
        nc.sync.dma_start(st2[:], cc2o[:])
        if eo == 0:
            nc.sync.dma_start(attE_d[:], attH[:])
        return st2

    st2E = att_pass(0, cst["expbE"], "woE", "n2wE", "n2bE")
    st2O = att_pass(1, cst["expbO"], "woO", "n2wO", "n2bO")

    # norm2 coefficients for both parities (deferred so neither parity's
    # post-AllReduce work head-of-line blocks the other pass's queues)
    sc2E = sm.tile([128, NSAMP], F32, tag="sc2E")
    sf2E = sm.tile([128, NSAMP], F32, tag="sf2E")
    sc2O = sm.tile([128, NSAMP], F32, tag="sc2O")
    sf2O = sm.tile([128, NSAMP], F32, tag="sf2O")
    m2 = sm.tile([128, NSAMP], F32, tag="m2")
    t2 = sm.tile([128, NSAMP], F32, tag="t2")
    norm_coeffs(st2E, sc2E, sf2E, m2, t2, cst["n2wE"], cst["n2bE"])
    norm_coeffs(st2O, sc2O, sf2O, m2, t2, cst["n2wO"], cst["n2bO"])

    # ---------------- Phase 9: conv2 + gamma + residual + store ----------
    # Sample-major chunks: chunk n = samples 4n..4n+4, each a dense 128-col
    # block.  norm2 is applied LAZILY here (per-sample tensor_scalar with
    # per-partition coeffs) right before the conv matmul; x / y DMAs are
    # contiguous 128x128 blocks per sample.
    xb = x.rearrange("(t b p) c -> b t p c", b=2, p=128)
    yb4 = y.rearrange("(t b p) c -> b t p c", b=2, p=128)
    for n in range(32):
        sl = slice(512 * n, 512 * (n + 1))
        aA = scr.tile([128, 512], BF16, tag="qh", name="aA")
        nc.sync.dma_start(aA[:], attE_d[:, sl])
        aAn = scr.tile([128, 512], BF16, tag="Rq", name="aAn")
        aOn = scr.tile([128, 512], BF16, tag="Rk", name="aOn")
        for j in range(4):
            smp = 4 * n + j
            csl = slice(128 * j, 128 * (j + 1))
            nc.any.tensor_scalar(out=aAn[:, csl], in0=aA[:, csl],
                                 scalar1=sc2E[:, smp: smp + 1],
                                 scalar2=sf2E[:, smp: smp + 1],
                                 op0=ALU.mult, op1=ALU.add)
            nc.any.tensor_scalar(out=aOn[:, csl], in0=attH[:, sl][:, csl],
                                 scalar1=sc2O[:, smp: smp + 1],
                                 scalar2=sf2O[:, smp: smp + 1],
                                 op0=ALU.mult, op1=ALU.add)
        po = ps.tile([128, 512], F32, tag="a", name="po")
        nc.tensor.matmul(po[:], cst["woE"][:], aAn[:], start=True, stop=False)
        nc.tensor.matmul(po[:], cst["woO"][:], aOn[:], start=False, stop=True)
        yb = scr.tile([128, 512], BF16, tag="sqq", name="yb")
        nc.any.tensor_scalar(out=yb[:], in0=po[:], scalar1=cst["b_o"][:, 0:1],
                             scalar2=cst["gamma"][:, 0:1], op0=ALU.add, op1=ALU.mult)
        xr = scr.tile([128, 512], F32, tag="ld", name="xr")
        b_, t0 = (4 * n) // 64, (4 * n) % 64
        nc.sync.dma_start(
            xr[:].rearrange("p (j c) -> p j c", j=4),
            xb[b_, t0: t0 + 4].rearrange("j p c -> p j c"))
        y8 = scr.tile([128, 512], F32, tag="y8f", name="y8")
        for j in range(4):
            pt = ps.tile([128, 128], BF16, tag="a", name="tp2")
            nc.tensor.transpose(pt[:], yb[:, 128 * j: 128 * (j + 1)], cst["identb"][:])
            nc.any.tensor_tensor(out=y8[:, 128 * j: 128 * (j + 1)], in0=pt[:],
                                 in1=xr[:, 128 * j: 128 * (j + 1)], op=ALU.add)
        nc.sync.dma_start(
            yb4[b_, t0: t0 + 4].rearrange("j p c -> p j c"),
            y8[:].rearrange("p (j c) -> p j c", j=4))

    for p_ in reversed(list(pools.values())):
        p_.release()


# ---- public entry point -------------------------------------------------
_NC = None


def _get_nc():
    global _NC
    if _NC is None:
        _NC = build_nc(ln_general=True)
    return _NC


def kernel(**inputs) -> np.ndarray:
    import ml_dtypes
    from concourse import bass_utils

    nc = _get_nc()
    consts = host_prep(inputs)
    cmap = {}
    for name, val in consts.items():
        v = np.asarray(val, np.float32)
        if name in _BF16_IN:
            v = v.astype(ml_dtypes.bfloat16)
        cmap[name] = v
    x = np.asarray(inputs["x"], np.float32)
    in_maps = []
    for c in range(NCORES):
        m = dict(cmap)
        m["x"] = np.ascontiguousarray(
            x[:, :, HL * c: HL * (c + 1), :, :].reshape(-1, C))
        in_maps.append(m)
    res = bass_utils.run_bass_kernel_spmd(
        nc, in_maps, core_ids=list(range(NCORES)), trace=False)
    y = np.zeros((T, B, H, W, C), np.float32)
    for c in range(NCORES):
        y[:, :, HL * c: HL * (c + 1), :, :] = \
            res.results[c]["y"].reshape(T, B, HL, W, C)
    return y


def kernel_traced(**inputs):
    """Like kernel() but returns (y, per_core_exec_ns, trace_path)."""
    import ml_dtypes
    from concourse import bass_utils

    nc = _get_nc()
    consts = host_prep(inputs)
    cmap = {}
    for name, val in consts.items():
        v = np.asarray(val, np.float32)
        if name in _BF16_IN:
            v = v.astype(ml_dtypes.bfloat16)
        cmap[name] = v
    x = np.asarray(inputs["x"], np.float32)
    in_maps = []
    for c in range(NCORES):
        m = dict(cmap)
        m["x"] = np.ascontiguousarray(
            x[:, :, HL * c: HL * (c + 1), :, :].reshape(-1, C))
        in_maps.append(m)
    res = bass_utils.run_bass_kernel_spmd(
        nc, in_maps, core_ids=list(range(NCORES)),
        trace=True, trace_cores=list(range(NCORES)))
    y = np.zeros((T, B, H, W, C), np.float32)
    for c in range(NCORES):
        y[:, :, HL * c: HL * (c + 1), :, :] = \
            res.results[c]["y"].reshape(T, B, HL, W, C)
    trace_path = (res.instructions_and_trace[1]
                  if res.instructions_and_trace else None)
    return y, res.exec_time_ns, trace_path



# revision 44
# speedup vs baseline: 1.6303x; 1.0057x over previous
"""8-NeuronCore Trainium2 Bass kernel for nn_AttentionBlock_17789754540111.

Self-contained: builds a hand-written Bass/Tile SPMD program (H-sharded over
8 cores, params replicated, instance-norm stats combined via on-device
AllReduce) and runs it on the 8 axon-tunneled TRN2 cores via
concourse.bass_utils.run_bass_kernel_spmd.
"""

import sys
import types
import ctypes
import contextlib

# ---- NTFF profile hook expected by bass_utils under axon ----------------
def _install_axon_hook():
    if "antenv.axon_hooks" in sys.modules:
        return
    hookmod = types.ModuleType("antenv.axon_hooks")

    def _make_hook():
        try:
            lib = ctypes.CDLL("/opt/axon/libaxon_pjrt.so")
        except OSError:
            return None
        if not hasattr(lib, "axon_start_nrt_profile"):
            return None
        lib.axon_start_nrt_profile.argtypes = [ctypes.POINTER(ctypes.c_int64),
                                               ctypes.c_size_t]
        lib.axon_start_nrt_profile.restype = ctypes.c_int64
        lib.axon_stop_nrt_profile.argtypes = [ctypes.c_char_p]
        lib.axon_stop_nrt_profile.restype = ctypes.c_int64

        @contextlib.contextmanager
        def _hook(output_dir, device_ids):
            import jax
            jax.devices()
            if device_ids:
                ids = (ctypes.c_int64 * len(device_ids))(*device_ids)
                rc = lib.axon_start_nrt_profile(ids, len(device_ids))
            else:
                rc = lib.axon_start_nrt_profile(None, 0)
            if rc != 0:
                raise RuntimeError(f"axon_start_nrt_profile rc={rc}")
            try:
                yield
            finally:
                lib.axon_stop_nrt_profile(str(output_dir).encode())
        return _hook

    hook = _make_hook()
    hookmod.get_axon_ntff_profile_hook = lambda: hook
    hookmod.set_axon_ntff_profile_hook = lambda h: None
    sys.modules["antenv.axon_hooks"] = hookmod

_install_axon_hook()

import math
import sys

sys.path.insert(0, "/opt/trn_rl_repo")

import numpy as np

import concourse.bass as bass  # noqa
import concourse.bacc as bacc
import concourse.mybir as mybir
from concourse import tile

F32 = mybir.dt.float32
BF16 = mybir.dt.bfloat16
AF = mybir.ActivationFunctionType
ALU = mybir.AluOpType
AX = mybir.AxisListType

T, B, H, W, C = 64, 2, 32, 32, 128
HE, HD = 8, 16
NCORES = 8
HL = H // NCORES
PIX = B * HL * W                  # 256
NP_ = T * PIX                     # 16384
NSAMP = T * B                     # 128
SPATIAL = H * W                   # 1024 (global)
EPS = 1e-5
VST = 136
NPAIR = PIX // 2                  # 128


def _rel_bias_table(rel_emb):
    rp = np.arange(T)[None, :] - np.arange(T)[:, None]
    n = -rp
    ret = (n < 0).astype(np.int64) * 16
    n = np.abs(n)
    mx = 8
    small = n < mx
    vl = mx + (np.log(np.maximum(n, 1) / mx) / math.log(32 / mx) * 8).astype(np.int64)
    vl = np.minimum(vl, 15)
    buckets = ret + np.where(small, n, vl)
    return np.ascontiguousarray(
        rel_emb[buckets].transpose(2, 0, 1)).astype(np.float32)  # (he, tq, tk)


def host_prep(inputs):
    w_in = np.asarray(inputs["w_in"], np.float32)
    b_in = np.asarray(inputs["b_in"], np.float32)
    rows = w_in.reshape(HE, 3, HD, C)
    b3 = b_in.reshape(HE, 3, HD)
    wq = rows[:, 0].reshape(HE * HD, C)
    wk = rows[:, 1].reshape(HE * HD, C)
    wv = rows[:, 2].reshape(HE * HD, C)

    bias = _rel_bias_table(np.asarray(inputs["rel_emb"], np.float32))
    eb = np.exp(bias)                                          # (he, tq, tk)
    expb, wo, n2w, n2b = {}, {}, {}, {}
    n2w_full = np.asarray(inputs["norm2_w"], np.float32)
    n2b_full = np.asarray(inputs["norm2_b"], np.float32)
    w_out = np.asarray(inputs["w_out"], np.float32)
    for eo in (0, 1):
        heads = [2 * j + eo for j in range(4)]
        h = eb[heads]                                          # (4, tq, tk)
        # col = j4*128 + jj*64 + tq ; rows = 2 px * 64 tk
        e1 = h.transpose(2, 0, 1)                              # (tk, j, tq)
        e2 = np.stack([e1, e1], axis=2).reshape(T, 4 * 2 * T)  # (tk, (j, jj, tq))
        expb[eo] = np.concatenate([e2, e2], axis=0).astype(np.float32)
        m = np.zeros((128, 128), np.float32)
        wv_ = np.zeros((128, 1), np.float32)
        bv_ = np.zeros((128, 1), np.float32)
        for j, he in enumerate(heads):
            m[32 * j: 32 * j + 16, :] = w_out[:, he * 16: (he + 1) * 16].T
            wv_[32 * j: 32 * j + 16, 0] = n2w_full[he * 16: (he + 1) * 16]
            bv_[32 * j: 32 * j + 16, 0] = n2b_full[he * 16: (he + 1) * 16]
        wo[eo], n2w[eo], n2b[eo] = m, wv_, bv_

    J = np.kron(np.eye(HE, dtype=np.float32), np.ones((HD, HD), np.float32))
    E4 = np.zeros((4, 128), np.float32)
    for s in range(4):
        E4[s, 32 * s: 32 * s + 16] = 1.0
    sel4 = np.zeros((128, 4), np.float32)
    for s in range(4):
        sel4[32 * s + 16, s] = 1.0
    sel32 = np.zeros((8, 128, 32), np.float32)
    for j in range(8):
        for s in range(4):
            sel32[j, 32 * s + 16, 4 * j + s] = 1.0
    E4b = np.zeros((8, 32, 128), np.float32)
    for j in range(8):
        for s in range(4):
            E4b[j, 4 * j + s, 32 * s: 32 * s + 16] = 1.0

    # Centered conv weights: folding the head-mean subtraction of the q/k
    # LayerNorm into the 1x1 conv (q_hat = (I - J/HD) q is exactly q - mean).
    Cc = np.eye(128, dtype=np.float32) - J / HD
    wq2 = Cc @ wq
    wk2 = Cc @ wk
    bq2 = Cc @ b3[:, 0].reshape(-1)
    bk2 = Cc @ b3[:, 1].reshape(-1)

    qn_w = np.asarray(inputs["qn_w"], np.float32)
    kn_w = np.asarray(inputs["kn_w"], np.float32)
    qn_b = np.asarray(inputs["qn_b"], np.float32)
    kn_b = np.asarray(inputs["kn_b"], np.float32)
    # Packed-variance selectors: chunk j (j%4 == m) of a 16-chunk batch puts
    # head h's variance at packed row 32*(j//4) + 8*m + h.
    S16P = np.zeros((4, 128, 32), np.float32)
    S8qP = np.zeros((4, 128, 128), np.float32)
    S8kEP = np.zeros((4, 128, 128), np.float32)
    S8kOP = np.zeros((4, 128, 128), np.float32)
    knbE = np.zeros(128, np.float32)
    knbO = np.zeros(128, np.float32)
    for p in range(128):
        h = p // 16
        if h % 2 == 0:
            knbE[p] = kn_b[p % 16]
        else:
            knbO[p] = kn_b[p % 16]
        for m in range(4):
            S16P[m, p, 8 * m + h] = 1.0 / HD
            for b in range(4):
                r = 32 * b + 8 * m + h
                S8qP[m, r, p] = qn_w[p % 16]
                if h % 2 == 0:
                    S8kEP[m, r, p] = kn_w[p % 16]
                else:
                    S8kOP[m, r, p] = kn_w[p % 16]

    col = lambda a: np.asarray(a, np.float32).reshape(-1, 1)
    return {
        "ident": np.eye(128, dtype=np.float32),
        "identb": np.eye(128, dtype=np.float32),
        "wqT": wq2.T.copy(), "wkT": wk2.T.copy(), "wvT": wv.T.copy(),
        "bq": col(bq2), "bk": col(bk2),
        "bvrow": np.broadcast_to(b3[:, 2].reshape(-1), (128, 128)).copy(),
        "expbE": expb[0], "expbO": expb[1],
        "E4": E4, "woE": wo[0], "woO": wo[1],
        "sel4": sel4,
        **{f"S16P{m}": S16P[m] for m in range(4)},
        **{f"S8qP{m}": S8qP[m] for m in range(4)},
        **{f"S8kEP{m}": S8kEP[m] for m in range(4)},
        **{f"S8kOP{m}": S8kOP[m] for m in range(4)},
        **{f"E4b{j}": E4b[j] for j in range(8)},
        **{f"sel32_{j}": sel32[j] for j in range(8)},
        "b_o": col(inputs["b_out"]), "gamma": col(inputs["gamma"]),
        "n1w": col(inputs["norm1_w"]), "n1b": col(inputs["norm1_b"]),
        "epscol": np.full((128, 1), EPS, np.float32),
        "n2wE": n2w[0], "n2bE": n2b[0], "n2wO": n2w[1], "n2bO": n2b[1],
        "qnb": col(np.tile(qn_b, HE)),
        "knbE": col(knbE), "knbO": col(knbO),
    }


_BF16_IN = {"wqT", "wkT", "wvT", "bvrow", "expbE", "expbO", "E4",
            "woE", "woO", "identb", "sel4",
            *[f"S16P{m}" for m in range(4)], *[f"S8qP{m}" for m in range(4)],
            *[f"S8kEP{m}" for m in range(4)], *[f"S8kOP{m}" for m in range(4)],
            *[f"E4b{j}" for j in range(8)],
            *[f"sel32_{j}" for j in range(8)]}
_CONST_SHAPES = {
    "ident": (128, 128), "identb": (128, 128), "wqT": (128, 128), "wkT": (128, 128), "wvT": (128, 128),
    "bq": (128, 1), "bk": (128, 1), "bvrow": (128, 128),
    "expbE": (128, 512), "expbO": (128, 512), "E4": (4, 128),
    "sel4": (128, 4),
    **{f"S16P{m}": (128, 32) for m in range(4)},
    **{f"S8qP{m}": (128, 128) for m in range(4)},
    **{f"S8kEP{m}": (128, 128) for m in range(4)},
    **{f"S8kOP{m}": (128, 128) for m in range(4)},
    **{f"E4b{j}": (32, 128) for j in range(8)},
    **{f"sel32_{j}": (128, 32) for j in range(8)},
    "woE": (128, 128), "woO": (128, 128), "b_o": (128, 1), "gamma": (128, 1),
    "n1w": (128, 1), "n1b": (128, 1), "epscol": (128, 1),
    "n2wE": (128, 1), "n2bE": (128, 1), "n2wO": (128, 1), "n2bO": (128, 1),
    "qnb": (128, 1), "knbE": (128, 1), "knbO": (128, 1),
}


def build_nc(ln_general=True):
    nc = bacc.Bacc("TRN2", target_bir_lowering=False, debug=False,
                   enable_asserts=False, num_devices=NCORES)
    ins = {"x": nc.dram_tensor("x", [NP_, C], F32, kind="ExternalInput").ap()}
    for name, shp in _CONST_SHAPES.items():
        dt = BF16 if name in _BF16_IN else F32
        ins[name] = nc.dram_tensor(name, list(shp), dt, kind="ExternalInput").ap()
    y = nc.dram_tensor("y", [NP_, C], F32, kind="ExternalOutput").ap()
    with tile.TileContext(nc) as tc:
        _body(tc, nc, ins, y, ln_general)
    nc.compile()
    return nc


def _body(tc, nc, ins, y, ln_general):
    x = ins["x"]
    pools = {}

    def pool(name, bufs=1, space="SBUF"):
        if name not in pools:
            pools[name] = tc.alloc_tile_pool(name=name, bufs=bufs, space=space)
        return pools[name]

    cp = pool("consts")
    big = pool("big")
    ps = pool("psA", bufs=4, space="PSUM")
    ps_s = pool("psS", bufs=2, space="PSUM")
    dram = pool("dram", bufs=1, space="DRAM")
    sm = pool("small", bufs=1)
    scr = pool("scr", bufs=2)
    esp = pool("esp", bufs=2)

    cst = {}
    for name in _CONST_SHAPES:
        ap = ins[name]
        t = cp.tile(list(ap.shape), ap.dtype, tag=name)
        nc.sync.dma_start(t[:], ap)
        cst[name] = t

    # Big slots: S1 {xT, v_t} / S2 {kk} / S3 {xn, attH} / S4 {q} / S5 {sq, kEO}
    xT = big.tile([128, VST * NPAIR + 32], BF16, tag="S1", name="xT")[:, :NP_]
    q = big.tile([128, NP_], BF16, tag="S4")
    kk = big.tile([128, NP_], BF16, tag="S2")

    # ---------------- Phase 0: load + transpose + norm1 stats ------------
    # Stats are computed per quarter as the transposed data lands, so the
    # reduce work overlaps the DMA/PE of later quarters and the AllReduce
    # can fire right after the last chunk.
    s1 = sm.tile([128, NSAMP], F32, tag="s1")
    s2 = sm.tile([128, NSAMP], F32, tag="s2")
    sq = big.tile([128, NP_], BF16, tag="S5")
    for k8 in range(32):
        ld = scr.tile([128, 512], F32, tag="ld", name="ld")
        src = x.rearrange("(k8 k p) c -> k8 p k c", p=128, k=4)[k8]
        nc.sync.dma_start(ld[:].rearrange("p (k c) -> p k c", k=4), src)
        for j in range(4):
            k = 4 * k8 + j
            pt = ps.tile([128, 128], F32, tag="a", name="tp")
            nc.tensor.transpose(pt[:], ld[:, 128 * j: 128 * (j + 1)], cst["ident"][:])
            if j % 2 == 0:
                nc.vector.tensor_copy(xT[:, 128 * k: 128 * (k + 1)], pt[:])
            else:
                nc.scalar.copy(xT[:, 128 * k: 128 * (k + 1)], pt[:])
        if k8 % 8 == 7:
            qq = k8 // 8
            qsl = slice(4096 * qq, 4096 * (qq + 1))
            ssl = slice(32 * qq, 32 * (qq + 1))
            nc.vector.reduce_sum(
                s1[:, ssl], xT[:, qsl].rearrange("p (k s) -> p k s", s=128),
                axis=AX.X)
            nc.scalar.square(sq[:, qsl], xT[:, qsl])
            nc.vector.reduce_sum(
                s2[:, ssl], sq[:, qsl].rearrange("p (k s) -> p k s", s=128),
                axis=AX.X)

    stats = sm.tile([128, 2 * NSAMP], F32, tag="st2")
    nc.vector.tensor_copy(stats[:, :NSAMP], s1[:])
    nc.vector.tensor_copy(stats[:, NSAMP:], s2[:])
    cc_in = dram.tile([128, 2 * NSAMP], F32, tag="cc_in")
    cc_out = dram.tile([128, 2 * NSAMP], F32, tag="cc_out")
    nc.gpsimd.dma_start(cc_in[:], stats[:])
    nc.gpsimd.collective_compute("AllReduce", ALU.add,
                                 replica_groups=[list(range(NCORES))],
                                 ins=[cc_in[:].opt()], outs=[cc_out[:].opt()])
    nc.sync.dma_start(stats[:], cc_out[:])

    mean = sm.tile([128, NSAMP], F32, tag="m2")
    scl1 = sm.tile([128, NSAMP], F32, tag="sc2")
    sft1 = sm.tile([128, NSAMP], F32, tag="sf2")
    tmp = sm.tile([128, NSAMP], F32, tag="t2")

    def norm_coeffs(st, scl, sft, mn, tp, w_ap, b_ap):
        nc.vector.tensor_scalar_mul(mn[:], st[:, :NSAMP], 1.0 / SPATIAL)
        nc.vector.tensor_scalar_mul(tp[:], st[:, NSAMP:], 1.0 / SPATIAL)
        nc.vector.tensor_tensor(out=scl[:], in0=mn[:], in1=mn[:], op=ALU.mult)
        nc.vector.tensor_tensor(out=tp[:], in0=tp[:], in1=scl[:], op=ALU.subtract)
        nc.scalar.activation(tp[:], tp[:], AF.Sqrt,
                             bias=cst["epscol"][:, 0:1], scale=1.0)
        nc.vector.reciprocal_approx_fast(out=scl[:], in_=tp[:])
        nc.vector.tensor_scalar_mul(scl[:], scl[:], w_ap[:, 0:1])
        nc.vector.tensor_tensor(out=sft[:], in0=mn[:], in1=scl[:], op=ALU.mult)
        nc.vector.tensor_scalar(out=sft[:], in0=sft[:], scalar1=-1.0,
                                scalar2=b_ap[:, 0:1], op0=ALU.mult, op1=ALU.add)

    norm_coeffs(stats, scl1, sft1, mean, tmp, cst["n1w"], cst["n1b"])

    # ---------------- Phase 2: norm1 apply -------------------------------
    xn = big.tile([128, NP_], BF16, tag="S3")
    for k in range(NSAMP):
        sl = slice(128 * k, 128 * (k + 1))
        nc.vector.tensor_scalar(out=xn[:, sl], in0=xT[:, sl],
                                scalar1=scl1[:, k: k + 1], scalar2=sft1[:, k: k + 1],
                                op0=ALU.mult, op1=ALU.add)

    # ---------------- Phase 3+4: conv1 + head-LN (centered weights) ------
    # wqT/wkT are pre-centered host-side: the conv directly yields
    # q_hat = q - mean_head(q) (written straight into q / kE).  Per-head
    # variances for 16 chunks are PACKED into one [128,512] PSUM tile via
    # 32-aligned accumulating selector matmuls (chunk j's 8 head-rows land
    # at partitions 8j), so ONE Sqrt + ONE reciprocal_approx_fast serves a
    # whole batch -- no activation-table thrash, no per-chunk 8-lane ops.
    # A second sweep broadcasts rstd back per chunk (band-sliced matmuls)
    # and applies scale+bias; the k apply writes both parity-masked copies
    # (kE: even heads, kO into kk's slot: odd).
    kE = big.tile([128, NP_], BF16, tag="S5")
    for bi in range(2):
        pvq = ps_s.tile([128, 512], F32, tag="s", name="pvq", bufs=2)
        pvk = ps_s.tile([128, 512], F32, tag="s", name="pvk", bufs=2)
        for jj in range(16):
            n = 16 * bi + jj
            sl = slice(512 * n, 512 * (n + 1))
            b, m = jj // 4, jj % 4
            pq = ps.tile([128, 512], F32, tag="a", name="pq")
            nc.tensor.matmul(pq[:], cst["wqT"][:], xn[:, sl], start=True, stop=True)
            nc.scalar.add(q[:, sl], pq[:], cst["bq"][:, 0:1])
            sqq = scr.tile([128, 512], BF16, tag="sqq", bufs=3)
            nc.vector.tensor_tensor(out=sqq[:], in0=q[:, sl], in1=q[:, sl],
                                    op=ALU.mult)
            nc.tensor.matmul(pvq[32 * b: 32 * b + 32, :], cst[f"S16P{m}"][:],
                             sqq[:], start=(m == 0), stop=(m == 3),
                             tile_position=(0, 32 * b))
            pk = ps.tile([128, 512], F32, tag="a", name="pk")
            nc.tensor.matmul(pk[:], cst["wkT"][:], xn[:, sl], start=True, stop=True)
            nc.scalar.add(kE[:, sl], pk[:], cst["bk"][:, 0:1])
            sqk = scr.tile([128, 512], BF16, tag="sqq", name="sqk", bufs=3)
            nc.vector.tensor_tensor(out=sqk[:], in0=kE[:, sl], in1=kE[:, sl],
                                    op=ALU.mult)
            nc.tensor.matmul(pvk[32 * b: 32 * b + 32, :], cst[f"S16P{m}"][:],
                             sqk[:], start=(m == 0), stop=(m == 3),
                             tile_position=(0, 32 * b))
        # batch rstd: std = sqrt(var + eps) on ACT, 1/std via the fast
        # Newton-Raphson custom DVE op (fp32), then cast for the matmuls.
        rq = scr.tile([128, 512], F32, tag="ld", name="rq")
        nc.scalar.activation(rq[:], pvq[:], AF.Sqrt,
                             bias=cst["epscol"][:, 0:1], scale=1.0)
        nc.vector.reciprocal_approx_fast(out=rq[:], in_=rq[:])
        rqb = scr.tile([128, 512], BF16, tag="rqb")
        nc.vector.tensor_copy(rqb[:], rq[:])
        rk = scr.tile([128, 512], F32, tag="ld", name="rk")
        nc.scalar.activation(rk[:], pvk[:], AF.Sqrt,
                             bias=cst["epscol"][:, 0:1], scale=1.0)
        nc.vector.reciprocal_approx_fast(out=rk[:], in_=rk[:])
        rkb = scr.tile([128, 512], BF16, tag="rqb", name="rkb")
        nc.vector.tensor_copy(rkb[:], rk[:])
        for jj in range(16):
            n = 16 * bi + jj
            sl = slice(512 * n, 512 * (n + 1))
            b, m = jj // 4, jj % 4
            bsl = slice(32 * b, 32 * b + 32)
            pRq = ps.tile([128, 512], F32, tag="a", name="pRq")
            nc.tensor.matmul(pRq[:], cst[f"S8qP{m}"][bsl, :], rqb[bsl, :],
                             start=True, stop=True, tile_position=(32 * b, 0))
            Rq = scr.tile([128, 512], BF16, tag="Rq", bufs=3)
            nc.scalar.copy(Rq[:], pRq[:])
            nc.vector.tensor_tensor(out=q[:, sl], in0=q[:, sl], in1=Rq[:],
                                    op=ALU.mult)
            nc.vector.tensor_scalar_add(q[:, sl], q[:, sl], cst["qnb"][:, 0:1])
            # kO first (reads pre-scale kE), then kE in place
            for sname, bname, dst in (("S8kOP", "knbO", kk), ("S8kEP", "knbE", kE)):
                pRk = ps.tile([128, 512], F32, tag="a", name="pRk")
                nc.tensor.matmul(pRk[:], cst[f"{sname}{m}"][bsl, :], rkb[bsl, :],
                                 start=True, stop=True, tile_position=(32 * b, 0))
                Rk = scr.tile([128, 512], BF16, tag="Rk", bufs=3)
                nc.scalar.copy(Rk[:], pRk[:])
                nc.vector.tensor_tensor(out=dst[:, sl], in0=kE[:, sl], in1=Rk[:],
                                        op=ALU.mult)
                nc.vector.tensor_scalar_add(dst[:, sl], dst[:, sl],
                                            cst[bname][:, 0:1])

    # ---------------- Phase 5: conv v (transposed layout) ----------------
    v_t = big.tile([128, VST * NPAIR + 32], BF16, tag="S1")
    nc.vector.memset(v_t[:], 0.0)
    ones_ap = v_t[:, :VST * NPAIR].rearrange(
        "p (m e d) -> p m e d", e=8, d=17)[:, :, :, 16:17]
    nc.vector.memset(ones_ap, 1.0)
    xnv = xn[:].rearrange("c (t m z) -> c t m z", m=NPAIR, z=2)
    for m in range(NPAIR):
        pv = ps.tile([128, 128], F32, tag="a", name="pvt")
        for z in range(2):
            nc.tensor.matmul(pv[64 * z: 64 * z + 64, :], xnv[:, :, m, z],
                             cst["wvT"][:], start=True, stop=True,
                             tile_position=(0, 64 * z))
        dst = v_t[:, VST * m: VST * m + 136].rearrange(
            "p (e d) -> p e d", d=17)[:, :, 0:16]
        nc.vector.tensor_tensor(out=dst,
                                in0=pv[:].rearrange("p (e d) -> p e d", d=16),
                                in1=cst["bvrow"][:].rearrange("p (e d) -> p e d", d=16),
                                op=ALU.add)

    # ---------------- Phases 6-8: attention (two head parities) ----------
    attH = big.tile([128, NP_], BF16, tag="S3")
    attE_d = dram.tile([128, NP_], BF16, tag="attE_d")
    qv = q[:].rearrange("a (t p) -> a t p", p=256)
    kvE = kE[:].rearrange("a (t p) -> a t p", p=256)
    kvO = kk[:].rearrange("a (t p) -> a t p", p=256)

    def att_pass(eo, expb_c, woname, wname, bname):
        kv = kvE if eo == 0 else kvO
        s1b = sm.tile([128, NSAMP], F32, tag="s1")
        s2b = sm.tile([128, NSAMP], F32, tag="s2")

        def rowsum_stats_block(nb):
            # rowsum division for chunks 8nb..8nb+8 + their norm2 stats,
            # emitted right after the producing groups so the DVE work
            # overlaps later groups' matmuls.
            rs_blk = esp.tile([32, 512], F32, tag="rs_blk", bufs=1)
            rsb_blk = esp.tile([32, 512], BF16, tag="rsb_blk", bufs=1)
            prs = ps.tile([32, 512], F32, tag="a", name="prs")
            for j in range(8):
                n = 8 * nb + j
                sl = slice(512 * n, 512 * (n + 1))
                nc.tensor.matmul(prs[:], cst[f"sel32_{j}"][:], attH[:, sl],
                                 start=(j == 0), stop=(j == 7))
            nc.vector.reciprocal(rs_blk[:], prs[:])
            nc.vector.tensor_copy(rsb_blk[:], rs_blk[:])
            for j in range(8):
                n = 8 * nb + j
                sl = slice(512 * n, 512 * (n + 1))
                pb = ps_s.tile([128, 512], F32, tag="s", name="pb")
                nc.tensor.matmul(pb[:], cst[f"E4b{j}"][:], rsb_blk[:],
                                 start=True, stop=True)
                rbc = esp.tile([128, 512], BF16, tag="rbc", bufs=1)
                nc.scalar.copy(rbc[:], pb[:])
                nc.vector.tensor_tensor(out=attH[:, sl], in0=attH[:, sl],
                                        in1=rbc[:], op=ALU.mult)
                sqc = scr.tile([128, 512], BF16, tag="Rq", name="sqc", bufs=3)
                nc.scalar.square(sqc[:], attH[:, sl])
                nc.vector.reduce_sum(
                    s1b[:, 4 * n: 4 * n + 4],
                    attH[:, sl].rearrange("c (s p) -> c s p", s=4), axis=AX.X)
                nc.vector.reduce_sum(
                    s2b[:, 4 * n: 4 * n + 4],
                    sqc[:].rearrange("c (s p) -> c s p", s=4), axis=AX.X)

        for grp in range(32):
            pa0 = ps.tile([128, 256], F32, tag="pa0", name="pa0", bufs=1)
            pa1 = ps.tile([128, 256], F32, tag="pa1", name="pa1", bufs=1)
            for sg in range(2):
                pairs = (8 * grp + 4 * sg, 8 * grp + 4 * sg + 2)
                es = esp.tile([128, 512], BF16, tag="es")
                # One PSUM bank PER ROW GROUP (concurrent row-tiled matmuls
                # must not write the same bank+partitions), with r-innermost
                # issue order so LDWEIGHTS of the next matmul (different row
                # group) overlaps the in-flight one.
                pqk = [ps.tile([128, 128], F32, tag="a", name=f"pqk{r}",
                               bufs=4) for r in range(4)]
                for jj, pbase in enumerate(pairs):
                    for h01 in range(2):
                        p = pbase + h01
                        for tkc in range(2):
                            for r in range(4):
                                prt = slice(32 * r, 32 * r + 32)
                                nc.tensor.matmul(
                                    pqk[r][64 * h01 + 32 * tkc:
                                           64 * h01 + 32 * tkc + 32,
                                           64 * jj: 64 * jj + 64],
                                    kv[prt, 32 * tkc: 32 * tkc + 32, p],
                                    qv[prt, :, p], start=True, stop=True,
                                    tile_position=(32 * r,
                                                   64 * h01 + 32 * tkc))
                for r in range(4):
                    nc.scalar.activation(es[:, 128 * r: 128 * r + 128],
                                         pqk[r][:], AF.Exp, bias=0.0, scale=0.25)
                nc.vector.tensor_tensor(out=es[:], in0=es[:], in1=expb_c[:],
                                        op=ALU.mult)
                for jj, pbase in enumerate(pairs):
                    pair = pbase // 2
                    lp = 2 * sg + jj
                    for j4 in range(4):
                        eg = 2 * j4 + eo
                        for h01, pah in ((0, pa0), (1, pa1)):
                            lhs_v = v_t[64 * h01: 64 * h01 + 64,
                                        VST * pair + 17 * eg: VST * pair + 17 * eg + 32]
                            nc.tensor.matmul(
                                pah[32 * j4: 32 * j4 + 32, 64 * lp: 64 * lp + 64],
                                lhs_v,
                                es[64 * h01: 64 * h01 + 64,
                                   128 * j4 + 64 * jj: 128 * j4 + 64 * jj + 64],
                                start=True, stop=True,
                                tile_position=(64 * h01, 32 * j4))
            # Sample-major attH: col = b*8192 + t*128 + pix, so the norm2
            # stats/apply see whole samples as dense 128-col blocks.
            b_ = grp // 16
            pp0 = (8 * grp) % 128 // 2
            att5 = attH[:].rearrange("c (b t pp z) -> c b t pp z", b=2, t=T, z=2)
            for h01, pah in ((0, pa0), (1, pa1)):
                dstv = att5[:, b_, :, pp0: pp0 + 4, h01].rearrange(
                    "c t l -> c l t")
                srcv = pah[:].rearrange("c (lp z) -> c lp z", z=64)
                if h01 == 0:
                    nc.vector.tensor_copy(dstv, srcv)
                else:
                    nc.scalar.copy(dstv, srcv)
            # chunk n (samples 4n..4n+4) is complete once all 16 groups of
            # its b-half have written their pixel stripes
            if grp == 15:
                rowsum_stats_block(0)
                rowsum_stats_block(1)
            elif grp == 31:
                rowsum_stats_block(2)
                rowsum_stats_block(3)

        st2 = sm.tile([128, 2 * NSAMP], F32, tag=f"st2_{eo}")
        nc.vector.tensor_copy(st2[:, :NSAMP], s1b[:])
        nc.vector.tensor_copy(st2[:, NSAMP:], s2b[:])
        cc2i = dram.tile([128, 2 * NSAMP], F32, tag="cc_in")
        cc2o = dram.tile([128, 2 * NSAMP], F32, tag="cc_out")
        nc.gpsimd.dma_start(cc2i[:], st2[:])
        nc.gpsimd.collective_compute("AllReduce", ALU.add,
                                     replica_groups=[list(range(NCORES))],
                                     ins=[cc2i[:].opt()], outs=[cc2o[:].opt()])
        nc.sync.dma_start(st2[:], cc2o[:])
        if eo == 0:
            nc.sync.dma_start(attE_d[:], attH[:])
        return st2

    st2E = att_pass(0, cst["expbE"], "woE", "n2wE", "n2bE")
    st2O = att_pass(1, cst["expbO"], "woO", "n2wO", "n2bO")

    # norm2 coefficients for both parities (deferred so neither parity's
    # post-AllReduce work head-of-line blocks the other pass's queues)
    sc2E = sm.tile([128, NSAMP], F32, tag="sc2E")
    sf2E = sm.tile([128, NSAMP], F32, tag="sf2E")
    sc2O = sm.tile([128, NSAMP], F32, tag="sc2O")
    sf2O = sm.tile([128, NSAMP], F32, tag="sf2O")
    m2 = sm.tile([128, NSAMP], F32, tag="m2")
    t2 = sm.tile([128, NSAMP], F32, tag="t2")
    norm_coeffs(st2E, sc2E, sf2E, m2, t2, cst["n2wE"], cst["n2bE"])
    norm_coeffs(st2O, sc2O, sf2O, m2, t2, cst["n2wO"], cst["n2bO"])

    # ---------------- Phase 9: conv2 + gamma + residual + store ----------
    # Sample-major chunks: chunk n = samples 4n..4n+4, each a dense 128-col
    # block.  norm2 is applied LAZILY here (per-sample tensor_scalar with
    # per-partition coeffs) right before the conv matmul; x / y DMAs are
    # contiguous 128x128 blocks per sample.
    xb = x.rearrange("(t b p) c -> b t p c", b=2, p=128)
    yb4 = y.rearrange("(t b p) c -> b t p c", b=2, p=128)
    for n in range(32):
        sl = slice(512 * n, 512 * (n + 1))
        aA = scr.tile([128, 512], BF16, tag="qh", name="aA")
        nc.sync.dma_start(aA[:], attE_d[:, sl])
        aAn = scr.tile([128, 512], BF16, tag="Rq", name="aAn", bufs=3)
        aOn = scr.tile([128, 512], BF16, tag="Rk", name="aOn", bufs=3)
        for j in range(4):
            smp = 4 * n + j
            csl = slice(128 * j, 128 * (j + 1))
            nc.any.tensor_scalar(out=aAn[:, csl], in0=aA[:, csl],
                                 scalar1=sc2E[:, smp: smp + 1],
                                 scalar2=sf2E[:, smp: smp + 1],
                                 op0=ALU.mult, op1=ALU.add)
            nc.any.tensor_scalar(out=aOn[:, csl], in0=attH[:, sl][:, csl],
                                 scalar1=sc2O[:, smp: smp + 1],
                                 scalar2=sf2O[:, smp: smp + 1],
                                 op0=ALU.mult, op1=ALU.add)
        po = ps.tile([128, 512], F32, tag="a", name="po")
        nc.tensor.matmul(po[:], cst["woE"][:], aAn[:], start=True, stop=False)
        nc.tensor.matmul(po[:], cst["woO"][:], aOn[:], start=False, stop=True)
        yb = scr.tile([128, 512], BF16, tag="sqq", name="yb", bufs=3)
        nc.any.tensor_scalar(out=yb[:], in0=po[:], scalar1=cst["b_o"][:, 0:1],
                             scalar2=cst["gamma"][:, 0:1], op0=ALU.add, op1=ALU.mult)
        xr = scr.tile([128, 512], F32, tag="ld", name="xr")
        b_, t0 = (4 * n) // 64, (4 * n) % 64
        nc.sync.dma_start(
            xr[:].rearrange("p (j c) -> p j c", j=4),
            xb[b_, t0: t0 + 4].rearrange("j p c -> p j c"))
        y8 = scr.tile([128, 512], F32, tag="y8f", name="y8")
        for j in range(4):
            pt = ps.tile([128, 128], BF16, tag="a", name="tp2")
            nc.tensor.transpose(pt[:], yb[:, 128 * j: 128 * (j + 1)], cst["identb"][:])
            nc.any.tensor_tensor(out=y8[:, 128 * j: 128 * (j + 1)], in0=pt[:],
                                 in1=xr[:, 128 * j: 128 * (j + 1)], op=ALU.add)
        nc.sync.dma_start(
            yb4[b_, t0: t0 + 4].rearrange("j p c -> p j c"),
            y8[:].rearrange("p (j c) -> p j c", j=4))

    for p_ in reversed(list(pools.values())):
        p_.release()


# ---- public entry point -------------------------------------------------
_NC = None


def _get_nc():
    global _NC
    if _NC is None:
        _NC = build_nc(ln_general=True)
    return _NC


def kernel(**inputs) -> np.ndarray:
    import ml_dtypes
    from concourse import bass_utils

    nc = _get_nc()
    consts = host_prep(inputs)
    cmap = {}
    for name, val in consts.items():
        v = np.asarray(val, np.float32)
        if name in _BF16_IN:
            v = v.astype(ml_dtypes.bfloat16)
        cmap[name] = v
    x = np.asarray(inputs["x"], np.float32)
    in_maps = []
    for c in range(NCORES):
        m = dict(cmap)
        m["x"] = np.ascontiguousarray(
            x[:, :, HL * c: HL * (c + 1), :, :].reshape(-1, C))
        in_maps.append(m)
    res = bass_utils.run_bass_kernel_spmd(
        nc, in_maps, core_ids=list(range(NCORES)), trace=False)
    y = np.zeros((T, B, H, W, C), np.float32)
    for c in range(NCORES):
        y[:, :, HL * c: HL * (c + 1), :, :] = \
            res.results[c]["y"].reshape(T, B, HL, W, C)
    return y


def kernel_traced(**inputs):
    """Like kernel() but returns (y, per_core_exec_ns, trace_path)."""
    import ml_dtypes
    from concourse import bass_utils

    nc = _get_nc()
    consts = host_prep(inputs)
    cmap = {}
    for name, val in consts.items():
        v = np.asarray(val, np.float32)
        if name in _BF16_IN:
            v = v.astype(ml_dtypes.bfloat16)
        cmap[name] = v
    x = np.asarray(inputs["x"], np.float32)
    in_maps = []
    for c in range(NCORES):
        m = dict(cmap)
        m["x"] = np.ascontiguousarray(
            x[:, :, HL * c: HL * (c + 1), :, :].reshape(-1, C))
        in_maps.append(m)
    res = bass_utils.run_bass_kernel_spmd(
        nc, in_maps, core_ids=list(range(NCORES)),
        trace=True, trace_cores=list(range(NCORES)))
    y = np.zeros((T, B, H, W, C), np.float32)
    for c in range(NCORES):
        y[:, :, HL * c: HL * (c + 1), :, :] = \
            res.results[c]["y"].reshape(T, B, HL, W, C)
    trace_path = (res.instructions_and_trace[1]
                  if res.instructions_and_trace else None)
    return y, res.exec_time_ns, trace_path

